# revision 39
# baseline (speedup 1.0000x reference)
"""MixerLayerKAN Trainium2 kernel.

x (B,T,C)=(32,512,512) fp32; token-mix FourierKAN(T->TD)+Linear, then
channel-mix FourierKAN(C->2C)+Linear, LN + residual around each.

Strategy (data-parallel over batch, 4 batches per NeuronCore, weights
replicated, no collectives):

* Fourier features cos(kx)/sin(kx), k=1..3, re-expressed in the product
  basis {s, c, s*c, s^2, s^3, c*s^2}; harmonic coefficients fold
  host-side into 6 effective weight matrices; the channel KAN further
  folds its post-KAN Linear (96 matmuls/batch instead of 224).
* LN1 normalize is folded into the feature chain: the range-wrap
  (round-to-int tensor_scalar + int32-input scalar_tensor_tensor, with
  per-partition scale/bias riding the scalar slots) and the Sin/Abs
  activations consume raw x directly -- no materialized normalized
  tensor and no int->float CAST on the token path.
* Transposes run in bf16 (1 cycle/row), two c-tiles packed per PSUM
  bank; the channel wrap chain reads transposed values straight from
  PSUM (no PSUM->SBUF copy ops).
* Each output tile is two DVE adds straight from PSUM (psum +
  residual, + channel bias) -- keeping the adds off the saturated PE
  and the copies off the scalar engine.
* PE FIFO is software-pipelined: mm1 of batch b+1 is split into two
  24-matmul chunks emitted around batch b's transposes, covering the
  two serial handoffs (LN2 chain, channel feature chain); mm3 runs
  q-major so the output adds overlap it; output DMAs trail one
  iteration so they never head-of-line-block the DVE FIFO.
* DMA: per-queue ring bandwidth is ~46GB/s, so batch-0's x tiles and
  the first token weights load as small split DMAs across rings, x is
  prefetched two batches ahead (before the big wchf load), and the
  final stores split across rings to shorten the drain.
* The last batch (no next-batch mm1 cover) runs its LN2 pair-wise with
  transposes interleaved to shorten the uncovered serial chain.

Measured ~194us/core unthrottled (~233us baseline); note the part
power-throttles under sustained load, adding up to ~40us run-to-run.
"""

import numpy as np
import ml_dtypes

import concourse.bass as bass
import concourse.mybir as mybir
from concourse import tile
from concourse.bass_utils import run_bass_kernel_spmd
from concourse.masks import make_identity

AF = mybir.ActivationFunctionType
OP = mybir.AluOpType
FP32 = mybir.dt.float32
BF16 = mybir.dt.bfloat16
I32 = mybir.dt.int32

B, T, C, TD, G = 32, 512, 512, 256, 3
NCORES = 8
NB = B // NCORES          # batches per core
P = 128
EPS = 1e-5
PI = float(np.pi)
TWO_PI = float(2 * np.pi)
INV_2PI = float(1.0 / (2 * np.pi))
FOUR_PI = float(4 * np.pi)
NF = 6                    # product-basis features
NT = T // P               # 4 t-tiles
NC_ = C // P              # 4 c-tiles
NO_TOK = TD // P          # 2 token KAN hidden tiles


def _split_multi_waits(nc):
    """This walrus build accepts at most ONE sync-wait command per
    instruction.  Tile emits several.  Fix: before each multi-wait
    instruction, splice in same-engine NOPs carrying one wait each (a wait
    executed earlier on the same engine is semantically identical)."""
    f = nc.m.functions[0]
    per_engine = {}
    for bb in f.blocks:
        for inst in bb.instructions:
            si = getattr(inst, "sync_info", None)
            if si is not None and si.on_wait and len(si.on_wait) > 1:
                per_engine[inst.engine] = per_engine.get(inst.engine, 0) + (
                    len(si.on_wait) - 1)
    if not per_engine:
        return
    nop_pool = {}
    for eng, cnt in per_engine.items():
        nop_pool[eng] = [nc.engines[eng].nop(nofuse=True).ins for _ in range(cnt)]
    created = {id(i) for h in nop_pool.values() for i in h}
    for bb in f.blocks:
        bb.instructions[:] = [i for i in bb.instructions if id(i) not in created]
    for bb in f.blocks:
        out = []
        for inst in bb.instructions:
            si = getattr(inst, "sync_info", None)
            if si is not None and si.on_wait and len(si.on_wait) > 1:
                waits = list(si.on_wait)
                si.on_wait = [waits[-1]]
                for w in waits[:-1]:
                    nop = nop_pool[inst.engine].pop()
                    nop.sync_info = mybir.SyncInfo(on_wait=[w], on_update=[])
                    out.append(nop)
            out.append(inst)
        bb.instructions[:] = out


def _cheb_weights(coef):
    """coef (2, O, I, G) -> effective basis weights (I, 6, O) for the
    {s, c, s*c, s^2, s^3, c*s^2} basis, plus the constant term (O,).

    cos(1x)=c; cos(2x)=1-2s^2; cos(3x)=c-4c s^2
    sin(1x)=s; sin(2x)=2 s c ; sin(3x)=3s-4s^3
    """
    cosw = coef[0]
    sinw = coef[1]
    O, I, _ = cosw.shape
    w = np.zeros((I, NF, O), np.float64)
    w[:, 0, :] = (sinw[:, :, 0] + 3.0 * sinw[:, :, 2]).T      # s
    w[:, 1, :] = (cosw[:, :, 0] + cosw[:, :, 2]).T            # c
    w[:, 2, :] = (2.0 * sinw[:, :, 1]).T                      # s*c
    w[:, 3, :] = (-2.0 * cosw[:, :, 1]).T                     # s^2
    w[:, 4, :] = (-4.0 * sinw[:, :, 2]).T                     # s^3
    w[:, 5, :] = (-4.0 * cosw[:, :, 2]).T                     # c*s^2
    const = cosw[:, :, 1].sum(axis=1)                         # from the "1" of cos(2x)
    return w, const


def _build(apply_ln1, apply_ln2):
    nc = bass.Bass()
    x_in = nc.dram_tensor("x", [NB, T, C], FP32, kind="ExternalInput")
    y_out = nc.dram_tensor("y", [NB, T, C], FP32, kind="ExternalOutput")
    wtok_in = nc.dram_tensor("wtok", [NT, P, NF * TD], BF16, kind="ExternalInput")
    wchf_in = nc.dram_tensor("wchf", [NC_, P, NF * C], BF16, kind="ExternalInput")
    tlw_in = nc.dram_tensor("tlw", [NO_TOK, P, T], BF16, kind="ExternalInput")
    btok_in = nc.dram_tensor("btok", [P, NT], FP32, kind="ExternalInput")
    bch_in = nc.dram_tensor("bch", [P, C], BF16, kind="ExternalInput")
    ln_in = nc.dram_tensor("lnwb", [P, 4 * C], FP32, kind="ExternalInput")

    with tile.TileContext(nc) as tc, \
         tc.tile_pool(name="singles", bufs=1) as singles, \
         tc.tile_pool(name="xpool", bufs=3) as xpool, \
         tc.tile_pool(name="fpool", bufs=2) as fpool, \
         tc.tile_pool(name="f2pool", bufs=1) as f2pool, \
         tc.tile_pool(name="scratch", bufs=1) as scratch, \
         tc.tile_pool(name="ypool", bufs=2) as ypool, \
         tc.tile_pool(name="x1pool", bufs=2) as x1pool, \
         tc.tile_pool(name="opool", bufs=8) as opool, \
         tc.tile_pool(name="stats", bufs=2) as stats, \
         tc.tile_pool(name="ptokp", bufs=2, space="PSUM") as ptokp, \
         tc.tile_pool(name="pza", bufs=2, space="PSUM") as pza, \
         tc.tile_pool(name="trpz", bufs=2, space="PSUM") as trpz, \
         tc.tile_pool(name="poutp", bufs=2, space="PSUM") as poutp:

        # ---- batch-0 x first so the big weight DMAs don't block start ----
        def load_x(b, nsplit=2):
            xt = xpool.tile([P, NT, C], FP32, tag="X", name=f"X{b}")
            step = NT // nsplit
            for i in range(0, NT, step):
                nc.sync.dma_start(
                    out=xt[:, i:i + step, :],
                    in_=x_in[b, i * P:(i + step) * P, :].rearrange(
                        "(i p) c -> p i c", p=P))
            return [xt[:, i, :] for i in range(NT)]

        # batch-0: per-tile DMAs interleaved with per-tile wtok loads so
        # tile-0 stats and the first mm1 weights arrive ASAP
        X0t = xpool.tile([P, NT, C], FP32, tag="X", name="X0")
        wtok_all = singles.tile([P, NT, NF, TD], BF16, tag="wtok")
        H = C // 2
        for i in (0, 1):
            for hh in (0, 1):
                nc.sync.dma_start(out=X0t[:, i, hh * H:(hh + 1) * H],
                                  in_=x_in[0, i * P:(i + 1) * P, hh * H:(hh + 1) * H])
        wtok0r = wtok_in[0].rearrange("p (f o) -> p f o", f=NF)
        nc.sync.dma_start(out=wtok_all[:, 0, 0:NF // 2], in_=wtok0r[:, 0:NF // 2])
        nc.sync.dma_start(out=wtok_all[:, 0, NF // 2:], in_=wtok0r[:, NF // 2:])
        for i in (2, 3):
            nc.sync.dma_start(out=X0t[:, i, :], in_=x_in[0, i * P:(i + 1) * P, :])
        for i in range(1, NT):
            nc.sync.dma_start(out=wtok_all[:, i],
                              in_=wtok_in[i].rearrange("p (f o) -> p f o", f=NF))
        X1 = load_x(1) if NB > 1 else None
        X0 = [X0t[:, i, :] for i in range(NT)]
        wtok = [wtok_all[:, i] for i in range(NT)]

        ident = singles.tile([P, P], BF16, tag="ident")
        make_identity(nc, ident)
        ones128 = singles.tile([P, P], BF16, tag="ones128")
        nc.vector.memset(ones128, float(1.0 / 128.0))
        halfpi = singles.tile([P, 1], FP32, tag="halfpi")
        nc.vector.memset(halfpi, PI / 2)
        actwarm = singles.tile([P, 1], FP32, tag="actwarm")
        nc.scalar.activation(out=actwarm, in_=halfpi, func=AF.Sin)
        tlw = []
        for j in range(NO_TOK):
            t_ = singles.tile([P, T], BF16, tag=f"tlw{j}")
            nc.sync.dma_start(out=t_, in_=tlw_in[j])
            tlw.append(t_)
        btok = singles.tile([P, NT], FP32, tag="btok")
        nc.sync.dma_start(out=btok, in_=btok_in[:, :])

        def load_weights_late():
            wchf_all = singles.tile([P, NC_, NF, C], BF16, tag="wchf")
            nc.sync.dma_start(out=wchf_all,
                              in_=wchf_in.rearrange("m p (f o) -> p m f o", f=NF))
            wchf = [wchf_all[:, m] for m in range(NC_)]
            bch = singles.tile([P, C], BF16, tag="bch")
            nc.sync.dma_start(out=bch, in_=bch_in[:, :])
            lnwb = None
            if apply_ln2 and not apply_ln1:
                lnwb = singles.tile([P, 4, C], FP32, tag="lnwb")
                nc.sync.dma_start(out=lnwb, in_=ln_in.rearrange("p (k c) -> p k c", k=4))
            return wchf, bch, lnwb

        # ---- helpers ----
        def ln1_stats(X, tiles, gkey):
            """bn stats + rsqrt Newton + derived wrap scalars for a group of
            tiles.  Returns {tile: (rstd, nb, aa, bv, cc2) [P,1] slices}."""
            n = len(tiles)
            mvs = stats.tile([P, n, 2], FP32, tag=f"mvs{gkey}", name="mvs")
            for k, i in enumerate(tiles):
                st6 = stats.tile([P, 6], FP32, tag=f"st6_{i % 2}", name="st6")
                nc.vector.bn_stats(out=st6, in_=X[i])
                nc.vector.bn_aggr(out=mvs[:, k, :], in_=st6)
            mean = mvs[:, :, 0]
            var = mvs[:, :, 1]
            h = stats.tile([P, n], FP32, tag=f"h{gkey}", name="h")
            nc.vector.tensor_scalar(out=h, in0=var, scalar1=EPS, scalar2=-0.5,
                                    op0=OP.add, op1=OP.mult)
            yi = stats.tile([P, n], I32, tag=f"yi{gkey}", name="yi")
            nc.vector.tensor_scalar(out=yi, in0=var.bitcast(I32), scalar1=1,
                                    scalar2=None, op0=OP.logical_shift_right)
            nc.vector.tensor_scalar(out=yi, in0=yi, scalar1=-1,
                                    scalar2=0x5F3759DF, op0=OP.mult, op1=OP.add)
            rstd = yi.bitcast(FP32)
            a2 = stats.tile([P, n], FP32, tag=f"a2{gkey}", name="a2")
            for _ in range(2):
                nc.vector.tensor_tensor(out=a2, in0=rstd, in1=rstd, op=OP.mult)
                nc.vector.tensor_tensor(out=a2, in0=a2, in1=h, op=OP.mult)
                nc.vector.scalar_tensor_tensor(out=rstd, in0=a2, scalar=1.5,
                                               in1=rstd, op0=OP.add, op1=OP.mult)
            nb = stats.tile([P, n], FP32, tag=f"nb{gkey}", name="nb")
            nc.vector.scalar_tensor_tensor(out=nb, in0=mean, scalar=-1.0, in1=rstd,
                                           op0=OP.mult, op1=OP.mult)
            aa = stats.tile([P, n], FP32, tag=f"aa{gkey}", name="aa")
            nc.vector.tensor_scalar(out=aa, in0=rstd, scalar1=INV_2PI, scalar2=None,
                                    op0=OP.mult)
            bv = stats.tile([P, n], FP32, tag=f"bv{gkey}", name="bv")
            nc.vector.tensor_scalar(out=bv, in0=nb, scalar1=INV_2PI, scalar2=None,
                                    op0=OP.mult)
            cc2 = stats.tile([P, n], FP32, tag=f"cc2{gkey}", name="cc2")
            nc.vector.scalar_tensor_tensor(out=cc2, in0=h, scalar=FOUR_PI, in1=rstd,
                                           op0=OP.mult, op1=OP.mult)
            return {i: tuple(t[:, k:k + 1] for t in (rstd, nb, aa, bv, cc2))
                    for k, i in enumerate(tiles)}

        def feat_tiles(pool, pref, i):
            return [pool.tile([P, C], BF16, tag=f"{pref}_{i}_{k}", name=f"{pref}{i}b{k}")
                    for k in range(NF)]

        def features_from_x(xt, i, rstd, nb, aa, bv, cc2, pref):
            """Token-path features straight from raw x (LN folded in)."""
            if apply_ln1:
                # general path: materialize normalized tensor, then wrap
                xn = scratch.tile([P, C], FP32, tag=f"xn{i % 2}", name="xn")
                nc.scalar.activation(out=xn, in_=xt, func=AF.Identity,
                                     bias=nb, scale=rstd)
                nc.vector.tensor_mul(out=xn, in0=xn, in1=lnwb[:, 0, :])
                nc.vector.tensor_add(out=xn, in0=xn, in1=lnwb[:, 1, :])
                return features_from_norm(xn, f"F1_{i}", fpool, pref, i)
            f = feat_tiles(fpool, pref, i)
            ni = scratch.tile([P, C], I32, tag=f"ni1_{i % 2}", name="ni")
            nc.vector.tensor_scalar(out=ni, in0=xt, scalar1=aa,
                                    scalar2=bv, op0=OP.mult, op1=OP.add)
            rt = scratch.tile([P, C], FP32, tag=f"rt1_{i % 2}", name="rt")
            nc.vector.scalar_tensor_tensor(out=rt, in0=ni, scalar=cc2,
                                           in1=xt, op0=OP.mult, op1=OP.add)
            nc.scalar.activation(out=f[0], in_=rt, func=AF.Sin,
                                 scale=rstd, bias=nb)
            ab = scratch.tile([P, C], FP32, tag=f"ab1_{i % 2}", name="ab")
            nc.scalar.activation(out=ab, in_=rt, func=AF.Abs,
                                 scale=rstd, bias=nb)
            nc.scalar.activation(out=f[1], in_=ab, func=AF.Sin, scale=-1.0,
                                 bias=halfpi[:, :])
            nc.vector.tensor_mul(out=f[3], in0=f[0], in1=f[0])   # ss
            nc.vector.tensor_mul(out=f[2], in0=f[0], in1=f[1])   # sc
            nc.vector.tensor_mul(out=f[4], in0=f[3], in1=f[0])   # sss
            nc.vector.tensor_mul(out=f[5], in0=f[3], in1=f[1])   # css
            return f

        def features_from_norm(src, key, pool, pref, i):
            """Channel-path features from an already-normalized source
            (SBUF tile or PSUM transpose slice)."""
            f = feat_tiles(pool, pref, i)
            ni = scratch.tile([P, C], I32, tag=f"ni_{key}" if apply_ln1 else f"ni2_{i % 2}",
                             name="ni")
            nc.vector.tensor_scalar(out=ni, in0=src, scalar1=INV_2PI, scalar2=None,
                                    op0=OP.mult)
            rt = scratch.tile([P, C], FP32, tag=f"rt_{key}" if apply_ln1 else f"rt2_{i % 2}",
                             name="rt")
            nc.vector.scalar_tensor_tensor(out=rt, in0=ni, scalar=-TWO_PI,
                                           in1=src, op0=OP.mult, op1=OP.add)
            nc.scalar.activation(out=f[0], in_=rt, func=AF.Sin)
            ab = scratch.tile([P, C], FP32, tag=f"ab_{key}" if apply_ln1 else f"ab2_{i % 2}",
                             name="ab")
            nc.scalar.activation(out=ab, in_=rt, func=AF.Abs)
            nc.scalar.activation(out=f[1], in_=ab, func=AF.Sin, scale=-1.0,
                                 bias=halfpi[:, :])
            nc.vector.tensor_mul(out=f[3], in0=f[0], in1=f[0])
            nc.vector.tensor_mul(out=f[2], in0=f[0], in1=f[1])
            nc.vector.tensor_mul(out=f[4], in0=f[3], in1=f[0])
            nc.vector.tensor_mul(out=f[5], in0=f[3], in1=f[1])
            return f

        def stage1_prefetch(X, gsize=NT):
            """LN1 + token features for a batch whose x is already loading.
            Small group sizes start tile-0's feature chain earlier (used
            during the DMA-bound fill)."""
            groups = [tuple(range(g, g + gsize)) for g in range(0, NT, gsize)]
            feats = [None] * NT
            for g, tiles in enumerate(groups):
                sc = ln1_stats(X, tiles, f"{len(tiles)}_{g % 2}")
                for i in tiles:
                    feats[i] = features_from_x(X[i], i, *sc[i], "F1")
            return feats

        def mm1(feats, ptok, tiles):
            for i in tiles:
                for j in range(NO_TOK):
                    for f in range(NF):
                        nc.tensor.matmul(ptok[j], wtok[i][:, f, j * P:(j + 1) * P],
                                         feats[i][f], start=(i == 0 and f == 0),
                                         stop=(i == NT - 1 and f == NF - 1))

        def ln2_chain(s1, e2, n, gkey):
            """LN2 rsqrt chain on DVE (latency-critical for the transposes).
            s1/e2 [P,n] slices -> (rstd2, nm2) [P,n]."""
            mn = stats.tile([P, n], FP32, tag=f"mn{gkey}", name="mn")
            nc.vector.tensor_scalar_mul(out=mn, in0=s1, scalar1=1.0 / C)
            vr = stats.tile([P, n], FP32, tag=f"vr{gkey}", name="vr")
            nc.vector.tensor_mul(out=vr, in0=mn, in1=mn)
            nc.vector.scalar_tensor_tensor(out=vr, in0=e2, scalar=1.0 / C, in1=vr,
                                           op0=OP.mult, op1=OP.subtract)
            h2 = stats.tile([P, n], FP32, tag=f"h2{gkey}", name="h2")
            nc.vector.tensor_scalar(out=h2, in0=vr, scalar1=EPS, scalar2=-0.5,
                                    op0=OP.add, op1=OP.mult)
            yi2 = stats.tile([P, n], I32, tag=f"yi2{gkey}", name="yi2")
            nc.vector.tensor_scalar(out=yi2, in0=vr.bitcast(I32), scalar1=1,
                                    scalar2=None, op0=OP.logical_shift_right)
            nc.vector.tensor_scalar(out=yi2, in0=yi2, scalar1=-1,
                                    scalar2=0x5F3759DF, op0=OP.mult, op1=OP.add)
            rstd2 = yi2.bitcast(FP32)
            a2 = stats.tile([P, n], FP32, tag=f"a2b{gkey}", name="a2b")
            for _ in range(2):
                nc.vector.tensor_mul(out=a2, in0=rstd2, in1=rstd2)
                nc.vector.tensor_mul(out=a2, in0=a2, in1=h2)
                nc.vector.scalar_tensor_tensor(out=rstd2, in0=a2, scalar=1.5,
                                               in1=rstd2, op0=OP.add, op1=OP.mult)
            nm2 = stats.tile([P, n], FP32, tag=f"nm2{gkey}", name="nm2")
            nc.vector.scalar_tensor_tensor(out=nm2, in0=mn, scalar=-1.0,
                                           in1=rstd2, op0=OP.mult, op1=OP.mult)
            return rstd2, nm2

        def mm2_resid(q, X, ytok, s1, e2, pz_pool):
            pz = pz_pool.tile([P, C], FP32, tag="pz", name="pz")
            for j in range(NO_TOK):
                nc.tensor.matmul(pz, tlw[j][:, q * P:(q + 1) * P], ytok[j],
                                 start=(j == 0), stop=(j == NO_TOK - 1))
            xt = x1pool.tile([P, C], BF16, tag=f"x1_{q}", name=f"x1_{q}")
            nc.vector.scalar_tensor_tensor(out=xt, in0=pz,
                                           scalar=btok[:, q:q + 1],
                                           in1=X[q], op0=OP.add, op1=OP.add,
                                           accum_out=s1[:, q:q + 1])
            sq = scratch.tile([P, C], FP32, tag="sq", name="sq")
            nc.vector.scalar_tensor_tensor(out=sq, in0=xt, scalar=1.0, in1=xt,
                                           op0=OP.mult, op1=OP.mult,
                                           accum_out=e2[:, q:q + 1])
            return xt

        def normalize_xn2(q, x1q, rstd2, nm2, k):
            xq = ypool.tile([P, C], BF16, tag=f"xn2_{q}", name=f"xn2_{q}", bufs=1)
            if apply_ln2:
                tmp = scratch.tile([P, C], FP32, tag="lntmp", name="lntmp")
                nc.scalar.activation(out=tmp, in_=x1q, func=AF.Identity,
                                     bias=nm2[:, k:k + 1], scale=rstd2[:, k:k + 1])
                nc.vector.tensor_mul(out=tmp, in0=tmp, in1=lnwb[:, 2, :])
                nc.vector.scalar_tensor_tensor(out=xq, in0=tmp, scalar=1.0,
                                               in1=lnwb[:, 3, :], op0=OP.mult,
                                               op1=OP.add)
            else:
                nc.scalar.activation(out=xq, in_=x1q, func=AF.Identity,
                                     bias=nm2[:, k:k + 1], scale=rstd2[:, k:k + 1])
            return xq

        def alloc_tr():
            # c-tiles m packed 2 per PSUM bank (the fp32 bank tile is viewed
            # as [P, 2C] bf16 so the tag matches the pz allocations rotating
            # through the same 2 banks)
            return [trpz.tile([P, C], FP32, tag="pz", name=f"tr{h}").bitcast(BF16)
                    for h in range(2)]

        def transpose_i(tr, xn2i, i):
            for m in range(NC_):
                nc.tensor.transpose(
                    tr[m // 2][:, (m % 2) * C + i * P:(m % 2) * C + (i + 1) * P],
                    xn2i[:, m * P:(m + 1) * P], ident)

        def ytok_copies(ptok):
            ytok = []
            for j in range(NO_TOK):
                ysb = ypool.tile([P, C], BF16, tag=f"ytok{j}", name="ysb")
                nc.scalar.copy(out=ysb, in_=ptok[j])
                ytok.append(ysb)
            return ytok

        def token_out(b, X, ytok):
            """mm2, residual+LN2 stats, normalize -> xn2."""
            s1 = stats.tile([P, NT], FP32, tag="s1", name="s1")
            e2 = stats.tile([P, NT], FP32, tag="e2", name="e2")
            x1 = [mm2_resid(q, X, ytok, s1, e2, pza if q < 2 else trpz)
                  for q in range(NT)]
            rstd2, nm2 = ln2_chain(s1, e2, NT, "")
            xn2 = [normalize_xn2(q, x1[q], rstd2, nm2, q) for q in range(NT)]
            return x1, xn2

        def token_out_last(b, X, ytok):
            """Last batch: no next-batch mm1 to cover the LN2 chain, so
            process pair-wise and interleave the transposes."""
            s1 = stats.tile([P, NT], FP32, tag="s1", name="s1")
            e2 = stats.tile([P, NT], FP32, tag="e2", name="e2")
            tr = alloc_tr()
            x1 = []
            for g in range(2):
                qs = (2 * g, 2 * g + 1)
                for q in qs:
                    x1.append(mm2_resid(q, X, ytok, s1, e2, pza))
                rstd2, nm2 = ln2_chain(s1[:, 2 * g:2 * g + 2],
                                       e2[:, 2 * g:2 * g + 2], 2, f"p{g}")
                for k, q in enumerate(qs):
                    xq = normalize_xn2(q, x1[q], rstd2, nm2, k)
                    transpose_i(tr, xq, q)
            return x1, tr

        def transposes(xn2):
            tr = alloc_tr()
            for i in range(NT):
                transpose_i(tr, xn2[i], i)
            return tr

        def channel_feats(tr):
            return [features_from_norm(tr[m // 2][:, (m % 2) * C:(m % 2 + 1) * C],
                                       f"c{m}", f2pool, "F2", m)
                    for m in range(NC_)]

        def mm3(fch):
            """q-major channel matmuls with the bias folded in via a
            ones-matmul; returns the 4 PSUM tiles."""
            pouts = []
            for q in range(NT):
                pout = poutp.tile([P, C], FP32, tag="pout", name=f"pout{q}")
                for m in range(NC_):
                    for f in range(NF):
                        nc.tensor.matmul(pout, fch[m][f][:, q * P:(q + 1) * P],
                                         wchf[m][:, f, :],
                                         start=(m == 0 and f == 0),
                                         stop=(m == NC_ - 1 and f == NF - 1))
                pouts.append(pout)
            return pouts

        def emit_out(b, pouts, x1, final=False):
            """residual add on DVE straight from PSUM, then store.  Emitted
            one iteration late so it never head-of-line-blocks the next
            batch's feature chain on the DVE FIFO.  For the final batch the
            stores split across queues so the tail transfer isn't bound by
            one ring's bandwidth."""
            for q in range(NT):
                ot = opool.tile([P, C], FP32, tag="out", name="out")
                nc.vector.tensor_tensor(out=ot, in0=pouts[q], in1=x1[q],
                                        op=OP.add)
                nc.vector.tensor_tensor(out=ot, in0=ot, in1=bch, op=OP.add)
                if final and q >= 2:
                    for hh in range(4):
                        nc.sync.dma_start(
                            out=y_out[b, q * P:(q + 1) * P,
                                      hh * (C // 4):(hh + 1) * (C // 4)],
                            in_=ot[:, hh * (C // 4):(hh + 1) * (C // 4)])
                else:
                    nc.sync.dma_start(out=y_out[b, q * P:(q + 1) * P, :], in_=ot)

        # ---- software-pipelined emission over batches ----
        # DMA order: X(0), wtok, [x(1)], then the big wchf -- the fill is
        # HBM-bandwidth-bound, so batch-0's dependencies go first.
        if apply_ln1:
            lnwb = singles.tile([P, 4, C], FP32, tag="lnwb")
            nc.sync.dma_start(out=lnwb, in_=ln_in.rearrange("p (k c) -> p k c", k=4))
        Xc, featsc = X0, stage1_prefetch(X0, gsize=1)
        Xmap = {0: X0}
        if NB > 1:
            Xmap[1] = X1
        if NB > 2:
            Xmap[2] = load_x(2)
        wchf, bch, lnwb2 = load_weights_late()
        if not apply_ln1:
            lnwb = lnwb2
        ptokc = [ptokp.tile([P, C], FP32, tag="ptok", name=f"ptok{j}")
                 for j in range(NO_TOK)]
        mm1(featsc, ptokc, range(NT))
        pending_out = None
        for b in range(NB):
            X, feats, ptok = Xc, featsc, ptokc
            # b=0: the next batch's Sin chain goes ahead of the ytok copies
            # on the ACT queue -- the copies wait for mm1(0) anyway, while
            # the sins' inputs are ready earlier (kills the ramp gaps).
            # Steady state: copies first (they gate mm2 on the PE; the sins'
            # inputs arrive early under the mm3 cover).
            if b == 0 and b + 1 < NB:
                Xc = Xmap[b + 1]
                featsc = stage1_prefetch(Xc, gsize=2)
                ytok = ytok_copies(ptok)
                if b + 2 < NB and (b + 2) not in Xmap:
                    Xmap[b + 2] = load_x(b + 2)
            else:
                ytok = ytok_copies(ptok)
                if b + 1 < NB:
                    Xc = Xmap[b + 1]
                    featsc = stage1_prefetch(Xc, gsize=NT)
                    if b + 2 < NB and (b + 2) not in Xmap:
                        Xmap[b + 2] = load_x(b + 2)
            if b + 1 < NB:
                x1, xn2 = token_out(b, X, ytok)
                if pending_out is not None:
                    emit_out(b - 1, *pending_out)
                ptokc = [ptokp.tile([P, C], FP32, tag="ptok", name=f"ptok{j}")
                         for j in range(NO_TOK)]
                mm1(featsc, ptokc, (0, 1))
                tr = transposes(xn2)
                mm1(featsc, ptokc, (2, 3))
            else:
                x1, tr = token_out_last(b, X, ytok)
                if pending_out is not None:
                    emit_out(b - 1, *pending_out)
            fch = channel_feats(tr)
            pending_out = (mm3(fch), x1)
        emit_out(NB - 1, *pending_out, final=True)

    _split_multi_waits(nc)
    return nc


_CACHE = {}


def _get_nc(apply_ln1, apply_ln2):
    key = (apply_ln1, apply_ln2)
    if key not in _CACHE:
        _CACHE[key] = _build(apply_ln1, apply_ln2)
    return _CACHE[key]


def prepare_in_maps(inputs):
    return _prepare(**inputs)


def _prepare(x, ln1_w, ln1_b, tok_coef, tok_kbias, tok_lw, tok_lb,
             ln2_w, ln2_b, ch_coef, ch_kbias, ch_lw, ch_lb):
    x = np.asarray(x, np.float32)
    f64 = np.float64

    wtok_eff, tok_const = _cheb_weights(np.asarray(tok_coef, f64))  # (T,6,TD)
    wch_eff, ch_const = _cheb_weights(np.asarray(ch_coef, f64))     # (C,6,2C)

    kbias_tok = np.asarray(tok_kbias, f64).reshape(-1) + tok_const
    kbias_ch = np.asarray(ch_kbias, f64).reshape(-1) + ch_const
    bias_tok = np.asarray(tok_lb, f64) + np.asarray(tok_lw, f64) @ kbias_tok
    bias_ch = np.asarray(ch_lb, f64) + np.asarray(ch_lw, f64) @ kbias_ch

    # fold the channel post-KAN linear into the KAN weights (fp64)
    wchf = np.einsum("cfo,ko->cfk", wch_eff, np.asarray(ch_lw, f64))  # (C,6,C)

    wtok_np = wtok_eff.reshape(NT, P, NF * TD).astype(ml_dtypes.bfloat16)
    wchf_np = wchf.reshape(NC_, P, NF * C).astype(ml_dtypes.bfloat16)
    tlw_np = np.ascontiguousarray(np.asarray(tok_lw, f64).T).reshape(
        NO_TOK, P, T).astype(ml_dtypes.bfloat16)
    btok_np = np.ascontiguousarray(bias_tok.reshape(NT, P).T).astype(np.float32)
    bch_np = np.broadcast_to(bias_ch.astype(ml_dtypes.bfloat16), (P, C)).copy()
    lnwb_np = np.broadcast_to(
        np.concatenate([np.asarray(ln1_w, f64), np.asarray(ln1_b, f64),
                        np.asarray(ln2_w, f64), np.asarray(ln2_b, f64)]).astype(
            np.float32), (P, 4 * C)).copy()

    apply_ln1 = not (np.all(np.asarray(ln1_w) == 1.0) and np.all(np.asarray(ln1_b) == 0.0))
    apply_ln2 = not (np.all(np.asarray(ln2_w) == 1.0) and np.all(np.asarray(ln2_b) == 0.0))

    shared = dict(wtok=wtok_np, wchf=wchf_np, tlw=tlw_np,
                  btok=btok_np, bch=bch_np, lnwb=lnwb_np)
    in_maps = []
    for core in range(NCORES):
        m = dict(shared)
        m["x"] = np.ascontiguousarray(x[core * NB:(core + 1) * NB])
        in_maps.append(m)
    return {"build_key": (apply_ln1, apply_ln2), "in_maps": in_maps}


def kernel(**inputs):
    prep = _prepare(**inputs)
    nc = _get_nc(*prep["build_key"])
    res = run_bass_kernel_spmd(nc, prep["in_maps"], list(range(NCORES)))
    return np.concatenate([res.results[i]["y"] for i in range(NCORES)], axis=0)


# revision 40
# speedup vs baseline: 1.0218x; 1.0218x over previous
"""MixerLayerKAN Trainium2 kernel.

x (B,T,C)=(32,512,512) fp32; token-mix FourierKAN(T->TD)+Linear, then
channel-mix FourierKAN(C->2C)+Linear, LN + residual around each.

Strategy (data-parallel over batch, 4 batches per NeuronCore, weights
replicated, no collectives):

* Fourier features cos(kx)/sin(kx), k=1..3, re-expressed in the product
  basis {s, c, s*c, s^2, s^3, c*s^2}; harmonic coefficients fold
  host-side into 6 effective weight matrices; the channel KAN further
  folds its post-KAN Linear (96 matmuls/batch instead of 224).
* LN1 normalize is folded into the feature chain: the range-wrap
  (round-to-int tensor_scalar + int32-input scalar_tensor_tensor, with
  per-partition scale/bias riding the scalar slots) and the Sin/Abs
  activations consume raw x directly -- no materialized normalized
  tensor and no int->float CAST on the token path.
* Transposes run in bf16 (1 cycle/row), two c-tiles packed per PSUM
  bank; the channel wrap chain reads transposed values straight from
  PSUM (no PSUM->SBUF copy ops).
* Each output tile is two DVE adds straight from PSUM (psum +
  residual, + channel bias) -- keeping the adds off the saturated PE
  and the copies off the scalar engine.
* PE FIFO is software-pipelined: mm1 of batch b+1 is split into two
  24-matmul chunks emitted around batch b's transposes, covering the
  two serial handoffs (LN2 chain, channel feature chain); mm3 runs
  q-major so the output adds overlap it; output DMAs trail one
  iteration so they never head-of-line-block the DVE FIFO.
* DMA: per-queue ring bandwidth is ~46GB/s, so batch-0's x tiles and
  the first token weights load as small split DMAs across rings, x is
  prefetched two batches ahead (before the big wchf load), and the
  final stores split across rings to shorten the drain.
* The last batch (no next-batch mm1 cover) runs its LN2 pair-wise with
  transposes interleaved to shorten the uncovered serial chain.

Measured ~194us/core unthrottled (~233us baseline); note the part
power-throttles under sustained load, adding up to ~40us run-to-run.
"""

import numpy as np
import ml_dtypes

import concourse.bass as bass
import concourse.mybir as mybir
from concourse import tile
from concourse.bass_utils import run_bass_kernel_spmd
from concourse.masks import make_identity

AF = mybir.ActivationFunctionType
OP = mybir.AluOpType
FP32 = mybir.dt.float32
BF16 = mybir.dt.bfloat16
I32 = mybir.dt.int32

B, T, C, TD, G = 32, 512, 512, 256, 3
NCORES = 8
NB = B // NCORES          # batches per core
P = 128
EPS = 1e-5
PI = float(np.pi)
TWO_PI = float(2 * np.pi)
INV_2PI = float(1.0 / (2 * np.pi))
FOUR_PI = float(4 * np.pi)
NF = 6                    # product-basis features
NT = T // P               # 4 t-tiles
NC_ = C // P              # 4 c-tiles
NO_TOK = TD // P          # 2 token KAN hidden tiles


def _split_multi_waits(nc):
    """This walrus build accepts at most ONE sync-wait command per
    instruction.  Tile emits several.  Fix: before each multi-wait
    instruction, splice in same-engine NOPs carrying one wait each (a wait
    executed earlier on the same engine is semantically identical)."""
    f = nc.m.functions[0]
    per_engine = {}
    for bb in f.blocks:
        for inst in bb.instructions:
            si = getattr(inst, "sync_info", None)
            if si is not None and si.on_wait and len(si.on_wait) > 1:
                per_engine[inst.engine] = per_engine.get(inst.engine, 0) + (
                    len(si.on_wait) - 1)
    if not per_engine:
        return
    nop_pool = {}
    for eng, cnt in per_engine.items():
        nop_pool[eng] = [nc.engines[eng].nop(nofuse=True).ins for _ in range(cnt)]
    created = {id(i) for h in nop_pool.values() for i in h}
    for bb in f.blocks:
        bb.instructions[:] = [i for i in bb.instructions if id(i) not in created]
    for bb in f.blocks:
        out = []
        for inst in bb.instructions:
            si = getattr(inst, "sync_info", None)
            if si is not None and si.on_wait and len(si.on_wait) > 1:
                waits = list(si.on_wait)
                si.on_wait = [waits[-1]]
                for w in waits[:-1]:
                    nop = nop_pool[inst.engine].pop()
                    nop.sync_info = mybir.SyncInfo(on_wait=[w], on_update=[])
                    out.append(nop)
            out.append(inst)
        bb.instructions[:] = out


def _cheb_weights(coef):
    """coef (2, O, I, G) -> effective basis weights (I, 6, O) for the
    {s, c, s*c, s^2, s^3, c*s^2} basis, plus the constant term (O,).

    cos(1x)=c; cos(2x)=1-2s^2; cos(3x)=c-4c s^2
    sin(1x)=s; sin(2x)=2 s c ; sin(3x)=3s-4s^3
    """
    cosw = coef[0]
    sinw = coef[1]
    O, I, _ = cosw.shape
    w = np.zeros((I, NF, O), np.float64)
    w[:, 0, :] = (sinw[:, :, 0] + 3.0 * sinw[:, :, 2]).T      # s
    w[:, 1, :] = (cosw[:, :, 0] + cosw[:, :, 2]).T            # c
    w[:, 2, :] = (2.0 * sinw[:, :, 1]).T                      # s*c
    w[:, 3, :] = (-2.0 * cosw[:, :, 1]).T                     # s^2
    w[:, 4, :] = (-4.0 * sinw[:, :, 2]).T                     # s^3
    w[:, 5, :] = (-4.0 * cosw[:, :, 2]).T                     # c*s^2
    const = cosw[:, :, 1].sum(axis=1)                         # from the "1" of cos(2x)
    return w, const


def _build(apply_ln1, apply_ln2):
    nc = bass.Bass()
    x_in = nc.dram_tensor("x", [NB, T, C], FP32, kind="ExternalInput")
    y_out = nc.dram_tensor("y", [NB, T, C], FP32, kind="ExternalOutput")
    wtok_in = nc.dram_tensor("wtok", [NT, P, NF * TD], BF16, kind="ExternalInput")
    wchf_in = nc.dram_tensor("wchf", [NC_, P, NF * C], BF16, kind="ExternalInput")
    tlw_in = nc.dram_tensor("tlw", [NO_TOK, P, T], BF16, kind="ExternalInput")
    btok_in = nc.dram_tensor("btok", [P, NT], FP32, kind="ExternalInput")
    bch_in = nc.dram_tensor("bch", [P, C], BF16, kind="ExternalInput")
    ln_in = nc.dram_tensor("lnwb", [P, 4 * C], FP32, kind="ExternalInput")

    with tile.TileContext(nc) as tc, \
         tc.tile_pool(name="singles", bufs=1) as singles, \
         tc.tile_pool(name="xpool", bufs=3) as xpool, \
         tc.tile_pool(name="fpool", bufs=2) as fpool, \
         tc.tile_pool(name="f2pool", bufs=1) as f2pool, \
         tc.tile_pool(name="scratch", bufs=1) as scratch, \
         tc.tile_pool(name="ypool", bufs=2) as ypool, \
         tc.tile_pool(name="x1pool", bufs=2) as x1pool, \
         tc.tile_pool(name="opool", bufs=8) as opool, \
         tc.tile_pool(name="stats", bufs=2) as stats, \
         tc.tile_pool(name="ptokp", bufs=2, space="PSUM") as ptokp, \
         tc.tile_pool(name="pza", bufs=2, space="PSUM") as pza, \
         tc.tile_pool(name="trpz", bufs=2, space="PSUM") as trpz, \
         tc.tile_pool(name="poutp", bufs=2, space="PSUM") as poutp:

        # ---- batch-0 x first so the big weight DMAs don't block start ----
        def load_x(b, nsplit=2):
            xt = xpool.tile([P, NT, C], FP32, tag="X", name=f"X{b}")
            step = NT // nsplit
            for i in range(0, NT, step):
                nc.sync.dma_start(
                    out=xt[:, i:i + step, :],
                    in_=x_in[b, i * P:(i + step) * P, :].rearrange(
                        "(i p) c -> p i c", p=P))
            return [xt[:, i, :] for i in range(NT)]

        # batch-0: per-tile DMAs interleaved with per-tile wtok loads so
        # tile-0 stats and the first mm1 weights arrive ASAP
        X0t = xpool.tile([P, NT, C], FP32, tag="X", name="X0")
        wtok_all = singles.tile([P, NT, NF, TD], BF16, tag="wtok")
        H = C // 2
        for i in (0, 1):
            for hh in (0, 1):
                nc.sync.dma_start(out=X0t[:, i, hh * H:(hh + 1) * H],
                                  in_=x_in[0, i * P:(i + 1) * P, hh * H:(hh + 1) * H])
        wtok0r = wtok_in[0].rearrange("p (f o) -> p f o", f=NF)
        nc.sync.dma_start(out=wtok_all[:, 0, 0:NF // 2], in_=wtok0r[:, 0:NF // 2])
        nc.sync.dma_start(out=wtok_all[:, 0, NF // 2:], in_=wtok0r[:, NF // 2:])
        for i in (2, 3):
            nc.sync.dma_start(out=X0t[:, i, :], in_=x_in[0, i * P:(i + 1) * P, :])
        for i in range(1, NT):
            nc.sync.dma_start(out=wtok_all[:, i],
                              in_=wtok_in[i].rearrange("p (f o) -> p f o", f=NF))
        X1 = load_x(1) if NB > 1 else None
        X0 = [X0t[:, i, :] for i in range(NT)]
        wtok = [wtok_all[:, i] for i in range(NT)]

        ident = singles.tile([P, P], BF16, tag="ident")
        make_identity(nc, ident)
        ones128 = singles.tile([P, P], BF16, tag="ones128")
        nc.vector.memset(ones128, float(1.0 / 128.0))
        halfpi = singles.tile([P, 1], FP32, tag="halfpi")
        nc.vector.memset(halfpi, PI / 2)
        actwarm = singles.tile([P, 1], FP32, tag="actwarm")
        nc.scalar.activation(out=actwarm, in_=halfpi, func=AF.Sin)
        tlw = []
        for j in range(NO_TOK):
            t_ = singles.tile([P, T], BF16, tag=f"tlw{j}")
            nc.sync.dma_start(out=t_, in_=tlw_in[j])
            tlw.append(t_)
        btok = singles.tile([P, NT], FP32, tag="btok")
        nc.sync.dma_start(out=btok, in_=btok_in[:, :])

        def load_weights_late():
            wchf_all = singles.tile([P, NC_, NF, C], BF16, tag="wchf")
            nc.sync.dma_start(out=wchf_all,
                              in_=wchf_in.rearrange("m p (f o) -> p m f o", f=NF))
            wchf = [wchf_all[:, m] for m in range(NC_)]
            bch = singles.tile([P, C], BF16, tag="bch")
            nc.sync.dma_start(out=bch, in_=bch_in[:, :])
            lnwb = None
            if apply_ln2 and not apply_ln1:
                lnwb = singles.tile([P, 4, C], FP32, tag="lnwb")
                nc.sync.dma_start(out=lnwb, in_=ln_in.rearrange("p (k c) -> p k c", k=4))
            return wchf, bch, lnwb

        # ---- helpers ----
        def ln1_stats(X, tiles, gkey):
            """bn stats + rsqrt Newton + derived wrap scalars for a group of
            tiles.  Returns {tile: (rstd, nb, aa, bv, cc2) [P,1] slices}."""
            n = len(tiles)
            mvs = stats.tile([P, n, 2], FP32, tag=f"mvs{gkey}", name="mvs")
            for k, i in enumerate(tiles):
                st6 = stats.tile([P, 6], FP32, tag=f"st6_{i % 2}", name="st6")
                nc.vector.bn_stats(out=st6, in_=X[i])
                nc.vector.bn_aggr(out=mvs[:, k, :], in_=st6)
            mean = mvs[:, :, 0]
            var = mvs[:, :, 1]
            h = stats.tile([P, n], FP32, tag=f"h{gkey}", name="h")
            nc.vector.tensor_scalar(out=h, in0=var, scalar1=EPS, scalar2=-0.5,
                                    op0=OP.add, op1=OP.mult)
            yi = stats.tile([P, n], I32, tag=f"yi{gkey}", name="yi")
            nc.vector.tensor_scalar(out=yi, in0=var.bitcast(I32), scalar1=1,
                                    scalar2=None, op0=OP.logical_shift_right)
            nc.vector.tensor_scalar(out=yi, in0=yi, scalar1=-1,
                                    scalar2=0x5F3759DF, op0=OP.mult, op1=OP.add)
            rstd = yi.bitcast(FP32)
            a2 = stats.tile([P, n], FP32, tag=f"a2{gkey}", name="a2")
            for _ in range(2):
                nc.vector.tensor_tensor(out=a2, in0=rstd, in1=rstd, op=OP.mult)
                nc.vector.tensor_tensor(out=a2, in0=a2, in1=h, op=OP.mult)
                nc.vector.scalar_tensor_tensor(out=rstd, in0=a2, scalar=1.5,
                                               in1=rstd, op0=OP.add, op1=OP.mult)
            nb = stats.tile([P, n], FP32, tag=f"nb{gkey}", name="nb")
            nc.vector.scalar_tensor_tensor(out=nb, in0=mean, scalar=-1.0, in1=rstd,
                                           op0=OP.mult, op1=OP.mult)
            aa = stats.tile([P, n], FP32, tag=f"aa{gkey}", name="aa")
            nc.vector.tensor_scalar(out=aa, in0=rstd, scalar1=INV_2PI, scalar2=None,
                                    op0=OP.mult)
            bv = stats.tile([P, n], FP32, tag=f"bv{gkey}", name="bv")
            nc.vector.tensor_scalar(out=bv, in0=nb, scalar1=INV_2PI, scalar2=None,
                                    op0=OP.mult)
            cc2 = stats.tile([P, n], FP32, tag=f"cc2{gkey}", name="cc2")
            nc.vector.scalar_tensor_tensor(out=cc2, in0=h, scalar=FOUR_PI, in1=rstd,
                                           op0=OP.mult, op1=OP.mult)
            return {i: tuple(t[:, k:k + 1] for t in (rstd, nb, aa, bv, cc2))
                    for k, i in enumerate(tiles)}

        def feat_tiles(pool, pref, i):
            return [pool.tile([P, C], BF16, tag=f"{pref}_{i}_{k}", name=f"{pref}{i}b{k}")
                    for k in range(NF)]

        def features_from_x(xt, i, rstd, nb, aa, bv, cc2, pref):
            """Token-path features straight from raw x (LN folded in)."""
            if apply_ln1:
                # general path: materialize normalized tensor, then wrap
                xn = scratch.tile([P, C], FP32, tag=f"xn{i % 2}", name="xn")
                nc.scalar.activation(out=xn, in_=xt, func=AF.Identity,
                                     bias=nb, scale=rstd)
                nc.vector.tensor_mul(out=xn, in0=xn, in1=lnwb[:, 0, :])
                nc.vector.tensor_add(out=xn, in0=xn, in1=lnwb[:, 1, :])
                return features_from_norm(xn, f"F1_{i}", fpool, pref, i)
            f = feat_tiles(fpool, pref, i)
            ni = scratch.tile([P, C], I32, tag=f"ni1_{i % 2}", name="ni")
            nc.vector.tensor_scalar(out=ni, in0=xt, scalar1=aa,
                                    scalar2=bv, op0=OP.mult, op1=OP.add)
            rt = scratch.tile([P, C], FP32, tag=f"rt1_{i % 2}", name="rt")
            nc.vector.scalar_tensor_tensor(out=rt, in0=ni, scalar=cc2,
                                           in1=xt, op0=OP.mult, op1=OP.add)
            nc.scalar.activation(out=f[0], in_=rt, func=AF.Sin,
                                 scale=rstd, bias=nb)
            ab = scratch.tile([P, C], FP32, tag=f"ab1_{i % 2}", name="ab")
            nc.scalar.activation(out=ab, in_=rt, func=AF.Abs,
                                 scale=rstd, bias=nb)
            nc.scalar.activation(out=f[1], in_=ab, func=AF.Sin, scale=-1.0,
                                 bias=halfpi[:, :])
            nc.vector.tensor_mul(out=f[3], in0=f[0], in1=f[0])   # ss
            nc.vector.tensor_mul(out=f[2], in0=f[0], in1=f[1])   # sc
            nc.vector.tensor_mul(out=f[4], in0=f[3], in1=f[0])   # sss
            nc.vector.tensor_mul(out=f[5], in0=f[3], in1=f[1])   # css
            return f

        def features_from_norm(src, key, pool, pref, i):
            """Channel-path features from an already-normalized source
            (SBUF tile or PSUM transpose slice)."""
            f = feat_tiles(pool, pref, i)
            ni = scratch.tile([P, C], I32, tag=f"ni_{key}" if apply_ln1 else f"ni2_{i % 2}",
                             name="ni")
            nc.vector.tensor_scalar(out=ni, in0=src, scalar1=INV_2PI, scalar2=None,
                                    op0=OP.mult)
            rt = scratch.tile([P, C], FP32, tag=f"rt_{key}" if apply_ln1 else f"rt2_{i % 2}",
                             name="rt")
            nc.vector.scalar_tensor_tensor(out=rt, in0=ni, scalar=-TWO_PI,
                                           in1=src, op0=OP.mult, op1=OP.add)
            nc.scalar.activation(out=f[0], in_=rt, func=AF.Sin)
            ab = scratch.tile([P, C], FP32, tag=f"ab_{key}" if apply_ln1 else f"ab2_{i % 2}",
                             name="ab")
            nc.scalar.activation(out=ab, in_=rt, func=AF.Abs)
            nc.scalar.activation(out=f[1], in_=ab, func=AF.Sin, scale=-1.0,
                                 bias=halfpi[:, :])
            nc.vector.tensor_mul(out=f[3], in0=f[0], in1=f[0])
            nc.vector.tensor_mul(out=f[2], in0=f[0], in1=f[1])
            nc.vector.tensor_mul(out=f[4], in0=f[3], in1=f[0])
            nc.vector.tensor_mul(out=f[5], in0=f[3], in1=f[1])
            return f

        def stage1_prefetch(X, gsize=NT):
            """LN1 + token features for a batch whose x is already loading.
            Small group sizes start tile-0's feature chain earlier (used
            during the DMA-bound fill)."""
            groups = [tuple(range(g, g + gsize)) for g in range(0, NT, gsize)]
            feats = [None] * NT
            for g, tiles in enumerate(groups):
                sc = ln1_stats(X, tiles, f"{len(tiles)}_{g % 2}")
                for i in tiles:
                    feats[i] = features_from_x(X[i], i, *sc[i], "F1")
            return feats

        def mm1(feats, ptok, tiles):
            for i in tiles:
                for j in range(NO_TOK):
                    for f in range(NF):
                        nc.tensor.matmul(ptok[j], wtok[i][:, f, j * P:(j + 1) * P],
                                         feats[i][f], start=(i == 0 and f == 0),
                                         stop=(i == NT - 1 and f == NF - 1))

        def ln2_chain(s1, e2, n, gkey):
            """LN2 rsqrt chain on DVE (latency-critical for the transposes).
            s1/e2 [P,n] slices -> (rstd2, nm2) [P,n]."""
            mn = stats.tile([P, n], FP32, tag=f"mn{gkey}", name="mn")
            nc.vector.tensor_scalar_mul(out=mn, in0=s1, scalar1=1.0 / C)
            vr = stats.tile([P, n], FP32, tag=f"vr{gkey}", name="vr")
            nc.vector.tensor_mul(out=vr, in0=mn, in1=mn)
            nc.vector.scalar_tensor_tensor(out=vr, in0=e2, scalar=1.0 / C, in1=vr,
                                           op0=OP.mult, op1=OP.subtract)
            h2 = stats.tile([P, n], FP32, tag=f"h2{gkey}", name="h2")
            nc.vector.tensor_scalar(out=h2, in0=vr, scalar1=EPS, scalar2=-0.5,
                                    op0=OP.add, op1=OP.mult)
            yi2 = stats.tile([P, n], I32, tag=f"yi2{gkey}", name="yi2")
            nc.vector.tensor_scalar(out=yi2, in0=vr.bitcast(I32), scalar1=1,
                                    scalar2=None, op0=OP.logical_shift_right)
            nc.vector.tensor_scalar(out=yi2, in0=yi2, scalar1=-1,
                                    scalar2=0x5F3759DF, op0=OP.mult, op1=OP.add)
            rstd2 = yi2.bitcast(FP32)
            a2 = stats.tile([P, n], FP32, tag=f"a2b{gkey}", name="a2b")
            for _ in range(2):
                nc.vector.tensor_mul(out=a2, in0=rstd2, in1=rstd2)
                nc.vector.tensor_mul(out=a2, in0=a2, in1=h2)
                nc.vector.scalar_tensor_tensor(out=rstd2, in0=a2, scalar=1.5,
                                               in1=rstd2, op0=OP.add, op1=OP.mult)
            nm2 = stats.tile([P, n], FP32, tag=f"nm2{gkey}", name="nm2")
            nc.vector.scalar_tensor_tensor(out=nm2, in0=mn, scalar=-1.0,
                                           in1=rstd2, op0=OP.mult, op1=OP.mult)
            return rstd2, nm2

        def mm2_resid(q, X, ytok, s1, e2, pz_pool):
            pz = pz_pool.tile([P, C], FP32, tag="pz", name="pz")
            for j in range(NO_TOK):
                nc.tensor.matmul(pz, tlw[j][:, q * P:(q + 1) * P], ytok[j],
                                 start=(j == 0), stop=(j == NO_TOK - 1))
            xt = x1pool.tile([P, C], BF16, tag=f"x1_{q}", name=f"x1_{q}")
            nc.vector.scalar_tensor_tensor(out=xt, in0=pz,
                                           scalar=btok[:, q:q + 1],
                                           in1=X[q], op0=OP.add, op1=OP.add,
                                           accum_out=s1[:, q:q + 1])
            sq = scratch.tile([P, C], FP32, tag="sq", name="sq")
            nc.vector.scalar_tensor_tensor(out=sq, in0=xt, scalar=1.0, in1=xt,
                                           op0=OP.mult, op1=OP.mult,
                                           accum_out=e2[:, q:q + 1])
            return xt

        def normalize_xn2(q, x1q, rstd2, nm2, k):
            xq = ypool.tile([P, C], BF16, tag=f"xn2_{q}", name=f"xn2_{q}", bufs=1)
            if apply_ln2:
                tmp = scratch.tile([P, C], FP32, tag="lntmp", name="lntmp")
                nc.scalar.activation(out=tmp, in_=x1q, func=AF.Identity,
                                     bias=nm2[:, k:k + 1], scale=rstd2[:, k:k + 1])
                nc.vector.tensor_mul(out=tmp, in0=tmp, in1=lnwb[:, 2, :])
                nc.vector.scalar_tensor_tensor(out=xq, in0=tmp, scalar=1.0,
                                               in1=lnwb[:, 3, :], op0=OP.mult,
                                               op1=OP.add)
            else:
                nc.scalar.activation(out=xq, in_=x1q, func=AF.Identity,
                                     bias=nm2[:, k:k + 1], scale=rstd2[:, k:k + 1])
            return xq

        def alloc_tr():
            # c-tiles m packed 2 per PSUM bank (the fp32 bank tile is viewed
            # as [P, 2C] bf16 so the tag matches the pz allocations rotating
            # through the same 2 banks)
            return [trpz.tile([P, C], FP32, tag="pz", name=f"tr{h}").bitcast(BF16)
                    for h in range(2)]

        def transpose_i(tr, xn2i, i):
            for m in range(NC_):
                nc.tensor.transpose(
                    tr[m // 2][:, (m % 2) * C + i * P:(m % 2) * C + (i + 1) * P],
                    xn2i[:, m * P:(m + 1) * P], ident)

        def ytok_copies(ptok):
            ytok = []
            for j in range(NO_TOK):
                ysb = ypool.tile([P, C], BF16, tag=f"ytok{j}", name="ysb")
                nc.scalar.copy(out=ysb, in_=ptok[j])
                ytok.append(ysb)
            return ytok

        def token_out(b, X, ytok):
            """mm2, residual+LN2 stats, normalize -> xn2."""
            s1 = stats.tile([P, NT], FP32, tag="s1", name="s1")
            e2 = stats.tile([P, NT], FP32, tag="e2", name="e2")
            x1 = [mm2_resid(q, X, ytok, s1, e2, pza if q < 2 else trpz)
                  for q in range(NT)]
            rstd2, nm2 = ln2_chain(s1, e2, NT, "")
            xn2 = [normalize_xn2(q, x1[q], rstd2, nm2, q) for q in range(NT)]
            return x1, xn2

        def token_out_last(b, X, ytok):
            """Last batch: no next-batch mm1 to cover the LN2 chain, so
            process pair-wise and interleave the transposes."""
            s1 = stats.tile([P, NT], FP32, tag="s1", name="s1")
            e2 = stats.tile([P, NT], FP32, tag="e2", name="e2")
            tr = alloc_tr()
            x1 = []
            for g in range(2):
                qs = (2 * g, 2 * g + 1)
                for q in qs:
                    x1.append(mm2_resid(q, X, ytok, s1, e2, pza))
                rstd2, nm2 = ln2_chain(s1[:, 2 * g:2 * g + 2],
                                       e2[:, 2 * g:2 * g + 2], 2, f"p{g}")
                for k, q in enumerate(qs):
                    xq = normalize_xn2(q, x1[q], rstd2, nm2, k)
                    transpose_i(tr, xq, q)
            return x1, tr

        def transposes(xn2):
            tr = alloc_tr()
            for i in range(NT):
                transpose_i(tr, xn2[i], i)
            return tr

        def channel_feats(tr):
            return [features_from_norm(tr[m // 2][:, (m % 2) * C:(m % 2 + 1) * C],
                                       f"c{m}", f2pool, "F2", m)
                    for m in range(NC_)]

        def mm3(fch):
            """q-major channel matmuls with the bias folded in via a
            ones-matmul; returns the 4 PSUM tiles."""
            pouts = []
            for q in range(NT):
                pout = poutp.tile([P, C], FP32, tag="pout", name=f"pout{q}")
                for m in range(NC_):
                    for f in range(NF):
                        nc.tensor.matmul(pout, fch[m][f][:, q * P:(q + 1) * P],
                                         wchf[m][:, f, :],
                                         start=(m == 0 and f == 0),
                                         stop=(m == NC_ - 1 and f == NF - 1))
                pouts.append(pout)
            return pouts

        def emit_out(b, pouts, x1, final=False):
            """residual add on DVE straight from PSUM, then store.  Emitted
            one iteration late so it never head-of-line-blocks the next
            batch's feature chain on the DVE FIFO.  For the final batch the
            stores split across queues so the tail transfer isn't bound by
            one ring's bandwidth."""
            for q in range(NT):
                ot = opool.tile([P, C], FP32, tag="out", name="out")
                nc.vector.tensor_tensor(out=ot, in0=pouts[q], in1=x1[q],
                                        op=OP.add)
                nc.vector.tensor_tensor(out=ot, in0=ot, in1=bch, op=OP.add)
                if final and q >= 2:
                    for hh in range(4):
                        nc.sync.dma_start(
                            out=y_out[b, q * P:(q + 1) * P,
                                      hh * (C // 4):(hh + 1) * (C // 4)],
                            in_=ot[:, hh * (C // 4):(hh + 1) * (C // 4)])
                else:
                    nc.sync.dma_start(out=y_out[b, q * P:(q + 1) * P, :], in_=ot)

        # ---- software-pipelined emission over batches ----
        # DMA order: X(0), wtok, [x(1)], then the big wchf -- the fill is
        # HBM-bandwidth-bound, so batch-0's dependencies go first.
        if apply_ln1:
            lnwb = singles.tile([P, 4, C], FP32, tag="lnwb")
            nc.sync.dma_start(out=lnwb, in_=ln_in.rearrange("p (k c) -> p k c", k=4))
        Xc, featsc = X0, stage1_prefetch(X0, gsize=1)
        Xmap = {0: X0}
        if NB > 1:
            Xmap[1] = X1
        if NB > 2:
            Xmap[2] = load_x(2)
        wchf, bch, lnwb2 = load_weights_late()
        if not apply_ln1:
            lnwb = lnwb2
        ptokc = [ptokp.tile([P, C], FP32, tag="ptok", name=f"ptok{j}")
                 for j in range(NO_TOK)]
        mm1(featsc, ptokc, range(NT))
        pending_out = None
        hoisted = None   # last batch's token stage, pre-emitted an iteration early
        for b in range(NB):
            X, feats, ptok = Xc, featsc, ptokc
            # b=0: the next batch's Sin chain goes ahead of the ytok copies
            # on the ACT queue -- the copies wait for mm1(0) anyway, while
            # the sins' inputs are ready earlier (kills the ramp gaps).
            # Steady state: copies first (they gate mm2 on the PE; the sins'
            # inputs arrive early under the mm3 cover).
            if b == 0 and b + 1 < NB:
                Xc = Xmap[b + 1]
                featsc = stage1_prefetch(Xc, gsize=2)
                ytok = ytok_copies(ptok)
                if b + 2 < NB and (b + 2) not in Xmap:
                    Xmap[b + 2] = load_x(b + 2)
            elif hoisted is None:
                ytok = ytok_copies(ptok)
                if b + 1 < NB:
                    Xc = Xmap[b + 1]
                    featsc = stage1_prefetch(Xc, gsize=NT)
                    if b + 2 < NB and (b + 2) not in Xmap:
                        Xmap[b + 2] = load_x(b + 2)
            if b + 1 < NB:
                x1, xn2 = token_out(b, X, ytok)
                if pending_out is not None:
                    emit_out(b - 1, *pending_out)
                ptokc = [ptokp.tile([P, C], FP32, tag="ptok", name=f"ptok{j}")
                         for j in range(NO_TOK)]
                mm1(featsc, ptokc, (0, 1))
                tr = transposes(xn2)
                mm1(featsc, ptokc, (2, 3))
                fch = channel_feats(tr)
                if b + 1 == NB - 1:
                    # hoist the final batch's token stage (ytok, mm2, LN2,
                    # normalize) ahead of this mm3 so its serial chain runs
                    # under the 21us matmul cover instead of stalling the PE
                    ytok_l = ytok_copies(ptokc)
                    hoisted = token_out(b + 1, Xc, ytok_l)
            else:
                if hoisted is not None:
                    x1, xn2 = hoisted
                    tr = transposes(xn2)
                else:
                    x1, tr = token_out_last(b, X, ytok)
                if pending_out is not None:
                    emit_out(b - 1, *pending_out)
                fch = channel_feats(tr)
            pending_out = (mm3(fch), x1)
        emit_out(NB - 1, *pending_out, final=True)

    _split_multi_waits(nc)
    return nc


_CACHE = {}


def _get_nc(apply_ln1, apply_ln2):
    key = (apply_ln1, apply_ln2)
    if key not in _CACHE:
        _CACHE[key] = _build(apply_ln1, apply_ln2)
    return _CACHE[key]


def prepare_in_maps(inputs):
    return _prepare(**inputs)


def _prepare(x, ln1_w, ln1_b, tok_coef, tok_kbias, tok_lw, tok_lb,
             ln2_w, ln2_b, ch_coef, ch_kbias, ch_lw, ch_lb):
    x = np.asarray(x, np.float32)
    f64 = np.float64

    wtok_eff, tok_const = _cheb_weights(np.asarray(tok_coef, f64))  # (T,6,TD)
    wch_eff, ch_const = _cheb_weights(np.asarray(ch_coef, f64))     # (C,6,2C)

    kbias_tok = np.asarray(tok_kbias, f64).reshape(-1) + tok_const
    kbias_ch = np.asarray(ch_kbias, f64).reshape(-1) + ch_const
    bias_tok = np.asarray(tok_lb, f64) + np.asarray(tok_lw, f64) @ kbias_tok
    bias_ch = np.asarray(ch_lb, f64) + np.asarray(ch_lw, f64) @ kbias_ch

    # fold the channel post-KAN linear into the KAN weights (fp64)
    wchf = np.einsum("cfo,ko->cfk", wch_eff, np.asarray(ch_lw, f64))  # (C,6,C)

    wtok_np = wtok_eff.reshape(NT, P, NF * TD).astype(ml_dtypes.bfloat16)
    wchf_np = wchf.reshape(NC_, P, NF * C).astype(ml_dtypes.bfloat16)
    tlw_np = np.ascontiguousarray(np.asarray(tok_lw, f64).T).reshape(
        NO_TOK, P, T).astype(ml_dtypes.bfloat16)
    btok_np = np.ascontiguousarray(bias_tok.reshape(NT, P).T).astype(np.float32)
    bch_np = np.broadcast_to(bias_ch.astype(ml_dtypes.bfloat16), (P, C)).copy()
    lnwb_np = np.broadcast_to(
        np.concatenate([np.asarray(ln1_w, f64), np.asarray(ln1_b, f64),
                        np.asarray(ln2_w, f64), np.asarray(ln2_b, f64)]).astype(
            np.float32), (P, 4 * C)).copy()

    apply_ln1 = not (np.all(np.asarray(ln1_w) == 1.0) and np.all(np.asarray(ln1_b) == 0.0))
    apply_ln2 = not (np.all(np.asarray(ln2_w) == 1.0) and np.all(np.asarray(ln2_b) == 0.0))

    shared = dict(wtok=wtok_np, wchf=wchf_np, tlw=tlw_np,
                  btok=btok_np, bch=bch_np, lnwb=lnwb_np)
    in_maps = []
    for core in range(NCORES):
        m = dict(shared)
        m["x"] = np.ascontiguousarray(x[core * NB:(core + 1) * NB])
        in_maps.append(m)
    return {"build_key": (apply_ln1, apply_ln2), "in_maps": in_maps}


def kernel(**inputs):
    prep = _prepare(**inputs)
    nc = _get_nc(*prep["build_key"])
    res = run_bass_kernel_spmd(nc, prep["in_maps"], list(range(NCORES)))
    return np.concatenate([res.results[i]["y"] for i in range(NCORES)], axis=0)


# revision 42
# speedup vs baseline: 1.0317x; 1.0097x over previous
"""MixerLayerKAN Trainium2 kernel.

x (B,T,C)=(32,512,512) fp32; token-mix FourierKAN(T->TD)+Linear, then
channel-mix FourierKAN(C->2C)+Linear, LN + residual around each.

Strategy (data-parallel over batch, 4 batches per NeuronCore, weights
replicated, no collectives):

* Fourier features cos(kx)/sin(kx), k=1..3, re-expressed in the product
  basis {s, c, s*c, s^2, s^3, c*s^2}; harmonic coefficients fold
  host-side into 6 effective weight matrices; the channel KAN further
  folds its post-KAN Linear (96 matmuls/batch instead of 224).
* LN1 normalize is folded into the feature chain: the range-wrap
  (round-to-int tensor_scalar + int32-input scalar_tensor_tensor, with
  per-partition scale/bias riding the scalar slots) and the Sin/Abs
  activations consume raw x directly -- no materialized normalized
  tensor and no int->float CAST on the token path.
* Transposes run in bf16 (1 cycle/row), two c-tiles packed per PSUM
  bank; the channel wrap chain reads transposed values straight from
  PSUM (no PSUM->SBUF copy ops).
* Each output tile is two DVE adds straight from PSUM (psum +
  residual, + channel bias) -- keeping the adds off the saturated PE
  and the copies off the scalar engine.
* PE FIFO is software-pipelined: mm1 of batch b+1 is split into two
  24-matmul chunks emitted around batch b's transposes, covering the
  two serial handoffs (LN2 chain, channel feature chain); mm3 runs
  q-major so the output adds overlap it; output DMAs trail one
  iteration so they never head-of-line-block the DVE FIFO.
* DMA: per-queue ring bandwidth is ~46GB/s, so batch-0's x tiles and
  the first token weights load as small split DMAs across rings, x is
  prefetched two batches ahead (before the big wchf load), and the
  final stores split across rings to shorten the drain.
* The last batch's token stage (ytok, mm2, LN2, normalize) is hoisted
  a full iteration early so its serial chain executes under the
  previous batch's 21us mm3 cover; only its transposes remain in the
  final iteration.

Measured ~190-192us/core (~233us baseline); note the part
power-throttles under sustained load, adding up to ~40us run-to-run.
"""

import numpy as np
import ml_dtypes

import concourse.bass as bass
import concourse.mybir as mybir
from concourse import tile
from concourse.bass_utils import run_bass_kernel_spmd
from concourse.masks import make_identity

AF = mybir.ActivationFunctionType
OP = mybir.AluOpType
FP32 = mybir.dt.float32
BF16 = mybir.dt.bfloat16
I32 = mybir.dt.int32

B, T, C, TD, G = 32, 512, 512, 256, 3
NCORES = 8
NB = B // NCORES          # batches per core
P = 128
EPS = 1e-5
PI = float(np.pi)
TWO_PI = float(2 * np.pi)
INV_2PI = float(1.0 / (2 * np.pi))
FOUR_PI = float(4 * np.pi)
NF = 6                    # product-basis features
NT = T // P               # 4 t-tiles
NC_ = C // P              # 4 c-tiles
NO_TOK = TD // P          # 2 token KAN hidden tiles


def _split_multi_waits(nc):
    """This walrus build accepts at most ONE sync-wait command per
    instruction.  Tile emits several.  Fix: before each multi-wait
    instruction, splice in same-engine NOPs carrying one wait each (a wait
    executed earlier on the same engine is semantically identical)."""
    f = nc.m.functions[0]
    per_engine = {}
    for bb in f.blocks:
        for inst in bb.instructions:
            si = getattr(inst, "sync_info", None)
            if si is not None and si.on_wait and len(si.on_wait) > 1:
                per_engine[inst.engine] = per_engine.get(inst.engine, 0) + (
                    len(si.on_wait) - 1)
    if not per_engine:
        return
    nop_pool = {}
    for eng, cnt in per_engine.items():
        nop_pool[eng] = [nc.engines[eng].nop(nofuse=True).ins for _ in range(cnt)]
    created = {id(i) for h in nop_pool.values() for i in h}
    for bb in f.blocks:
        bb.instructions[:] = [i for i in bb.instructions if id(i) not in created]
    for bb in f.blocks:
        out = []
        for inst in bb.instructions:
            si = getattr(inst, "sync_info", None)
            if si is not None and si.on_wait and len(si.on_wait) > 1:
                waits = list(si.on_wait)
                si.on_wait = [waits[-1]]
                for w in waits[:-1]:
                    nop = nop_pool[inst.engine].pop()
                    nop.sync_info = mybir.SyncInfo(on_wait=[w], on_update=[])
                    out.append(nop)
            out.append(inst)
        bb.instructions[:] = out


def _cheb_weights(coef):
    """coef (2, O, I, G) -> effective basis weights (I, 6, O) for the
    {s, c, s*c, s^2, s^3, c*s^2} basis, plus the constant term (O,).

    cos(1x)=c; cos(2x)=1-2s^2; cos(3x)=c-4c s^2
    sin(1x)=s; sin(2x)=2 s c ; sin(3x)=3s-4s^3
    """
    cosw = coef[0]
    sinw = coef[1]
    O, I, _ = cosw.shape
    w = np.zeros((I, NF, O), np.float64)
    w[:, 0, :] = (sinw[:, :, 0] + 3.0 * sinw[:, :, 2]).T      # s
    w[:, 1, :] = (cosw[:, :, 0] + cosw[:, :, 2]).T            # c
    w[:, 2, :] = (2.0 * sinw[:, :, 1]).T                      # s*c
    w[:, 3, :] = (-2.0 * cosw[:, :, 1]).T                     # s^2
    w[:, 4, :] = (-4.0 * sinw[:, :, 2]).T                     # s^3
    w[:, 5, :] = (-4.0 * cosw[:, :, 2]).T                     # c*s^2
    const = cosw[:, :, 1].sum(axis=1)                         # from the "1" of cos(2x)
    return w, const


def _build(apply_ln1, apply_ln2):
    nc = bass.Bass()
    x_in = nc.dram_tensor("x", [NB, T, C], FP32, kind="ExternalInput")
    y_out = nc.dram_tensor("y", [NB, T, C], FP32, kind="ExternalOutput")
    wtok_in = nc.dram_tensor("wtok", [NT, P, NF * TD], BF16, kind="ExternalInput")
    wchf_in = nc.dram_tensor("wchf", [NC_, P, NF * C], BF16, kind="ExternalInput")
    tlw_in = nc.dram_tensor("tlw", [NO_TOK, P, T], BF16, kind="ExternalInput")
    btok_in = nc.dram_tensor("btok", [P, NT], FP32, kind="ExternalInput")
    bch_in = nc.dram_tensor("bch", [P, C], BF16, kind="ExternalInput")
    ln_in = nc.dram_tensor("lnwb", [P, 4 * C], FP32, kind="ExternalInput")

    with tile.TileContext(nc) as tc, \
         tc.tile_pool(name="singles", bufs=1) as singles, \
         tc.tile_pool(name="xpool", bufs=3) as xpool, \
         tc.tile_pool(name="fpool", bufs=2) as fpool, \
         tc.tile_pool(name="f2pool", bufs=1) as f2pool, \
         tc.tile_pool(name="scratch", bufs=1) as scratch, \
         tc.tile_pool(name="ypool", bufs=2) as ypool, \
         tc.tile_pool(name="x1pool", bufs=2) as x1pool, \
         tc.tile_pool(name="opool", bufs=8) as opool, \
         tc.tile_pool(name="stats", bufs=2) as stats, \
         tc.tile_pool(name="ptokp", bufs=2, space="PSUM") as ptokp, \
         tc.tile_pool(name="pza", bufs=2, space="PSUM") as pza, \
         tc.tile_pool(name="trpz", bufs=2, space="PSUM") as trpz, \
         tc.tile_pool(name="poutp", bufs=2, space="PSUM") as poutp:

        # ---- batch-0 x first so the big weight DMAs don't block start ----
        def load_x(b, nsplit=2):
            xt = xpool.tile([P, NT, C], FP32, tag="X", name=f"X{b}")
            step = NT // nsplit
            for i in range(0, NT, step):
                nc.sync.dma_start(
                    out=xt[:, i:i + step, :],
                    in_=x_in[b, i * P:(i + step) * P, :].rearrange(
                        "(i p) c -> p i c", p=P))
            return [xt[:, i, :] for i in range(NT)]

        # batch-0: per-tile DMAs interleaved with per-tile wtok loads so
        # tile-0 stats and the first mm1 weights arrive ASAP
        X0t = xpool.tile([P, NT, C], FP32, tag="X", name="X0")
        wtok_all = singles.tile([P, NT, NF, TD], BF16, tag="wtok")
        H = C // 2
        for i in (0, 1):
            for hh in (0, 1):
                nc.sync.dma_start(out=X0t[:, i, hh * H:(hh + 1) * H],
                                  in_=x_in[0, i * P:(i + 1) * P, hh * H:(hh + 1) * H])
        wtok0r = wtok_in[0].rearrange("p (f o) -> p f o", f=NF)
        nc.sync.dma_start(out=wtok_all[:, 0, 0:NF // 2], in_=wtok0r[:, 0:NF // 2])
        nc.sync.dma_start(out=wtok_all[:, 0, NF // 2:], in_=wtok0r[:, NF // 2:])
        for i in (2, 3):
            nc.sync.dma_start(out=X0t[:, i, :], in_=x_in[0, i * P:(i + 1) * P, :])
        for i in range(1, NT):
            nc.sync.dma_start(out=wtok_all[:, i],
                              in_=wtok_in[i].rearrange("p (f o) -> p f o", f=NF))
        X1 = load_x(1) if NB > 1 else None
        X0 = [X0t[:, i, :] for i in range(NT)]
        wtok = [wtok_all[:, i] for i in range(NT)]

        ident = singles.tile([P, P], BF16, tag="ident")
        make_identity(nc, ident)
        ones128 = singles.tile([P, P], BF16, tag="ones128")
        nc.vector.memset(ones128, float(1.0 / 128.0))
        halfpi = singles.tile([P, 1], FP32, tag="halfpi")
        nc.vector.memset(halfpi, PI / 2)
        actwarm = singles.tile([P, 1], FP32, tag="actwarm")
        nc.scalar.activation(out=actwarm, in_=halfpi, func=AF.Sin)
        tlw = []
        for j in range(NO_TOK):
            t_ = singles.tile([P, T], BF16, tag=f"tlw{j}")
            nc.sync.dma_start(out=t_, in_=tlw_in[j])
            tlw.append(t_)
        btok = singles.tile([P, NT], FP32, tag="btok")
        nc.sync.dma_start(out=btok, in_=btok_in[:, :])

        def load_weights_late():
            wchf_all = singles.tile([P, NC_, NF, C], BF16, tag="wchf")
            nc.sync.dma_start(out=wchf_all,
                              in_=wchf_in.rearrange("m p (f o) -> p m f o", f=NF))
            wchf = [wchf_all[:, m] for m in range(NC_)]
            bch = singles.tile([P, C], BF16, tag="bch")
            nc.sync.dma_start(out=bch, in_=bch_in[:, :])
            lnwb = None
            if apply_ln2 and not apply_ln1:
                lnwb = singles.tile([P, 4, C], FP32, tag="lnwb")
                nc.sync.dma_start(out=lnwb, in_=ln_in.rearrange("p (k c) -> p k c", k=4))
            return wchf, bch, lnwb

        # ---- helpers ----
        def ln1_stats(X, tiles, gkey):
            """bn stats + rsqrt Newton + derived wrap scalars for a group of
            tiles.  Returns {tile: (rstd, nb, aa, bv, cc2) [P,1] slices}."""
            n = len(tiles)
            mvs = stats.tile([P, n, 2], FP32, tag=f"mvs{gkey}", name="mvs")
            for k, i in enumerate(tiles):
                st6 = stats.tile([P, 6], FP32, tag=f"st6_{i % 2}", name="st6")
                nc.vector.bn_stats(out=st6, in_=X[i])
                nc.vector.bn_aggr(out=mvs[:, k, :], in_=st6)
            mean = mvs[:, :, 0]
            var = mvs[:, :, 1]
            h = stats.tile([P, n], FP32, tag=f"h{gkey}", name="h")
            nc.vector.tensor_scalar(out=h, in0=var, scalar1=EPS, scalar2=-0.5,
                                    op0=OP.add, op1=OP.mult)
            yi = stats.tile([P, n], I32, tag=f"yi{gkey}", name="yi")
            nc.vector.tensor_scalar(out=yi, in0=var.bitcast(I32), scalar1=1,
                                    scalar2=None, op0=OP.logical_shift_right)
            nc.vector.tensor_scalar(out=yi, in0=yi, scalar1=-1,
                                    scalar2=0x5F3759DF, op0=OP.mult, op1=OP.add)
            rstd = yi.bitcast(FP32)
            a2 = stats.tile([P, n], FP32, tag=f"a2{gkey}", name="a2")
            for _ in range(2):
                nc.vector.tensor_tensor(out=a2, in0=rstd, in1=rstd, op=OP.mult)
                nc.vector.tensor_tensor(out=a2, in0=a2, in1=h, op=OP.mult)
                nc.vector.scalar_tensor_tensor(out=rstd, in0=a2, scalar=1.5,
                                               in1=rstd, op0=OP.add, op1=OP.mult)
            nb = stats.tile([P, n], FP32, tag=f"nb{gkey}", name="nb")
            nc.vector.scalar_tensor_tensor(out=nb, in0=mean, scalar=-1.0, in1=rstd,
                                           op0=OP.mult, op1=OP.mult)
            aa = stats.tile([P, n], FP32, tag=f"aa{gkey}", name="aa")
            nc.vector.tensor_scalar(out=aa, in0=rstd, scalar1=INV_2PI, scalar2=None,
                                    op0=OP.mult)
            bv = stats.tile([P, n], FP32, tag=f"bv{gkey}", name="bv")
            nc.vector.tensor_scalar(out=bv, in0=nb, scalar1=INV_2PI, scalar2=None,
                                    op0=OP.mult)
            cc2 = stats.tile([P, n], FP32, tag=f"cc2{gkey}", name="cc2")
            nc.vector.scalar_tensor_tensor(out=cc2, in0=h, scalar=FOUR_PI, in1=rstd,
                                           op0=OP.mult, op1=OP.mult)
            return {i: tuple(t[:, k:k + 1] for t in (rstd, nb, aa, bv, cc2))
                    for k, i in enumerate(tiles)}

        def feat_tiles(pool, pref, i):
            return [pool.tile([P, C], BF16, tag=f"{pref}_{i}_{k}", name=f"{pref}{i}b{k}")
                    for k in range(NF)]

        def features_from_x(xt, i, rstd, nb, aa, bv, cc2, pref):
            """Token-path features straight from raw x (LN folded in)."""
            if apply_ln1:
                # general path: materialize normalized tensor, then wrap
                xn = scratch.tile([P, C], FP32, tag=f"xn{i % 2}", name="xn")
                nc.scalar.activation(out=xn, in_=xt, func=AF.Identity,
                                     bias=nb, scale=rstd)
                nc.vector.tensor_mul(out=xn, in0=xn, in1=lnwb[:, 0, :])
                nc.vector.tensor_add(out=xn, in0=xn, in1=lnwb[:, 1, :])
                return features_from_norm(xn, f"F1_{i}", fpool, pref, i)
            f = feat_tiles(fpool, pref, i)
            ni = scratch.tile([P, C], I32, tag=f"ni1_{i % 2}", name="ni")
            nc.vector.tensor_scalar(out=ni, in0=xt, scalar1=aa,
                                    scalar2=bv, op0=OP.mult, op1=OP.add)
            rt = scratch.tile([P, C], FP32, tag=f"rt1_{i % 2}", name="rt")
            nc.vector.scalar_tensor_tensor(out=rt, in0=ni, scalar=cc2,
                                           in1=xt, op0=OP.mult, op1=OP.add)
            nc.scalar.activation(out=f[0], in_=rt, func=AF.Sin,
                                 scale=rstd, bias=nb)
            ab = scratch.tile([P, C], FP32, tag=f"ab1_{i % 2}", name="ab")
            nc.scalar.activation(out=ab, in_=rt, func=AF.Abs,
                                 scale=rstd, bias=nb)
            nc.scalar.activation(out=f[1], in_=ab, func=AF.Sin, scale=-1.0,
                                 bias=halfpi[:, :])
            nc.vector.tensor_mul(out=f[3], in0=f[0], in1=f[0])   # ss
            nc.vector.tensor_mul(out=f[2], in0=f[0], in1=f[1])   # sc
            nc.vector.tensor_mul(out=f[4], in0=f[3], in1=f[0])   # sss
            nc.vector.tensor_mul(out=f[5], in0=f[3], in1=f[1])   # css
            return f

        def features_from_norm(src, key, pool, pref, i):
            """Channel-path features from an already-normalized source
            (SBUF tile or PSUM transpose slice)."""
            f = feat_tiles(pool, pref, i)
            ni = scratch.tile([P, C], I32, tag=f"ni_{key}" if apply_ln1 else f"ni2_{i % 2}",
                             name="ni")
            nc.vector.tensor_scalar(out=ni, in0=src, scalar1=INV_2PI, scalar2=None,
                                    op0=OP.mult)
            rt = scratch.tile([P, C], FP32, tag=f"rt_{key}" if apply_ln1 else f"rt2_{i % 2}",
                             name="rt")
            nc.vector.scalar_tensor_tensor(out=rt, in0=ni, scalar=-TWO_PI,
                                           in1=src, op0=OP.mult, op1=OP.add)
            nc.scalar.activation(out=f[0], in_=rt, func=AF.Sin)
            ab = scratch.tile([P, C], FP32, tag=f"ab_{key}" if apply_ln1 else f"ab2_{i % 2}",
                             name="ab")
            nc.scalar.activation(out=ab, in_=rt, func=AF.Abs)
            nc.scalar.activation(out=f[1], in_=ab, func=AF.Sin, scale=-1.0,
                                 bias=halfpi[:, :])
            nc.vector.tensor_mul(out=f[3], in0=f[0], in1=f[0])
            nc.vector.tensor_mul(out=f[2], in0=f[0], in1=f[1])
            nc.vector.tensor_mul(out=f[4], in0=f[3], in1=f[0])
            nc.vector.tensor_mul(out=f[5], in0=f[3], in1=f[1])
            return f

        def stage1_prefetch(X, gsize=NT):
            """LN1 + token features for a batch whose x is already loading.
            Small group sizes start tile-0's feature chain earlier (used
            during the DMA-bound fill)."""
            groups = [tuple(range(g, g + gsize)) for g in range(0, NT, gsize)]
            feats = [None] * NT
            for g, tiles in enumerate(groups):
                sc = ln1_stats(X, tiles, f"{len(tiles)}_{g % 2}")
                for i in tiles:
                    feats[i] = features_from_x(X[i], i, *sc[i], "F1")
            return feats

        def mm1(feats, ptok, tiles):
            for i in tiles:
                for j in range(NO_TOK):
                    for f in range(NF):
                        nc.tensor.matmul(ptok[j], wtok[i][:, f, j * P:(j + 1) * P],
                                         feats[i][f], start=(i == 0 and f == 0),
                                         stop=(i == NT - 1 and f == NF - 1))

        def ln2_chain(s1, e2, n, gkey):
            """LN2 rsqrt chain on DVE (latency-critical for the transposes).
            s1/e2 [P,n] slices -> (rstd2, nm2) [P,n]."""
            mn = stats.tile([P, n], FP32, tag=f"mn{gkey}", name="mn")
            nc.vector.tensor_scalar_mul(out=mn, in0=s1, scalar1=1.0 / C)
            vr = stats.tile([P, n], FP32, tag=f"vr{gkey}", name="vr")
            nc.vector.tensor_mul(out=vr, in0=mn, in1=mn)
            nc.vector.scalar_tensor_tensor(out=vr, in0=e2, scalar=1.0 / C, in1=vr,
                                           op0=OP.mult, op1=OP.subtract)
            h2 = stats.tile([P, n], FP32, tag=f"h2{gkey}", name="h2")
            nc.vector.tensor_scalar(out=h2, in0=vr, scalar1=EPS, scalar2=-0.5,
                                    op0=OP.add, op1=OP.mult)
            yi2 = stats.tile([P, n], I32, tag=f"yi2{gkey}", name="yi2")
            nc.vector.tensor_scalar(out=yi2, in0=vr.bitcast(I32), scalar1=1,
                                    scalar2=None, op0=OP.logical_shift_right)
            nc.vector.tensor_scalar(out=yi2, in0=yi2, scalar1=-1,
                                    scalar2=0x5F3759DF, op0=OP.mult, op1=OP.add)
            rstd2 = yi2.bitcast(FP32)
            a2 = stats.tile([P, n], FP32, tag=f"a2b{gkey}", name="a2b")
            for _ in range(2):
                nc.vector.tensor_mul(out=a2, in0=rstd2, in1=rstd2)
                nc.vector.tensor_mul(out=a2, in0=a2, in1=h2)
                nc.vector.scalar_tensor_tensor(out=rstd2, in0=a2, scalar=1.5,
                                               in1=rstd2, op0=OP.add, op1=OP.mult)
            nm2 = stats.tile([P, n], FP32, tag=f"nm2{gkey}", name="nm2")
            nc.vector.scalar_tensor_tensor(out=nm2, in0=mn, scalar=-1.0,
                                           in1=rstd2, op0=OP.mult, op1=OP.mult)
            return rstd2, nm2

        def mm2_resid(q, X, ytok, s1, e2, pz_pool):
            pz = pz_pool.tile([P, C], FP32, tag="pz", name="pz")
            for j in range(NO_TOK):
                nc.tensor.matmul(pz, tlw[j][:, q * P:(q + 1) * P], ytok[j],
                                 start=(j == 0), stop=(j == NO_TOK - 1))
            xt = x1pool.tile([P, C], BF16, tag=f"x1_{q}", name=f"x1_{q}")
            nc.vector.scalar_tensor_tensor(out=xt, in0=pz,
                                           scalar=btok[:, q:q + 1],
                                           in1=X[q], op0=OP.add, op1=OP.add,
                                           accum_out=s1[:, q:q + 1])
            sq = scratch.tile([P, C], FP32, tag="sq", name="sq")
            nc.vector.scalar_tensor_tensor(out=sq, in0=xt, scalar=1.0, in1=xt,
                                           op0=OP.mult, op1=OP.mult,
                                           accum_out=e2[:, q:q + 1])
            return xt

        def normalize_xn2(q, x1q, rstd2, nm2, k):
            xq = ypool.tile([P, C], BF16, tag=f"xn2_{q}", name=f"xn2_{q}", bufs=1)
            if apply_ln2:
                tmp = scratch.tile([P, C], FP32, tag="lntmp", name="lntmp")
                nc.scalar.activation(out=tmp, in_=x1q, func=AF.Identity,
                                     bias=nm2[:, k:k + 1], scale=rstd2[:, k:k + 1])
                nc.vector.tensor_mul(out=tmp, in0=tmp, in1=lnwb[:, 2, :])
                nc.vector.scalar_tensor_tensor(out=xq, in0=tmp, scalar=1.0,
                                               in1=lnwb[:, 3, :], op0=OP.mult,
                                               op1=OP.add)
            else:
                nc.scalar.activation(out=xq, in_=x1q, func=AF.Identity,
                                     bias=nm2[:, k:k + 1], scale=rstd2[:, k:k + 1])
            return xq

        def alloc_tr():
            # c-tiles m packed 2 per PSUM bank (the fp32 bank tile is viewed
            # as [P, 2C] bf16 so the tag matches the pz allocations rotating
            # through the same 2 banks)
            return [trpz.tile([P, C], FP32, tag="pz", name=f"tr{h}").bitcast(BF16)
                    for h in range(2)]

        def transpose_i(tr, xn2i, i):
            for m in range(NC_):
                nc.tensor.transpose(
                    tr[m // 2][:, (m % 2) * C + i * P:(m % 2) * C + (i + 1) * P],
                    xn2i[:, m * P:(m + 1) * P], ident)

        def ytok_copies(ptok):
            ytok = []
            for j in range(NO_TOK):
                ysb = ypool.tile([P, C], BF16, tag=f"ytok{j}", name="ysb")
                nc.scalar.copy(out=ysb, in_=ptok[j])
                ytok.append(ysb)
            return ytok

        def token_out(b, X, ytok):
            """mm2, residual+LN2 stats, normalize -> xn2."""
            s1 = stats.tile([P, NT], FP32, tag="s1", name="s1")
            e2 = stats.tile([P, NT], FP32, tag="e2", name="e2")
            x1 = [mm2_resid(q, X, ytok, s1, e2, pza if q < 2 else trpz)
                  for q in range(NT)]
            rstd2, nm2 = ln2_chain(s1, e2, NT, "")
            xn2 = [normalize_xn2(q, x1[q], rstd2, nm2, q) for q in range(NT)]
            return x1, xn2

        def token_out_last(b, X, ytok):
            """Last batch: no next-batch mm1 to cover the LN2 chain, so
            process pair-wise and interleave the transposes."""
            s1 = stats.tile([P, NT], FP32, tag="s1", name="s1")
            e2 = stats.tile([P, NT], FP32, tag="e2", name="e2")
            tr = alloc_tr()
            x1 = []
            for g in range(2):
                qs = (2 * g, 2 * g + 1)
                for q in qs:
                    x1.append(mm2_resid(q, X, ytok, s1, e2, pza))
                rstd2, nm2 = ln2_chain(s1[:, 2 * g:2 * g + 2],
                                       e2[:, 2 * g:2 * g + 2], 2, f"p{g}")
                for k, q in enumerate(qs):
                    xq = normalize_xn2(q, x1[q], rstd2, nm2, k)
                    transpose_i(tr, xq, q)
            return x1, tr

        def transposes(xn2):
            tr = alloc_tr()
            for i in range(NT):
                transpose_i(tr, xn2[i], i)
            return tr

        def channel_feats(tr):
            return [features_from_norm(tr[m // 2][:, (m % 2) * C:(m % 2 + 1) * C],
                                       f"c{m}", f2pool, "F2", m)
                    for m in range(NC_)]

        def mm3(fch):
            """q-major channel matmuls with the bias folded in via a
            ones-matmul; returns the 4 PSUM tiles."""
            pouts = []
            for q in range(NT):
                pout = poutp.tile([P, C], FP32, tag="pout", name=f"pout{q}")
                for m in range(NC_):
                    for f in range(NF):
                        nc.tensor.matmul(pout, fch[m][f][:, q * P:(q + 1) * P],
                                         wchf[m][:, f, :],
                                         start=(m == 0 and f == 0),
                                         stop=(m == NC_ - 1 and f == NF - 1))
                pouts.append(pout)
            return pouts

        def emit_out(b, pouts, x1, final=False):
            """residual add on DVE straight from PSUM, then store.  Emitted
            one iteration late so it never head-of-line-blocks the next
            batch's feature chain on the DVE FIFO.  For the final batch the
            stores split across queues so the tail transfer isn't bound by
            one ring's bandwidth."""
            for q in range(NT):
                ot = opool.tile([P, C], FP32, tag="out", name="out")
                nc.vector.tensor_tensor(out=ot, in0=pouts[q], in1=x1[q],
                                        op=OP.add)
                nc.vector.tensor_tensor(out=ot, in0=ot, in1=bch, op=OP.add)
                if final and q >= 2:
                    for hh in range(4):
                        nc.sync.dma_start(
                            out=y_out[b, q * P:(q + 1) * P,
                                      hh * (C // 4):(hh + 1) * (C // 4)],
                            in_=ot[:, hh * (C // 4):(hh + 1) * (C // 4)])
                else:
                    nc.sync.dma_start(out=y_out[b, q * P:(q + 1) * P, :], in_=ot)

        # ---- software-pipelined emission over batches ----
        # DMA order: X(0), wtok, [x(1)], then the big wchf -- the fill is
        # HBM-bandwidth-bound, so batch-0's dependencies go first.
        if apply_ln1:
            lnwb = singles.tile([P, 4, C], FP32, tag="lnwb")
            nc.sync.dma_start(out=lnwb, in_=ln_in.rearrange("p (k c) -> p k c", k=4))
        Xc, featsc = X0, stage1_prefetch(X0, gsize=2)
        Xmap = {0: X0}
        if NB > 1:
            Xmap[1] = X1
        if NB > 2:
            Xmap[2] = load_x(2)
        wchf, bch, lnwb2 = load_weights_late()
        if not apply_ln1:
            lnwb = lnwb2
        ptokc = [ptokp.tile([P, C], FP32, tag="ptok", name=f"ptok{j}")
                 for j in range(NO_TOK)]
        mm1(featsc, ptokc, range(NT))
        pending_out = None
        hoisted = None   # last batch's token stage, pre-emitted an iteration early
        for b in range(NB):
            X, feats, ptok = Xc, featsc, ptokc
            # b=0: the next batch's Sin chain goes ahead of the ytok copies
            # on the ACT queue -- the copies wait for mm1(0) anyway, while
            # the sins' inputs are ready earlier (kills the ramp gaps).
            # Steady state: copies first (they gate mm2 on the PE; the sins'
            # inputs arrive early under the mm3 cover).
            if b == 0 and b + 1 < NB:
                Xc = Xmap[b + 1]
                featsc = stage1_prefetch(Xc, gsize=2)
                ytok = ytok_copies(ptok)
                if b + 2 < NB and (b + 2) not in Xmap:
                    Xmap[b + 2] = load_x(b + 2)
            elif hoisted is None:
                ytok = ytok_copies(ptok)
                if b + 1 < NB:
                    Xc = Xmap[b + 1]
                    featsc = stage1_prefetch(Xc, gsize=NT)
                    if b + 2 < NB and (b + 2) not in Xmap:
                        Xmap[b + 2] = load_x(b + 2)
            if b + 1 < NB:
                x1, xn2 = token_out(b, X, ytok)
                if pending_out is not None:
                    emit_out(b - 1, *pending_out)
                ptokc = [ptokp.tile([P, C], FP32, tag="ptok", name=f"ptok{j}")
                         for j in range(NO_TOK)]
                mm1(featsc, ptokc, (0, 1))
                tr = transposes(xn2)
                mm1(featsc, ptokc, (2, 3))
                fch = channel_feats(tr)
                if b + 1 == NB - 1:
                    # hoist the final batch's token stage (ytok, mm2, LN2,
                    # normalize) ahead of this mm3 so its serial chain runs
                    # under the 21us matmul cover instead of stalling the PE
                    ytok_l = ytok_copies(ptokc)
                    hoisted = token_out(b + 1, Xc, ytok_l)
            else:
                if hoisted is not None:
                    x1, xn2 = hoisted
                    tr = transposes(xn2)
                else:
                    x1, tr = token_out_last(b, X, ytok)
                if pending_out is not None:
                    emit_out(b - 1, *pending_out)
                fch = channel_feats(tr)
            pending_out = (mm3(fch), x1)
        emit_out(NB - 1, *pending_out, final=True)

    _split_multi_waits(nc)
    return nc


_CACHE = {}


def _get_nc(apply_ln1, apply_ln2):
    key = (apply_ln1, apply_ln2)
    if key not in _CACHE:
        _CACHE[key] = _build(apply_ln1, apply_ln2)
    return _CACHE[key]


def prepare_in_maps(inputs):
    return _prepare(**inputs)


def _prepare(x, ln1_w, ln1_b, tok_coef, tok_kbias, tok_lw, tok_lb,
             ln2_w, ln2_b, ch_coef, ch_kbias, ch_lw, ch_lb):
    x = np.asarray(x, np.float32)
    f64 = np.float64

    wtok_eff, tok_const = _cheb_weights(np.asarray(tok_coef, f64))  # (T,6,TD)
    wch_eff, ch_const = _cheb_weights(np.asarray(ch_coef, f64))     # (C,6,2C)

    kbias_tok = np.asarray(tok_kbias, f64).reshape(-1) + tok_const
    kbias_ch = np.asarray(ch_kbias, f64).reshape(-1) + ch_const
    bias_tok = np.asarray(tok_lb, f64) + np.asarray(tok_lw, f64) @ kbias_tok
    bias_ch = np.asarray(ch_lb, f64) + np.asarray(ch_lw, f64) @ kbias_ch

    # fold the channel post-KAN linear into the KAN weights (fp64)
    wchf = np.einsum("cfo,ko->cfk", wch_eff, np.asarray(ch_lw, f64))  # (C,6,C)

    wtok_np = wtok_eff.reshape(NT, P, NF * TD).astype(ml_dtypes.bfloat16)
    wchf_np = wchf.reshape(NC_, P, NF * C).astype(ml_dtypes.bfloat16)
    tlw_np = np.ascontiguousarray(np.asarray(tok_lw, f64).T).reshape(
        NO_TOK, P, T).astype(ml_dtypes.bfloat16)
    btok_np = np.ascontiguousarray(bias_tok.reshape(NT, P).T).astype(np.float32)
    bch_np = np.broadcast_to(bias_ch.astype(ml_dtypes.bfloat16), (P, C)).copy()
    lnwb_np = np.broadcast_to(
        np.concatenate([np.asarray(ln1_w, f64), np.asarray(ln1_b, f64),
                        np.asarray(ln2_w, f64), np.asarray(ln2_b, f64)]).astype(
            np.float32), (P, 4 * C)).copy()

    apply_ln1 = not (np.all(np.asarray(ln1_w) == 1.0) and np.all(np.asarray(ln1_b) == 0.0))
    apply_ln2 = not (np.all(np.asarray(ln2_w) == 1.0) and np.all(np.asarray(ln2_b) == 0.0))

    shared = dict(wtok=wtok_np, wchf=wchf_np, tlw=tlw_np,
                  btok=btok_np, bch=bch_np, lnwb=lnwb_np)
    in_maps = []
    for core in range(NCORES):
        m = dict(shared)
        m["x"] = np.ascontiguousarray(x[core * NB:(core + 1) * NB])
        in_maps.append(m)
    return {"build_key": (apply_ln1, apply_ln2), "in_maps": in_maps}


def kernel(**inputs):
    prep = _prepare(**inputs)
    nc = _get_nc(*prep["build_key"])
    res = run_bass_kernel_spmd(nc, prep["in_maps"], list(range(NCORES)))
    return np.concatenate([res.results[i]["y"] for i in range(NCORES)], axis=0)


# revision 43
# speedup vs baseline: 1.0447x; 1.0126x over previous
"""MixerLayerKAN Trainium2 kernel.

x (B,T,C)=(32,512,512) fp32; token-mix FourierKAN(T->TD)+Linear, then
channel-mix FourierKAN(C->2C)+Linear, LN + residual around each.

Strategy (data-parallel over batch, 4 batches per NeuronCore, weights
replicated, no collectives):

* Fourier features cos(kx)/sin(kx), k=1..3, re-expressed in the product
  basis {s, c, s*c, s^2, s^3, c*s^2}; harmonic coefficients fold
  host-side into 6 effective weight matrices; the channel KAN further
  folds its post-KAN Linear (96 matmuls/batch instead of 224).
* LN1 normalize is folded into the feature chain: the range-wrap
  (round-to-int tensor_scalar + int32-input scalar_tensor_tensor, with
  per-partition scale/bias riding the scalar slots) and the Sin/Abs
  activations consume raw x directly -- no materialized normalized
  tensor and no int->float CAST on the token path.
* Transposes run in bf16 (1 cycle/row), two c-tiles packed per PSUM
  bank; the channel wrap chain reads transposed values straight from
  PSUM (no PSUM->SBUF copy ops).
* Each output tile is two DVE adds straight from PSUM (psum +
  residual, + channel bias) -- keeping the adds off the saturated PE
  and the copies off the scalar engine.
* PE FIFO is software-pipelined: mm1 of batch b+1 is split into two
  24-matmul chunks emitted around batch b's transposes, covering the
  two serial handoffs (LN2 chain, channel feature chain); mm3 runs
  q-major so the output adds overlap it; output DMAs trail one
  iteration so they never head-of-line-block the DVE FIFO.
* DMA: per-queue ring bandwidth is ~46GB/s, so batch-0's x tiles and
  the first token weights load as small split DMAs across rings, x is
  prefetched two batches ahead (before the big wchf load), and the
  final stores split across rings to shorten the drain.
* The last batch's token stage (ytok, mm2, LN2, normalize) is hoisted
  a full iteration early so its serial chain executes under the
  previous batch's 21us mm3 cover; only its transposes remain in the
  final iteration.

Measured ~190-192us/core (~233us baseline); note the part
power-throttles under sustained load, adding up to ~40us run-to-run.
"""

import numpy as np
import ml_dtypes

import concourse.bass as bass
import concourse.mybir as mybir
from concourse import tile
from concourse.bass_utils import run_bass_kernel_spmd
from concourse.masks import make_identity

AF = mybir.ActivationFunctionType
OP = mybir.AluOpType
FP32 = mybir.dt.float32
BF16 = mybir.dt.bfloat16
I32 = mybir.dt.int32

B, T, C, TD, G = 32, 512, 512, 256, 3
NCORES = 8
NB = B // NCORES          # batches per core
P = 128
EPS = 1e-5
PI = float(np.pi)
TWO_PI = float(2 * np.pi)
INV_2PI = float(1.0 / (2 * np.pi))
FOUR_PI = float(4 * np.pi)
NF = 6                    # product-basis features
NT = T // P               # 4 t-tiles
NC_ = C // P              # 4 c-tiles
NO_TOK = TD // P          # 2 token KAN hidden tiles


def _split_multi_waits(nc):
    """This walrus build accepts at most ONE sync-wait command per
    instruction.  Tile emits several.  Fix: before each multi-wait
    instruction, splice in same-engine NOPs carrying one wait each (a wait
    executed earlier on the same engine is semantically identical)."""
    f = nc.m.functions[0]
    per_engine = {}
    for bb in f.blocks:
        for inst in bb.instructions:
            si = getattr(inst, "sync_info", None)
            if si is not None and si.on_wait and len(si.on_wait) > 1:
                per_engine[inst.engine] = per_engine.get(inst.engine, 0) + (
                    len(si.on_wait) - 1)
    if not per_engine:
        return
    nop_pool = {}
    for eng, cnt in per_engine.items():
        nop_pool[eng] = [nc.engines[eng].nop(nofuse=True).ins for _ in range(cnt)]
    created = {id(i) for h in nop_pool.values() for i in h}
    for bb in f.blocks:
        bb.instructions[:] = [i for i in bb.instructions if id(i) not in created]
    for bb in f.blocks:
        out = []
        for inst in bb.instructions:
            si = getattr(inst, "sync_info", None)
            if si is not None and si.on_wait and len(si.on_wait) > 1:
                waits = list(si.on_wait)
                si.on_wait = [waits[-1]]
                for w in waits[:-1]:
                    nop = nop_pool[inst.engine].pop()
                    nop.sync_info = mybir.SyncInfo(on_wait=[w], on_update=[])
                    out.append(nop)
            out.append(inst)
        bb.instructions[:] = out


def _cheb_weights(coef):
    """coef (2, O, I, G) -> effective basis weights (I, 6, O) for the
    {s, c, s*c, s^2, s^3, c*s^2} basis, plus the constant term (O,).

    cos(1x)=c; cos(2x)=1-2s^2; cos(3x)=c-4c s^2
    sin(1x)=s; sin(2x)=2 s c ; sin(3x)=3s-4s^3
    """
    cosw = coef[0]
    sinw = coef[1]
    O, I, _ = cosw.shape
    w = np.zeros((I, NF, O), np.float64)
    w[:, 0, :] = (sinw[:, :, 0] + 3.0 * sinw[:, :, 2]).T      # s
    w[:, 1, :] = (cosw[:, :, 0] + cosw[:, :, 2]).T            # c
    w[:, 2, :] = (2.0 * sinw[:, :, 1]).T                      # s*c
    w[:, 3, :] = (-2.0 * cosw[:, :, 1]).T                     # s^2
    w[:, 4, :] = (-4.0 * sinw[:, :, 2]).T                     # s^3
    w[:, 5, :] = (-4.0 * cosw[:, :, 2]).T                     # c*s^2
    const = cosw[:, :, 1].sum(axis=1)                         # from the "1" of cos(2x)
    return w, const


def _build(apply_ln1, apply_ln2):
    nc = bass.Bass()
    x_in = nc.dram_tensor("x", [NB, T, C], FP32, kind="ExternalInput")
    y_out = nc.dram_tensor("y", [NB, T, C], FP32, kind="ExternalOutput")
    wtok_in = nc.dram_tensor("wtok", [NT, P, NF * TD], BF16, kind="ExternalInput")
    wchf_in = nc.dram_tensor("wchf", [NC_, P, NF * C], BF16, kind="ExternalInput")
    tlw_in = nc.dram_tensor("tlw", [NO_TOK, P, T], BF16, kind="ExternalInput")
    btok_in = nc.dram_tensor("btok", [P, NT], FP32, kind="ExternalInput")
    bch_in = nc.dram_tensor("bch", [P, C], BF16, kind="ExternalInput")
    ln_in = nc.dram_tensor("lnwb", [P, 4 * C], FP32, kind="ExternalInput")

    with tile.TileContext(nc) as tc, \
         tc.tile_pool(name="singles", bufs=1) as singles, \
         tc.tile_pool(name="xpool", bufs=3) as xpool, \
         tc.tile_pool(name="fpool", bufs=2) as fpool, \
         tc.tile_pool(name="f2pool", bufs=1) as f2pool, \
         tc.tile_pool(name="scratch", bufs=1) as scratch, \
         tc.tile_pool(name="ypool", bufs=2) as ypool, \
         tc.tile_pool(name="x1pool", bufs=2) as x1pool, \
         tc.tile_pool(name="opool", bufs=8) as opool, \
         tc.tile_pool(name="stats", bufs=2) as stats, \
         tc.tile_pool(name="ptokp", bufs=2, space="PSUM") as ptokp, \
         tc.tile_pool(name="pza", bufs=2, space="PSUM") as pza, \
         tc.tile_pool(name="trpz", bufs=2, space="PSUM") as trpz, \
         tc.tile_pool(name="poutp", bufs=2, space="PSUM") as poutp:

        # ---- batch-0 x first so the big weight DMAs don't block start ----
        def load_x(b, nsplit=2):
            xt = xpool.tile([P, NT, C], FP32, tag="X", name=f"X{b}")
            step = NT // nsplit
            for i in range(0, NT, step):
                nc.sync.dma_start(
                    out=xt[:, i:i + step, :],
                    in_=x_in[b, i * P:(i + step) * P, :].rearrange(
                        "(i p) c -> p i c", p=P))
            return [xt[:, i, :] for i in range(NT)]

        # batch-0: per-tile DMAs interleaved with per-tile wtok loads so
        # tile-0 stats and the first mm1 weights arrive ASAP
        X0t = xpool.tile([P, NT, C], FP32, tag="X", name="X0")
        wtok_all = singles.tile([P, NT, NF, TD], BF16, tag="wtok")
        H = C // 2
        for i in (0, 1):
            for hh in (0, 1):
                nc.sync.dma_start(out=X0t[:, i, hh * H:(hh + 1) * H],
                                  in_=x_in[0, i * P:(i + 1) * P, hh * H:(hh + 1) * H])
        wtok0r = wtok_in[0].rearrange("p (f o) -> p f o", f=NF)
        nc.sync.dma_start(out=wtok_all[:, 0, 0:NF // 2], in_=wtok0r[:, 0:NF // 2])
        nc.sync.dma_start(out=wtok_all[:, 0, NF // 2:], in_=wtok0r[:, NF // 2:])
        for i in (2, 3):
            nc.sync.dma_start(out=X0t[:, i, :], in_=x_in[0, i * P:(i + 1) * P, :])
        for i in range(1, NT):
            nc.sync.dma_start(out=wtok_all[:, i],
                              in_=wtok_in[i].rearrange("p (f o) -> p f o", f=NF))
        X1 = load_x(1) if NB > 1 else None
        X0 = [X0t[:, i, :] for i in range(NT)]
        wtok = [wtok_all[:, i] for i in range(NT)]

        ident = singles.tile([P, P], BF16, tag="ident")
        make_identity(nc, ident)
        ones128 = singles.tile([P, P], BF16, tag="ones128")
        nc.vector.memset(ones128, float(1.0 / 128.0))
        halfpi = singles.tile([P, 1], FP32, tag="halfpi")
        nc.vector.memset(halfpi, PI / 2)
        actwarm = singles.tile([P, 1], FP32, tag="actwarm")
        nc.scalar.activation(out=actwarm, in_=halfpi, func=AF.Sin)
        tlw = []
        for j in range(NO_TOK):
            t_ = singles.tile([P, T], BF16, tag=f"tlw{j}")
            nc.sync.dma_start(out=t_, in_=tlw_in[j])
            tlw.append(t_)
        btok = singles.tile([P, NT], FP32, tag="btok")
        nc.sync.dma_start(out=btok, in_=btok_in[:, :])

        def load_weights_late():
            wchf_all = singles.tile([P, NC_, NF, C], BF16, tag="wchf")
            nc.sync.dma_start(out=wchf_all,
                              in_=wchf_in.rearrange("m p (f o) -> p m f o", f=NF))
            wchf = [wchf_all[:, m] for m in range(NC_)]
            bch = singles.tile([P, C], BF16, tag="bch")
            nc.sync.dma_start(out=bch, in_=bch_in[:, :])
            lnwb = None
            if apply_ln2 and not apply_ln1:
                lnwb = singles.tile([P, 4, C], FP32, tag="lnwb")
                nc.sync.dma_start(out=lnwb, in_=ln_in.rearrange("p (k c) -> p k c", k=4))
            return wchf, bch, lnwb

        # ---- helpers ----
        def ln1_stats(X, tiles, gkey):
            """bn stats + rsqrt Newton + derived wrap scalars for a group of
            tiles.  Returns {tile: (rstd, nb, aa, bv, cc2) [P,1] slices}."""
            n = len(tiles)
            mvs = stats.tile([P, n, 2], FP32, tag=f"mvs{gkey}", name="mvs")
            for k, i in enumerate(tiles):
                st6 = stats.tile([P, 6], FP32, tag=f"st6_{i % 2}", name="st6")
                nc.vector.bn_stats(out=st6, in_=X[i])
                nc.vector.bn_aggr(out=mvs[:, k, :], in_=st6)
            mean = mvs[:, :, 0]
            var = mvs[:, :, 1]
            h = stats.tile([P, n], FP32, tag=f"h{gkey}", name="h")
            nc.vector.tensor_scalar(out=h, in0=var, scalar1=EPS, scalar2=-0.5,
                                    op0=OP.add, op1=OP.mult)
            yi = stats.tile([P, n], I32, tag=f"yi{gkey}", name="yi")
            nc.vector.tensor_scalar(out=yi, in0=var.bitcast(I32), scalar1=1,
                                    scalar2=None, op0=OP.logical_shift_right)
            nc.vector.tensor_scalar(out=yi, in0=yi, scalar1=-1,
                                    scalar2=0x5F3759DF, op0=OP.mult, op1=OP.add)
            rstd = yi.bitcast(FP32)
            a2 = stats.tile([P, n], FP32, tag=f"a2{gkey}", name="a2")
            for _ in range(2):
                nc.vector.tensor_tensor(out=a2, in0=rstd, in1=rstd, op=OP.mult)
                nc.vector.tensor_tensor(out=a2, in0=a2, in1=h, op=OP.mult)
                nc.vector.scalar_tensor_tensor(out=rstd, in0=a2, scalar=1.5,
                                               in1=rstd, op0=OP.add, op1=OP.mult)
            nb = stats.tile([P, n], FP32, tag=f"nb{gkey}", name="nb")
            nc.vector.scalar_tensor_tensor(out=nb, in0=mean, scalar=-1.0, in1=rstd,
                                           op0=OP.mult, op1=OP.mult)
            aa = stats.tile([P, n], FP32, tag=f"aa{gkey}", name="aa")
            nc.vector.tensor_scalar(out=aa, in0=rstd, scalar1=INV_2PI, scalar2=None,
                                    op0=OP.mult)
            bv = stats.tile([P, n], FP32, tag=f"bv{gkey}", name="bv")
            nc.vector.tensor_scalar(out=bv, in0=nb, scalar1=INV_2PI, scalar2=None,
                                    op0=OP.mult)
            cc2 = stats.tile([P, n], FP32, tag=f"cc2{gkey}", name="cc2")
            nc.vector.scalar_tensor_tensor(out=cc2, in0=h, scalar=FOUR_PI, in1=rstd,
                                           op0=OP.mult, op1=OP.mult)
            return {i: tuple(t[:, k:k + 1] for t in (rstd, nb, aa, bv, cc2))
                    for k, i in enumerate(tiles)}

        def feat_tiles(pool, pref, i):
            return [pool.tile([P, C], BF16, tag=f"{pref}_{i}_{k}", name=f"{pref}{i}b{k}")
                    for k in range(NF)]

        def features_from_x(xt, i, rstd, nb, aa, bv, cc2, pref):
            """Token-path features straight from raw x (LN folded in)."""
            if apply_ln1:
                # general path: materialize normalized tensor, then wrap
                xn = scratch.tile([P, C], FP32, tag=f"xn{i % 2}", name="xn")
                nc.scalar.activation(out=xn, in_=xt, func=AF.Identity,
                                     bias=nb, scale=rstd)
                nc.vector.tensor_mul(out=xn, in0=xn, in1=lnwb[:, 0, :])
                nc.vector.tensor_add(out=xn, in0=xn, in1=lnwb[:, 1, :])
                return features_from_norm(xn, f"F1_{i}", fpool, pref, i)
            f = feat_tiles(fpool, pref, i)
            ni = scratch.tile([P, C], I32, tag=f"ni1_{i % 2}", name="ni")
            nc.vector.tensor_scalar(out=ni, in0=xt, scalar1=aa,
                                    scalar2=bv, op0=OP.mult, op1=OP.add)
            rt = scratch.tile([P, C], FP32, tag=f"rt1_{i % 2}", name="rt")
            nc.vector.scalar_tensor_tensor(out=rt, in0=ni, scalar=cc2,
                                           in1=xt, op0=OP.mult, op1=OP.add)
            nc.scalar.activation(out=f[0], in_=rt, func=AF.Sin,
                                 scale=rstd, bias=nb)
            ab = scratch.tile([P, C], FP32, tag=f"ab1_{i % 2}", name="ab")
            nc.scalar.activation(out=ab, in_=rt, func=AF.Abs,
                                 scale=rstd, bias=nb)
            nc.scalar.activation(out=f[1], in_=ab, func=AF.Sin, scale=-1.0,
                                 bias=halfpi[:, :])
            nc.vector.tensor_mul(out=f[3], in0=f[0], in1=f[0])   # ss
            nc.vector.tensor_mul(out=f[2], in0=f[0], in1=f[1])   # sc
            nc.vector.tensor_mul(out=f[4], in0=f[3], in1=f[0])   # sss
            nc.vector.tensor_mul(out=f[5], in0=f[3], in1=f[1])   # css
            return f

        def features_from_norm(src, key, pool, pref, i):
            """Channel-path features from an already-normalized source
            (SBUF tile or PSUM transpose slice)."""
            f = feat_tiles(pool, pref, i)
            ni = scratch.tile([P, C], I32, tag=f"ni_{key}" if apply_ln1 else f"ni2_{i % 2}",
                             name="ni")
            nc.vector.tensor_scalar(out=ni, in0=src, scalar1=INV_2PI, scalar2=None,
                                    op0=OP.mult)
            rt = scratch.tile([P, C], FP32, tag=f"rt_{key}" if apply_ln1 else f"rt2_{i % 2}",
                             name="rt")
            nc.vector.scalar_tensor_tensor(out=rt, in0=ni, scalar=-TWO_PI,
                                           in1=src, op0=OP.mult, op1=OP.add)
            nc.scalar.activation(out=f[0], in_=rt, func=AF.Sin)
            ab = scratch.tile([P, C], FP32, tag=f"ab_{key}" if apply_ln1 else f"ab2_{i % 2}",
                             name="ab")
            nc.scalar.activation(out=ab, in_=rt, func=AF.Abs)
            nc.scalar.activation(out=f[1], in_=ab, func=AF.Sin, scale=-1.0,
                                 bias=halfpi[:, :])
            nc.vector.tensor_mul(out=f[3], in0=f[0], in1=f[0])
            nc.vector.tensor_mul(out=f[2], in0=f[0], in1=f[1])
            nc.vector.tensor_mul(out=f[4], in0=f[3], in1=f[0])
            nc.vector.tensor_mul(out=f[5], in0=f[3], in1=f[1])
            return f

        def stage1_prefetch(X, gsize=NT, groups=None):
            """LN1 + token features for a batch whose x is already loading.
            Small group sizes start tile-0's feature chain earlier (used
            during the DMA-bound fill)."""
            if groups is None:
                groups = [tuple(range(g, g + gsize)) for g in range(0, NT, gsize)]
            feats = [None] * NT
            for g, tiles in enumerate(groups):
                sc = ln1_stats(X, tiles, f"{len(tiles)}_{g % 2}")
                for i in tiles:
                    feats[i] = features_from_x(X[i], i, *sc[i], "F1")
            return feats

        def mm1(feats, ptok, tiles):
            for i in tiles:
                for j in range(NO_TOK):
                    for f in range(NF):
                        nc.tensor.matmul(ptok[j], wtok[i][:, f, j * P:(j + 1) * P],
                                         feats[i][f], start=(i == 0 and f == 0),
                                         stop=(i == NT - 1 and f == NF - 1))

        def ln2_chain(s1, e2, n, gkey):
            """LN2 rsqrt chain on DVE (latency-critical for the transposes).
            s1/e2 [P,n] slices -> (rstd2, nm2) [P,n]."""
            mn = stats.tile([P, n], FP32, tag=f"mn{gkey}", name="mn")
            nc.vector.tensor_scalar_mul(out=mn, in0=s1, scalar1=1.0 / C)
            vr = stats.tile([P, n], FP32, tag=f"vr{gkey}", name="vr")
            nc.vector.tensor_mul(out=vr, in0=mn, in1=mn)
            nc.vector.scalar_tensor_tensor(out=vr, in0=e2, scalar=1.0 / C, in1=vr,
                                           op0=OP.mult, op1=OP.subtract)
            h2 = stats.tile([P, n], FP32, tag=f"h2{gkey}", name="h2")
            nc.vector.tensor_scalar(out=h2, in0=vr, scalar1=EPS, scalar2=-0.5,
                                    op0=OP.add, op1=OP.mult)
            yi2 = stats.tile([P, n], I32, tag=f"yi2{gkey}", name="yi2")
            nc.vector.tensor_scalar(out=yi2, in0=vr.bitcast(I32), scalar1=1,
                                    scalar2=None, op0=OP.logical_shift_right)
            nc.vector.tensor_scalar(out=yi2, in0=yi2, scalar1=-1,
                                    scalar2=0x5F3759DF, op0=OP.mult, op1=OP.add)
            rstd2 = yi2.bitcast(FP32)
            a2 = stats.tile([P, n], FP32, tag=f"a2b{gkey}", name="a2b")
            for _ in range(2):
                nc.vector.tensor_mul(out=a2, in0=rstd2, in1=rstd2)
                nc.vector.tensor_mul(out=a2, in0=a2, in1=h2)
                nc.vector.scalar_tensor_tensor(out=rstd2, in0=a2, scalar=1.5,
                                               in1=rstd2, op0=OP.add, op1=OP.mult)
            nm2 = stats.tile([P, n], FP32, tag=f"nm2{gkey}", name="nm2")
            nc.vector.scalar_tensor_tensor(out=nm2, in0=mn, scalar=-1.0,
                                           in1=rstd2, op0=OP.mult, op1=OP.mult)
            return rstd2, nm2

        def mm2_resid(q, X, ytok, s1, e2, pz_pool):
            pz = pz_pool.tile([P, C], FP32, tag="pz", name="pz")
            for j in range(NO_TOK):
                nc.tensor.matmul(pz, tlw[j][:, q * P:(q + 1) * P], ytok[j],
                                 start=(j == 0), stop=(j == NO_TOK - 1))
            xt = x1pool.tile([P, C], BF16, tag=f"x1_{q}", name=f"x1_{q}")
            nc.vector.scalar_tensor_tensor(out=xt, in0=pz,
                                           scalar=btok[:, q:q + 1],
                                           in1=X[q], op0=OP.add, op1=OP.add,
                                           accum_out=s1[:, q:q + 1])
            sq = scratch.tile([P, C], FP32, tag="sq", name="sq")
            nc.vector.scalar_tensor_tensor(out=sq, in0=xt, scalar=1.0, in1=xt,
                                           op0=OP.mult, op1=OP.mult,
                                           accum_out=e2[:, q:q + 1])
            return xt

        def normalize_xn2(q, x1q, rstd2, nm2, k):
            xq = ypool.tile([P, C], BF16, tag=f"xn2_{q}", name=f"xn2_{q}", bufs=1)
            if apply_ln2:
                tmp = scratch.tile([P, C], FP32, tag="lntmp", name="lntmp")
                nc.scalar.activation(out=tmp, in_=x1q, func=AF.Identity,
                                     bias=nm2[:, k:k + 1], scale=rstd2[:, k:k + 1])
                nc.vector.tensor_mul(out=tmp, in0=tmp, in1=lnwb[:, 2, :])
                nc.vector.scalar_tensor_tensor(out=xq, in0=tmp, scalar=1.0,
                                               in1=lnwb[:, 3, :], op0=OP.mult,
                                               op1=OP.add)
            else:
                nc.scalar.activation(out=xq, in_=x1q, func=AF.Identity,
                                     bias=nm2[:, k:k + 1], scale=rstd2[:, k:k + 1])
            return xq

        def alloc_tr():
            # c-tiles m packed 2 per PSUM bank (the fp32 bank tile is viewed
            # as [P, 2C] bf16 so the tag matches the pz allocations rotating
            # through the same 2 banks)
            return [trpz.tile([P, C], FP32, tag="pz", name=f"tr{h}").bitcast(BF16)
                    for h in range(2)]

        def transpose_i(tr, xn2i, i):
            for m in range(NC_):
                nc.tensor.transpose(
                    tr[m // 2][:, (m % 2) * C + i * P:(m % 2) * C + (i + 1) * P],
                    xn2i[:, m * P:(m + 1) * P], ident)

        def ytok_copies(ptok):
            ytok = []
            for j in range(NO_TOK):
                ysb = ypool.tile([P, C], BF16, tag=f"ytok{j}", name="ysb")
                nc.scalar.copy(out=ysb, in_=ptok[j])
                ytok.append(ysb)
            return ytok

        def token_out(b, X, ytok):
            """mm2, residual+LN2 stats, normalize -> xn2."""
            s1 = stats.tile([P, NT], FP32, tag="s1", name="s1")
            e2 = stats.tile([P, NT], FP32, tag="e2", name="e2")
            x1 = [mm2_resid(q, X, ytok, s1, e2, pza if q < 2 else trpz)
                  for q in range(NT)]
            rstd2, nm2 = ln2_chain(s1, e2, NT, "")
            xn2 = [normalize_xn2(q, x1[q], rstd2, nm2, q) for q in range(NT)]
            return x1, xn2

        def token_out_last(b, X, ytok):
            """Last batch: no next-batch mm1 to cover the LN2 chain, so
            process pair-wise and interleave the transposes."""
            s1 = stats.tile([P, NT], FP32, tag="s1", name="s1")
            e2 = stats.tile([P, NT], FP32, tag="e2", name="e2")
            tr = alloc_tr()
            x1 = []
            for g in range(2):
                qs = (2 * g, 2 * g + 1)
                for q in qs:
                    x1.append(mm2_resid(q, X, ytok, s1, e2, pza))
                rstd2, nm2 = ln2_chain(s1[:, 2 * g:2 * g + 2],
                                       e2[:, 2 * g:2 * g + 2], 2, f"p{g}")
                for k, q in enumerate(qs):
                    xq = normalize_xn2(q, x1[q], rstd2, nm2, k)
                    transpose_i(tr, xq, q)
            return x1, tr

        def transposes(xn2):
            tr = alloc_tr()
            for i in range(NT):
                transpose_i(tr, xn2[i], i)
            return tr

        def channel_feats(tr):
            return [features_from_norm(tr[m // 2][:, (m % 2) * C:(m % 2 + 1) * C],
                                       f"c{m}", f2pool, "F2", m)
                    for m in range(NC_)]

        def mm3(fch):
            """q-major channel matmuls with the bias folded in via a
            ones-matmul; returns the 4 PSUM tiles."""
            pouts = []
            for q in range(NT):
                pout = poutp.tile([P, C], FP32, tag="pout", name=f"pout{q}")
                for m in range(NC_):
                    for f in range(NF):
                        nc.tensor.matmul(pout, fch[m][f][:, q * P:(q + 1) * P],
                                         wchf[m][:, f, :],
                                         start=(m == 0 and f == 0),
                                         stop=(m == NC_ - 1 and f == NF - 1))
                pouts.append(pout)
            return pouts

        def emit_out(b, pouts, x1, final=False):
            """residual add on DVE straight from PSUM, then store.  Emitted
            one iteration late so it never head-of-line-blocks the next
            batch's feature chain on the DVE FIFO.  For the final batch the
            stores split across queues so the tail transfer isn't bound by
            one ring's bandwidth."""
            for q in range(NT):
                ot = opool.tile([P, C], FP32, tag="out", name="out")
                nc.vector.tensor_tensor(out=ot, in0=pouts[q], in1=x1[q],
                                        op=OP.add)
                nc.vector.tensor_tensor(out=ot, in0=ot, in1=bch, op=OP.add)
                if final and q >= 2:
                    for hh in range(4):
                        nc.sync.dma_start(
                            out=y_out[b, q * P:(q + 1) * P,
                                      hh * (C // 4):(hh + 1) * (C // 4)],
                            in_=ot[:, hh * (C // 4):(hh + 1) * (C // 4)])
                else:
                    nc.sync.dma_start(out=y_out[b, q * P:(q + 1) * P, :], in_=ot)

        # ---- software-pipelined emission over batches ----
        # DMA order: X(0), wtok, [x(1)], then the big wchf -- the fill is
        # HBM-bandwidth-bound, so batch-0's dependencies go first.
        if apply_ln1:
            lnwb = singles.tile([P, 4, C], FP32, tag="lnwb")
            nc.sync.dma_start(out=lnwb, in_=ln_in.rearrange("p (k c) -> p k c", k=4))
        Xc, featsc = X0, stage1_prefetch(X0, groups=[(0,), (1,), (2, 3)])
        Xmap = {0: X0}
        if NB > 1:
            Xmap[1] = X1
        if NB > 2:
            Xmap[2] = load_x(2)
        wchf, bch, lnwb2 = load_weights_late()
        if not apply_ln1:
            lnwb = lnwb2
        ptokc = [ptokp.tile([P, C], FP32, tag="ptok", name=f"ptok{j}")
                 for j in range(NO_TOK)]
        mm1(featsc, ptokc, range(NT))
        pending_out = None
        hoisted = None   # last batch's token stage, pre-emitted an iteration early
        for b in range(NB):
            X, feats, ptok = Xc, featsc, ptokc
            # b=0: the next batch's Sin chain goes ahead of the ytok copies
            # on the ACT queue -- the copies wait for mm1(0) anyway, while
            # the sins' inputs are ready earlier (kills the ramp gaps).
            # Steady state: copies first (they gate mm2 on the PE; the sins'
            # inputs arrive early under the mm3 cover).
            if b == 0 and b + 1 < NB:
                Xc = Xmap[b + 1]
                featsc = stage1_prefetch(Xc, gsize=2)
                ytok = ytok_copies(ptok)
                if b + 2 < NB and (b + 2) not in Xmap:
                    Xmap[b + 2] = load_x(b + 2)
            elif hoisted is None:
                ytok = ytok_copies(ptok)
                if b + 1 < NB:
                    Xc = Xmap[b + 1]
                    featsc = stage1_prefetch(Xc, gsize=NT)
                    if b + 2 < NB and (b + 2) not in Xmap:
                        Xmap[b + 2] = load_x(b + 2)
            if b + 1 < NB:
                x1, xn2 = token_out(b, X, ytok)
                if pending_out is not None:
                    emit_out(b - 1, *pending_out)
                ptokc = [ptokp.tile([P, C], FP32, tag="ptok", name=f"ptok{j}")
                         for j in range(NO_TOK)]
                mm1(featsc, ptokc, (0, 1))
                tr = transposes(xn2)
                mm1(featsc, ptokc, (2, 3))
                fch = channel_feats(tr)
                if b + 1 == NB - 1:
                    # hoist the final batch's token stage (ytok, mm2, LN2,
                    # normalize) ahead of this mm3 so its serial chain runs
                    # under the 21us matmul cover instead of stalling the PE
                    ytok_l = ytok_copies(ptokc)
                    hoisted = token_out(b + 1, Xc, ytok_l)
            else:
                if hoisted is not None:
                    x1, xn2 = hoisted
                    tr = transposes(xn2)
                else:
                    x1, tr = token_out_last(b, X, ytok)
                if pending_out is not None:
                    emit_out(b - 1, *pending_out)
                fch = channel_feats(tr)
            pending_out = (mm3(fch), x1)
        emit_out(NB - 1, *pending_out, final=True)

    _split_multi_waits(nc)
    return nc


_CACHE = {}


def _get_nc(apply_ln1, apply_ln2):
    key = (apply_ln1, apply_ln2)
    if key not in _CACHE:
        _CACHE[key] = _build(apply_ln1, apply_ln2)
    return _CACHE[key]


def prepare_in_maps(inputs):
    return _prepare(**inputs)


def _prepare(x, ln1_w, ln1_b, tok_coef, tok_kbias, tok_lw, tok_lb,
             ln2_w, ln2_b, ch_coef, ch_kbias, ch_lw, ch_lb):
    x = np.asarray(x, np.float32)
    f64 = np.float64

    wtok_eff, tok_const = _cheb_weights(np.asarray(tok_coef, f64))  # (T,6,TD)
    wch_eff, ch_const = _cheb_weights(np.asarray(ch_coef, f64))     # (C,6,2C)

    kbias_tok = np.asarray(tok_kbias, f64).reshape(-1) + tok_const
    kbias_ch = np.asarray(ch_kbias, f64).reshape(-1) + ch_const
    bias_tok = np.asarray(tok_lb, f64) + np.asarray(tok_lw, f64) @ kbias_tok
    bias_ch = np.asarray(ch_lb, f64) + np.asarray(ch_lw, f64) @ kbias_ch

    # fold the channel post-KAN linear into the KAN weights (fp64)
    wchf = np.einsum("cfo,ko->cfk", wch_eff, np.asarray(ch_lw, f64))  # (C,6,C)

    wtok_np = wtok_eff.reshape(NT, P, NF * TD).astype(ml_dtypes.bfloat16)
    wchf_np = wchf.reshape(NC_, P, NF * C).astype(ml_dtypes.bfloat16)
    tlw_np = np.ascontiguousarray(np.asarray(tok_lw, f64).T).reshape(
        NO_TOK, P, T).astype(ml_dtypes.bfloat16)
    btok_np = np.ascontiguousarray(bias_tok.reshape(NT, P).T).astype(np.float32)
    bch_np = np.broadcast_to(bias_ch.astype(ml_dtypes.bfloat16), (P, C)).copy()
    lnwb_np = np.broadcast_to(
        np.concatenate([np.asarray(ln1_w, f64), np.asarray(ln1_b, f64),
                        np.asarray(ln2_w, f64), np.asarray(ln2_b, f64)]).astype(
            np.float32), (P, 4 * C)).copy()

    apply_ln1 = not (np.all(np.asarray(ln1_w) == 1.0) and np.all(np.asarray(ln1_b) == 0.0))
    apply_ln2 = not (np.all(np.asarray(ln2_w) == 1.0) and np.all(np.asarray(ln2_b) == 0.0))

    shared = dict(wtok=wtok_np, wchf=wchf_np, tlw=tlw_np,
                  btok=btok_np, bch=bch_np, lnwb=lnwb_np)
    in_maps = []
    for core in range(NCORES):
        m = dict(shared)
        m["x"] = np.ascontiguousarray(x[core * NB:(core + 1) * NB])
        in_maps.append(m)
    return {"build_key": (apply_ln1, apply_ln2), "in_maps": in_maps}


def kernel(**inputs):
    prep = _prepare(**inputs)
    nc = _get_nc(*prep["build_key"])
    res = run_bass_kernel_spmd(nc, prep["in_maps"], list(range(NCORES)))
    return np.concatenate([res.results[i]["y"] for i in range(NCORES)], axis=0)


# revision 47
# speedup vs baseline: 1.0570x; 1.0117x over previous
"""MixerLayerKAN Trainium2 kernel.

x (B,T,C)=(32,512,512) fp32; token-mix FourierKAN(T->TD)+Linear, then
channel-mix FourierKAN(C->2C)+Linear, LN + residual around each.

Strategy (data-parallel over batch, 4 batches per NeuronCore, weights
replicated, no collectives):

* Fourier features cos(kx)/sin(kx), k=1..3, re-expressed in the product
  basis {s, c, s*c, s^2, s^3, c*s^2}; harmonic coefficients fold
  host-side into 6 effective weight matrices; the channel KAN further
  folds its post-KAN Linear (96 matmuls/batch instead of 224).
* LN1 normalize is folded into the feature chain: the range-wrap
  (round-to-int tensor_scalar + int32-input scalar_tensor_tensor, with
  per-partition scale/bias riding the scalar slots) and the Sin/Abs
  activations consume raw x directly -- no materialized normalized
  tensor and no int->float CAST on the token path.
* Transposes run in bf16 (1 cycle/row), two c-tiles packed per PSUM
  bank; the channel wrap chain reads transposed values straight from
  PSUM (no PSUM->SBUF copy ops).
* Each output tile is two DVE adds straight from PSUM (psum +
  residual, + channel bias) -- keeping the adds off the saturated PE
  and the copies off the scalar engine.
* PE FIFO is software-pipelined: mm1 of batch b+1 is split into two
  24-matmul chunks emitted around batch b's transposes, covering the
  two serial handoffs (LN2 chain, channel feature chain); mm3 runs
  q-major so the output adds overlap it; output DMAs trail one
  iteration so they never head-of-line-block the DVE FIFO.
* DMA: per-queue ring bandwidth is ~46GB/s, so batch-0's x tiles and
  the first token weights load as small split DMAs across rings, x is
  prefetched two batches ahead (before the big wchf load), and the
  final stores split across rings to shorten the drain.
* The last batch's token stage (ytok, mm2, LN2, normalize) is hoisted
  a full iteration early so its serial chain executes under the
  previous batch's 21us mm3 cover; only its transposes remain in the
  final iteration.

Measured ~188us/core (~233us baseline); note the part power-throttles
under sustained load, adding up to ~40us run-to-run.
"""

import numpy as np
import ml_dtypes

import concourse.bass as bass
import concourse.mybir as mybir
from concourse import tile
from concourse.bass_utils import run_bass_kernel_spmd
from concourse.masks import make_identity

AF = mybir.ActivationFunctionType
OP = mybir.AluOpType
FP32 = mybir.dt.float32
BF16 = mybir.dt.bfloat16
I32 = mybir.dt.int32

B, T, C, TD, G = 32, 512, 512, 256, 3
NCORES = 8
NB = B // NCORES          # batches per core
P = 128
EPS = 1e-5
PI = float(np.pi)
TWO_PI = float(2 * np.pi)
INV_2PI = float(1.0 / (2 * np.pi))
FOUR_PI = float(4 * np.pi)
NF = 6                    # product-basis features
NT = T // P               # 4 t-tiles
NC_ = C // P              # 4 c-tiles
NO_TOK = TD // P          # 2 token KAN hidden tiles


def _split_multi_waits(nc):
    """This walrus build accepts at most ONE sync-wait command per
    instruction.  Tile emits several.  Fix: before each multi-wait
    instruction, splice in same-engine NOPs carrying one wait each (a wait
    executed earlier on the same engine is semantically identical)."""
    f = nc.m.functions[0]
    per_engine = {}
    for bb in f.blocks:
        for inst in bb.instructions:
            si = getattr(inst, "sync_info", None)
            if si is not None and si.on_wait and len(si.on_wait) > 1:
                per_engine[inst.engine] = per_engine.get(inst.engine, 0) + (
                    len(si.on_wait) - 1)
    if not per_engine:
        return
    nop_pool = {}
    for eng, cnt in per_engine.items():
        nop_pool[eng] = [nc.engines[eng].nop(nofuse=True).ins for _ in range(cnt)]
    created = {id(i) for h in nop_pool.values() for i in h}
    for bb in f.blocks:
        bb.instructions[:] = [i for i in bb.instructions if id(i) not in created]
    for bb in f.blocks:
        out = []
        for inst in bb.instructions:
            si = getattr(inst, "sync_info", None)
            if si is not None and si.on_wait and len(si.on_wait) > 1:
                waits = list(si.on_wait)
                si.on_wait = [waits[-1]]
                for w in waits[:-1]:
                    nop = nop_pool[inst.engine].pop()
                    nop.sync_info = mybir.SyncInfo(on_wait=[w], on_update=[])
                    out.append(nop)
            out.append(inst)
        bb.instructions[:] = out


def _cheb_weights(coef):
    """coef (2, O, I, G) -> effective basis weights (I, 6, O) for the
    {s, c, s*c, s^2, s^3, c*s^2} basis, plus the constant term (O,).

    cos(1x)=c; cos(2x)=1-2s^2; cos(3x)=c-4c s^2
    sin(1x)=s; sin(2x)=2 s c ; sin(3x)=3s-4s^3
    """
    cosw = coef[0]
    sinw = coef[1]
    O, I, _ = cosw.shape
    w = np.zeros((I, NF, O), np.float64)
    w[:, 0, :] = (sinw[:, :, 0] + 3.0 * sinw[:, :, 2]).T      # s
    w[:, 1, :] = (cosw[:, :, 0] + cosw[:, :, 2]).T            # c
    w[:, 2, :] = (2.0 * sinw[:, :, 1]).T                      # s*c
    w[:, 3, :] = (-2.0 * cosw[:, :, 1]).T                     # s^2
    w[:, 4, :] = (-4.0 * sinw[:, :, 2]).T                     # s^3
    w[:, 5, :] = (-4.0 * cosw[:, :, 2]).T                     # c*s^2
    const = cosw[:, :, 1].sum(axis=1)                         # from the "1" of cos(2x)
    return w, const


def _build(apply_ln1, apply_ln2):
    nc = bass.Bass()
    x_in = nc.dram_tensor("x", [NB, T, C], FP32, kind="ExternalInput")
    y_out = nc.dram_tensor("y", [NB, T, C], FP32, kind="ExternalOutput")
    wtok_in = nc.dram_tensor("wtok", [NT, P, NF * TD], BF16, kind="ExternalInput")
    wchf_in = nc.dram_tensor("wchf", [NC_, P, NF * C], BF16, kind="ExternalInput")
    tlw_in = nc.dram_tensor("tlw", [NO_TOK, P, T], BF16, kind="ExternalInput")
    btok_in = nc.dram_tensor("btok", [P, NT], FP32, kind="ExternalInput")
    bch_in = nc.dram_tensor("bch", [P, C], BF16, kind="ExternalInput")
    ln_in = nc.dram_tensor("lnwb", [P, 4 * C], FP32, kind="ExternalInput")

    with tile.TileContext(nc) as tc, \
         tc.tile_pool(name="singles", bufs=1) as singles, \
         tc.tile_pool(name="xpool", bufs=3) as xpool, \
         tc.tile_pool(name="fpool", bufs=2) as fpool, \
         tc.tile_pool(name="f2pool", bufs=1) as f2pool, \
         tc.tile_pool(name="scratch", bufs=1) as scratch, \
         tc.tile_pool(name="ypool", bufs=2) as ypool, \
         tc.tile_pool(name="x1pool", bufs=2) as x1pool, \
         tc.tile_pool(name="opool", bufs=8) as opool, \
         tc.tile_pool(name="stats", bufs=2) as stats, \
         tc.tile_pool(name="ptokp", bufs=2, space="PSUM") as ptokp, \
         tc.tile_pool(name="pza", bufs=2, space="PSUM") as pza, \
         tc.tile_pool(name="trpz", bufs=2, space="PSUM") as trpz, \
         tc.tile_pool(name="poutp", bufs=2, space="PSUM") as poutp:

        # ---- batch-0 x first so the big weight DMAs don't block start ----
        def load_x(b, nsplit=2):
            xt = xpool.tile([P, NT, C], FP32, tag="X", name=f"X{b}")
            step = NT // nsplit
            for i in range(0, NT, step):
                nc.sync.dma_start(
                    out=xt[:, i:i + step, :],
                    in_=x_in[b, i * P:(i + step) * P, :].rearrange(
                        "(i p) c -> p i c", p=P))
            return [xt[:, i, :] for i in range(NT)]

        # batch-0: per-tile DMAs interleaved with per-tile wtok loads so
        # tile-0 stats and the first mm1 weights arrive ASAP
        X0t = xpool.tile([P, NT, C], FP32, tag="X", name="X0")
        wtok_all = singles.tile([P, NT, NF, TD], BF16, tag="wtok")
        H = C // 2
        for i in (0, 1):
            for hh in (0, 1):
                nc.sync.dma_start(out=X0t[:, i, hh * H:(hh + 1) * H],
                                  in_=x_in[0, i * P:(i + 1) * P, hh * H:(hh + 1) * H])
        wtok0r = wtok_in[0].rearrange("p (f o) -> p f o", f=NF)
        nc.sync.dma_start(out=wtok_all[:, 0, 0:NF // 2], in_=wtok0r[:, 0:NF // 2])
        nc.sync.dma_start(out=wtok_all[:, 0, NF // 2:], in_=wtok0r[:, NF // 2:])
        for i in (2, 3):
            nc.sync.dma_start(out=X0t[:, i, :], in_=x_in[0, i * P:(i + 1) * P, :])
        for i in range(1, NT):
            nc.sync.dma_start(out=wtok_all[:, i],
                              in_=wtok_in[i].rearrange("p (f o) -> p f o", f=NF))
        X1 = load_x(1) if NB > 1 else None
        X0 = [X0t[:, i, :] for i in range(NT)]
        wtok = [wtok_all[:, i] for i in range(NT)]

        ident = singles.tile([P, P], BF16, tag="ident")
        make_identity(nc, ident)
        ones128 = singles.tile([P, P], BF16, tag="ones128")
        nc.vector.memset(ones128, float(1.0 / 128.0))
        halfpi = singles.tile([P, 1], FP32, tag="halfpi")
        nc.vector.memset(halfpi, PI / 2)
        actwarm = singles.tile([P, 1], FP32, tag="actwarm")
        nc.scalar.activation(out=actwarm, in_=halfpi, func=AF.Sin)
        tlw = []
        for j in range(NO_TOK):
            t_ = singles.tile([P, T], BF16, tag=f"tlw{j}")
            nc.sync.dma_start(out=t_, in_=tlw_in[j])
            tlw.append(t_)
        btok = singles.tile([P, NT], FP32, tag="btok")
        nc.sync.dma_start(out=btok, in_=btok_in[:, :])

        def load_weights_late():
            wchf_all = singles.tile([P, NC_, NF, C], BF16, tag="wchf")
            nc.sync.dma_start(out=wchf_all,
                              in_=wchf_in.rearrange("m p (f o) -> p m f o", f=NF))
            wchf = [wchf_all[:, m] for m in range(NC_)]
            bch = singles.tile([P, C], BF16, tag="bch")
            nc.sync.dma_start(out=bch, in_=bch_in[:, :])
            lnwb = None
            if apply_ln2 and not apply_ln1:
                lnwb = singles.tile([P, 4, C], FP32, tag="lnwb")
                nc.sync.dma_start(out=lnwb, in_=ln_in.rearrange("p (k c) -> p k c", k=4))
            return wchf, bch, lnwb

        # ---- helpers ----
        def ln1_stats(X, tiles, gkey):
            """bn stats + rsqrt Newton + derived wrap scalars for a group of
            tiles.  Returns {tile: (rstd, nb, aa, bv, cc2) [P,1] slices}."""
            n = len(tiles)
            mvs = stats.tile([P, n, 2], FP32, tag=f"mvs{gkey}", name="mvs")
            for k, i in enumerate(tiles):
                st6 = stats.tile([P, 6], FP32, tag=f"st6_{i % 2}", name="st6")
                nc.vector.bn_stats(out=st6, in_=X[i])
                nc.vector.bn_aggr(out=mvs[:, k, :], in_=st6)
            mean = mvs[:, :, 0]
            var = mvs[:, :, 1]
            h = stats.tile([P, n], FP32, tag=f"h{gkey}", name="h")
            nc.vector.tensor_scalar(out=h, in0=var, scalar1=EPS, scalar2=-0.5,
                                    op0=OP.add, op1=OP.mult)
            yi = stats.tile([P, n], I32, tag=f"yi{gkey}", name="yi")
            nc.vector.tensor_scalar(out=yi, in0=var.bitcast(I32), scalar1=1,
                                    scalar2=None, op0=OP.logical_shift_right)
            nc.vector.tensor_scalar(out=yi, in0=yi, scalar1=-1,
                                    scalar2=0x5F3759DF, op0=OP.mult, op1=OP.add)
            rstd = yi.bitcast(FP32)
            a2 = stats.tile([P, n], FP32, tag=f"a2{gkey}", name="a2")
            for _ in range(2):
                nc.vector.tensor_tensor(out=a2, in0=rstd, in1=rstd, op=OP.mult)
                nc.vector.tensor_tensor(out=a2, in0=a2, in1=h, op=OP.mult)
                nc.vector.scalar_tensor_tensor(out=rstd, in0=a2, scalar=1.5,
                                               in1=rstd, op0=OP.add, op1=OP.mult)
            nb = stats.tile([P, n], FP32, tag=f"nb{gkey}", name="nb")
            nc.vector.scalar_tensor_tensor(out=nb, in0=mean, scalar=-1.0, in1=rstd,
                                           op0=OP.mult, op1=OP.mult)
            aa = stats.tile([P, n], FP32, tag=f"aa{gkey}", name="aa")
            nc.vector.tensor_scalar(out=aa, in0=rstd, scalar1=INV_2PI, scalar2=None,
                                    op0=OP.mult)
            bv = stats.tile([P, n], FP32, tag=f"bv{gkey}", name="bv")
            nc.vector.tensor_scalar(out=bv, in0=nb, scalar1=INV_2PI, scalar2=None,
                                    op0=OP.mult)
            cc2 = stats.tile([P, n], FP32, tag=f"cc2{gkey}", name="cc2")
            nc.vector.scalar_tensor_tensor(out=cc2, in0=h, scalar=FOUR_PI, in1=rstd,
                                           op0=OP.mult, op1=OP.mult)
            return {i: tuple(t[:, k:k + 1] for t in (rstd, nb, aa, bv, cc2))
                    for k, i in enumerate(tiles)}

        def feat_tiles(pool, pref, i):
            return [pool.tile([P, C], BF16, tag=f"{pref}_{i}_{k}", name=f"{pref}{i}b{k}")
                    for k in range(NF)]

        def features_from_x(xt, i, rstd, nb, aa, bv, cc2, pref):
            """Token-path features straight from raw x (LN folded in)."""
            if apply_ln1:
                # general path: materialize normalized tensor, then wrap
                xn = scratch.tile([P, C], FP32, tag=f"xn{i % 2}", name="xn")
                nc.scalar.activation(out=xn, in_=xt, func=AF.Identity,
                                     bias=nb, scale=rstd)
                nc.vector.tensor_mul(out=xn, in0=xn, in1=lnwb[:, 0, :])
                nc.vector.tensor_add(out=xn, in0=xn, in1=lnwb[:, 1, :])
                return features_from_norm(xn, f"F1_{i}", fpool, pref, i)
            f = feat_tiles(fpool, pref, i)
            ni = scratch.tile([P, C], I32, tag=f"ni1_{i % 2}", name="ni")
            nc.vector.tensor_scalar(out=ni, in0=xt, scalar1=aa,
                                    scalar2=bv, op0=OP.mult, op1=OP.add)
            rt = scratch.tile([P, C], FP32, tag=f"rt1_{i % 2}", name="rt")
            nc.vector.scalar_tensor_tensor(out=rt, in0=ni, scalar=cc2,
                                           in1=xt, op0=OP.mult, op1=OP.add)
            nc.scalar.activation(out=f[0], in_=rt, func=AF.Sin,
                                 scale=rstd, bias=nb)
            ab = scratch.tile([P, C], FP32, tag=f"ab1_{i % 2}", name="ab")
            nc.scalar.activation(out=ab, in_=rt, func=AF.Abs,
                                 scale=rstd, bias=nb)
            nc.scalar.activation(out=f[1], in_=ab, func=AF.Sin, scale=-1.0,
                                 bias=halfpi[:, :])
            nc.vector.tensor_mul(out=f[3], in0=f[0], in1=f[0])   # ss
            nc.vector.tensor_mul(out=f[2], in0=f[0], in1=f[1])   # sc
            nc.vector.tensor_mul(out=f[4], in0=f[3], in1=f[0])   # sss
            nc.vector.tensor_mul(out=f[5], in0=f[3], in1=f[1])   # css
            return f

        def features_from_norm(src, key, pool, pref, i):
            """Channel-path features from an already-normalized source
            (SBUF tile or PSUM transpose slice)."""
            f = feat_tiles(pool, pref, i)
            ni = scratch.tile([P, C], I32, tag=f"ni_{key}" if apply_ln1 else f"ni2_{i % 2}",
                             name="ni")
            nc.vector.tensor_scalar(out=ni, in0=src, scalar1=INV_2PI, scalar2=None,
                                    op0=OP.mult)
            rt = scratch.tile([P, C], FP32, tag=f"rt_{key}" if apply_ln1 else f"rt2_{i % 2}",
                             name="rt")
            nc.vector.scalar_tensor_tensor(out=rt, in0=ni, scalar=-TWO_PI,
                                           in1=src, op0=OP.mult, op1=OP.add)
            nc.scalar.activation(out=f[0], in_=rt, func=AF.Sin)
            ab = scratch.tile([P, C], FP32, tag=f"ab_{key}" if apply_ln1 else f"ab2_{i % 2}",
                             name="ab")
            nc.scalar.activation(out=ab, in_=rt, func=AF.Abs)
            nc.scalar.activation(out=f[1], in_=ab, func=AF.Sin, scale=-1.0,
                                 bias=halfpi[:, :])
            nc.vector.tensor_mul(out=f[3], in0=f[0], in1=f[0])
            nc.vector.tensor_mul(out=f[2], in0=f[0], in1=f[1])
            nc.vector.tensor_mul(out=f[4], in0=f[3], in1=f[0])
            nc.vector.tensor_mul(out=f[5], in0=f[3], in1=f[1])
            return f

        def stage1_prefetch(X, gsize=NT, groups=None):
            """LN1 + token features for a batch whose x is already loading.
            Small group sizes start tile-0's feature chain earlier (used
            during the DMA-bound fill)."""
            if groups is None:
                groups = [tuple(range(g, g + gsize)) for g in range(0, NT, gsize)]
            feats = [None] * NT
            for g, tiles in enumerate(groups):
                sc = ln1_stats(X, tiles, f"{len(tiles)}_{g % 2}")
                for i in tiles:
                    feats[i] = features_from_x(X[i], i, *sc[i], "F1")
            return feats

        def mm1(feats, ptok, tiles):
            for i in tiles:
                for j in range(NO_TOK):
                    for f in range(NF):
                        nc.tensor.matmul(ptok[j], wtok[i][:, f, j * P:(j + 1) * P],
                                         feats[i][f], start=(i == 0 and f == 0),
                                         stop=(i == NT - 1 and f == NF - 1))

        def ln2_chain(s1, e2, n, gkey):
            """LN2 rsqrt chain on DVE (latency-critical for the transposes).
            s1/e2 [P,n] slices -> (rstd2, nm2) [P,n]."""
            mn = stats.tile([P, n], FP32, tag=f"mn{gkey}", name="mn")
            nc.vector.tensor_scalar_mul(out=mn, in0=s1, scalar1=1.0 / C)
            vr = stats.tile([P, n], FP32, tag=f"vr{gkey}", name="vr")
            nc.vector.tensor_mul(out=vr, in0=mn, in1=mn)
            nc.vector.scalar_tensor_tensor(out=vr, in0=e2, scalar=1.0 / C, in1=vr,
                                           op0=OP.mult, op1=OP.subtract)
            h2 = stats.tile([P, n], FP32, tag=f"h2{gkey}", name="h2")
            nc.vector.tensor_scalar(out=h2, in0=vr, scalar1=EPS, scalar2=-0.5,
                                    op0=OP.add, op1=OP.mult)
            yi2 = stats.tile([P, n], I32, tag=f"yi2{gkey}", name="yi2")
            nc.vector.tensor_scalar(out=yi2, in0=vr.bitcast(I32), scalar1=1,
                                    scalar2=None, op0=OP.logical_shift_right)
            nc.vector.tensor_scalar(out=yi2, in0=yi2, scalar1=-1,
                                    scalar2=0x5F3759DF, op0=OP.mult, op1=OP.add)
            rstd2 = yi2.bitcast(FP32)
            a2 = stats.tile([P, n], FP32, tag=f"a2b{gkey}", name="a2b")
            for _ in range(2):
                nc.vector.tensor_mul(out=a2, in0=rstd2, in1=rstd2)
                nc.vector.tensor_mul(out=a2, in0=a2, in1=h2)
                nc.vector.scalar_tensor_tensor(out=rstd2, in0=a2, scalar=1.5,
                                               in1=rstd2, op0=OP.add, op1=OP.mult)
            nm2 = stats.tile([P, n], FP32, tag=f"nm2{gkey}", name="nm2")
            nc.vector.scalar_tensor_tensor(out=nm2, in0=mn, scalar=-1.0,
                                           in1=rstd2, op0=OP.mult, op1=OP.mult)
            return rstd2, nm2

        def mm2_resid(q, X, ytok, s1, e2, pz_pool):
            pz = pz_pool.tile([P, C], FP32, tag="pz", name="pz")
            for j in range(NO_TOK):
                nc.tensor.matmul(pz, tlw[j][:, q * P:(q + 1) * P], ytok[j],
                                 start=(j == 0), stop=(j == NO_TOK - 1))
            xt = x1pool.tile([P, C], BF16, tag=f"x1_{q}", name=f"x1_{q}")
            nc.vector.scalar_tensor_tensor(out=xt, in0=pz,
                                           scalar=btok[:, q:q + 1],
                                           in1=X[q], op0=OP.add, op1=OP.add,
                                           accum_out=s1[:, q:q + 1])
            sq = scratch.tile([P, C], FP32, tag="sq", name="sq")
            nc.vector.scalar_tensor_tensor(out=sq, in0=xt, scalar=1.0, in1=xt,
                                           op0=OP.mult, op1=OP.mult,
                                           accum_out=e2[:, q:q + 1])
            return xt

        def normalize_xn2(q, x1q, rstd2, nm2, k):
            xq = ypool.tile([P, C], BF16, tag=f"xn2_{q}", name=f"xn2_{q}", bufs=1)
            if apply_ln2:
                tmp = scratch.tile([P, C], FP32, tag="lntmp", name="lntmp")
                nc.scalar.activation(out=tmp, in_=x1q, func=AF.Identity,
                                     bias=nm2[:, k:k + 1], scale=rstd2[:, k:k + 1])
                nc.vector.tensor_mul(out=tmp, in0=tmp, in1=lnwb[:, 2, :])
                nc.vector.scalar_tensor_tensor(out=xq, in0=tmp, scalar=1.0,
                                               in1=lnwb[:, 3, :], op0=OP.mult,
                                               op1=OP.add)
            else:
                nc.scalar.activation(out=xq, in_=x1q, func=AF.Identity,
                                     bias=nm2[:, k:k + 1], scale=rstd2[:, k:k + 1])
            return xq

        def alloc_tr():
            # c-tiles m packed 2 per PSUM bank (the fp32 bank tile is viewed
            # as [P, 2C] bf16 so the tag matches the pz allocations rotating
            # through the same 2 banks)
            return [trpz.tile([P, C], FP32, tag="pz", name=f"tr{h}").bitcast(BF16)
                    for h in range(2)]

        def transpose_i(tr, xn2i, i):
            for m in range(NC_):
                nc.tensor.transpose(
                    tr[m // 2][:, (m % 2) * C + i * P:(m % 2) * C + (i + 1) * P],
                    xn2i[:, m * P:(m + 1) * P], ident)

        def ytok_copies(ptok):
            ytok = []
            for j in range(NO_TOK):
                ysb = ypool.tile([P, C], BF16, tag=f"ytok{j}", name="ysb")
                nc.scalar.copy(out=ysb, in_=ptok[j])
                ytok.append(ysb)
            return ytok

        def token_out(b, X, ytok):
            """mm2, residual+LN2 stats, normalize -> xn2."""
            s1 = stats.tile([P, NT], FP32, tag="s1", name="s1")
            e2 = stats.tile([P, NT], FP32, tag="e2", name="e2")
            x1 = [mm2_resid(q, X, ytok, s1, e2, pza if q < 2 else trpz)
                  for q in range(NT)]
            rstd2, nm2 = ln2_chain(s1, e2, NT, "")
            xn2 = [normalize_xn2(q, x1[q], rstd2, nm2, q) for q in range(NT)]
            return x1, xn2

        def token_out_last(b, X, ytok):
            """Last batch: no next-batch mm1 to cover the LN2 chain, so
            process pair-wise and interleave the transposes."""
            s1 = stats.tile([P, NT], FP32, tag="s1", name="s1")
            e2 = stats.tile([P, NT], FP32, tag="e2", name="e2")
            tr = alloc_tr()
            x1 = []
            for g in range(2):
                qs = (2 * g, 2 * g + 1)
                for q in qs:
                    x1.append(mm2_resid(q, X, ytok, s1, e2, pza))
                rstd2, nm2 = ln2_chain(s1[:, 2 * g:2 * g + 2],
                                       e2[:, 2 * g:2 * g + 2], 2, f"p{g}")
                for k, q in enumerate(qs):
                    xq = normalize_xn2(q, x1[q], rstd2, nm2, k)
                    transpose_i(tr, xq, q)
            return x1, tr

        def transposes(xn2):
            tr = alloc_tr()
            for i in range(NT):
                transpose_i(tr, xn2[i], i)
            return tr

        def channel_feats(tr, pool=None, pref="F2"):
            # the final batch borrows the (freed) token-feature pool so its
            # chain isn't WAR-blocked behind the previous mm3's reads
            return [features_from_norm(tr[m // 2][:, (m % 2) * C:(m % 2 + 1) * C],
                                       f"c{m}", pool or f2pool, pref, m)
                    for m in range(NC_)]

        def mm3(fch, after_q0=None):
            """q-major channel matmuls; after_q0 lets the caller splice
            other PE work (the final batch's transposes) behind the first
            output's accumulation, where its normalize is just ready."""
            pouts = []
            for q in range(NT):
                pout = poutp.tile([P, C], FP32, tag="pout", name=f"pout{q}")
                for m in range(NC_):
                    for f in range(NF):
                        nc.tensor.matmul(pout, fch[m][f][:, q * P:(q + 1) * P],
                                         wchf[m][:, f, :],
                                         start=(m == 0 and f == 0),
                                         stop=(m == NC_ - 1 and f == NF - 1))
                pouts.append(pout)
                if q == 0 and after_q0 is not None:
                    after_q0()
            return pouts

        def emit_out(b, pouts, x1, final=False):
            """residual add on DVE straight from PSUM, then store.  Emitted
            one iteration late so it never head-of-line-blocks the next
            batch's feature chain on the DVE FIFO.  For the final batch the
            stores split across queues so the tail transfer isn't bound by
            one ring's bandwidth."""
            for q in range(NT):
                ot = opool.tile([P, C], FP32, tag="out", name="out")
                nc.vector.tensor_tensor(out=ot, in0=pouts[q], in1=x1[q],
                                        op=OP.add)
                nc.vector.tensor_tensor(out=ot, in0=ot, in1=bch, op=OP.add)
                if final and q >= 2:
                    for hh in range(4):
                        nc.sync.dma_start(
                            out=y_out[b, q * P:(q + 1) * P,
                                      hh * (C // 4):(hh + 1) * (C // 4)],
                            in_=ot[:, hh * (C // 4):(hh + 1) * (C // 4)])
                else:
                    nc.sync.dma_start(out=y_out[b, q * P:(q + 1) * P, :], in_=ot)

        # ---- software-pipelined emission over batches ----
        # DMA order: X(0), wtok, [x(1)], then the big wchf -- the fill is
        # HBM-bandwidth-bound, so batch-0's dependencies go first.
        if apply_ln1:
            lnwb = singles.tile([P, 4, C], FP32, tag="lnwb")
            nc.sync.dma_start(out=lnwb, in_=ln_in.rearrange("p (k c) -> p k c", k=4))
        Xc, featsc = X0, stage1_prefetch(X0, groups=[(0,), (1,), (2, 3)])
        Xmap = {0: X0}
        if NB > 1:
            Xmap[1] = X1
        if NB > 2:
            Xmap[2] = load_x(2)
        wchf, bch, lnwb2 = load_weights_late()
        if not apply_ln1:
            lnwb = lnwb2
        ptokc = [ptokp.tile([P, C], FP32, tag="ptok", name=f"ptok{j}")
                 for j in range(NO_TOK)]
        mm1(featsc, ptokc, range(NT))
        pending_out = None
        hoisted = None   # last batch's token stage, pre-emitted an iteration early
        for b in range(NB):
            X, feats, ptok = Xc, featsc, ptokc
            # b=0: the next batch's Sin chain goes ahead of the ytok copies
            # on the ACT queue -- the copies wait for mm1(0) anyway, while
            # the sins' inputs are ready earlier (kills the ramp gaps).
            # Steady state: copies first (they gate mm2 on the PE; the sins'
            # inputs arrive early under the mm3 cover).
            if b == 0 and b + 1 < NB:
                Xc = Xmap[b + 1]
                featsc = stage1_prefetch(Xc, gsize=2)
                ytok = ytok_copies(ptok)
                if b + 2 < NB and (b + 2) not in Xmap:
                    Xmap[b + 2] = load_x(b + 2)
            elif hoisted is None:
                ytok = ytok_copies(ptok)
                if b + 1 < NB:
                    Xc = Xmap[b + 1]
                    featsc = stage1_prefetch(Xc, gsize=NT)
                    if b + 2 < NB and (b + 2) not in Xmap:
                        Xmap[b + 2] = load_x(b + 2)
            if b + 1 < NB:
                x1, xn2 = token_out(b, X, ytok)
                if pending_out is not None:
                    emit_out(b - 1, *pending_out)
                ptokc = [ptokp.tile([P, C], FP32, tag="ptok", name=f"ptok{j}")
                         for j in range(NO_TOK)]
                mm1(featsc, ptokc, (0, 1))
                tr = transposes(xn2)
                mm1(featsc, ptokc, (2, 3))
                fch = channel_feats(tr)
                if b + 1 == NB - 1:
                    # hoist the final batch's token stage (ytok, mm2, LN2,
                    # normalize) ahead of this mm3 so its serial chain runs
                    # under the 21us matmul cover; its transposes splice in
                    # behind this mm3's first output (the normalize is just
                    # done by then) and its feature chain runs under the
                    # remaining 15us of matmuls.
                    ytok_l = ytok_copies(ptokc)
                    x1_l, xn2_l = token_out(b + 1, Xc, ytok_l)
                    box = []
                    pending_out = (mm3(fch, after_q0=lambda: box.append(
                        transposes(xn2_l))), x1)
                    hoisted = (x1_l, channel_feats(box[0], pool=fpool, pref="F1"))
                else:
                    pending_out = (mm3(fch), x1)
            else:
                if hoisted is not None:
                    x1, fch = hoisted
                else:
                    x1, tr = token_out_last(b, X, ytok)
                    fch = channel_feats(tr)
                if pending_out is not None:
                    emit_out(b - 1, *pending_out)
                pending_out = (mm3(fch), x1)
        emit_out(NB - 1, *pending_out, final=True)

    _split_multi_waits(nc)
    return nc


_CACHE = {}


def _get_nc(apply_ln1, apply_ln2):
    key = (apply_ln1, apply_ln2)
    if key not in _CACHE:
        _CACHE[key] = _build(apply_ln1, apply_ln2)
    return _CACHE[key]


def prepare_in_maps(inputs):
    return _prepare(**inputs)


def _prepare(x, ln1_w, ln1_b, tok_coef, tok_kbias, tok_lw, tok_lb,
             ln2_w, ln2_b, ch_coef, ch_kbias, ch_lw, ch_lb):
    x = np.asarray(x, np.float32)
    f64 = np.float64

    wtok_eff, tok_const = _cheb_weights(np.asarray(tok_coef, f64))  # (T,6,TD)
    wch_eff, ch_const = _cheb_weights(np.asarray(ch_coef, f64))     # (C,6,2C)

    kbias_tok = np.asarray(tok_kbias, f64).reshape(-1) + tok_const
    kbias_ch = np.asarray(ch_kbias, f64).reshape(-1) + ch_const
    bias_tok = np.asarray(tok_lb, f64) + np.asarray(tok_lw, f64) @ kbias_tok
    bias_ch = np.asarray(ch_lb, f64) + np.asarray(ch_lw, f64) @ kbias_ch

    # fold the channel post-KAN linear into the KAN weights (fp64)
    wchf = np.einsum("cfo,ko->cfk", wch_eff, np.asarray(ch_lw, f64))  # (C,6,C)

    wtok_np = wtok_eff.reshape(NT, P, NF * TD).astype(ml_dtypes.bfloat16)
    wchf_np = wchf.reshape(NC_, P, NF * C).astype(ml_dtypes.bfloat16)
    tlw_np = np.ascontiguousarray(np.asarray(tok_lw, f64).T).reshape(
        NO_TOK, P, T).astype(ml_dtypes.bfloat16)
    btok_np = np.ascontiguousarray(bias_tok.reshape(NT, P).T).astype(np.float32)
    bch_np = np.broadcast_to(bias_ch.astype(ml_dtypes.bfloat16), (P, C)).copy()
    lnwb_np = np.broadcast_to(
        np.concatenate([np.asarray(ln1_w, f64), np.asarray(ln1_b, f64),
                        np.asarray(ln2_w, f64), np.asarray(ln2_b, f64)]).astype(
            np.float32), (P, 4 * C)).copy()

    apply_ln1 = not (np.all(np.asarray(ln1_w) == 1.0) and np.all(np.asarray(ln1_b) == 0.0))
    apply_ln2 = not (np.all(np.asarray(ln2_w) == 1.0) and np.all(np.asarray(ln2_b) == 0.0))

    shared = dict(wtok=wtok_np, wchf=wchf_np, tlw=tlw_np,
                  btok=btok_np, bch=bch_np, lnwb=lnwb_np)
    in_maps = []
    for core in range(NCORES):
        m = dict(shared)
        m["x"] = np.ascontiguousarray(x[core * NB:(core + 1) * NB])
        in_maps.append(m)
    return {"build_key": (apply_ln1, apply_ln2), "in_maps": in_maps}


def kernel(**inputs):
    prep = _prepare(**inputs)
    nc = _get_nc(*prep["build_key"])
    res = run_bass_kernel_spmd(nc, prep["in_maps"], list(range(NCORES)))
    return np.concatenate([res.results[i]["y"] for i in range(NCORES)], axis=0)


# revision 48
# speedup vs baseline: 1.0580x; 1.0010x over previous
"""MixerLayerKAN Trainium2 kernel.

x (B,T,C)=(32,512,512) fp32; token-mix FourierKAN(T->TD)+Linear, then
channel-mix FourierKAN(C->2C)+Linear, LN + residual around each.

Strategy (data-parallel over batch, 4 batches per NeuronCore, weights
replicated, no collectives):

* Fourier features cos(kx)/sin(kx), k=1..3, re-expressed in the product
  basis {s, c, s*c, s^2, s^3, c*s^2}; harmonic coefficients fold
  host-side into 6 effective weight matrices; the channel KAN further
  folds its post-KAN Linear (96 matmuls/batch instead of 224).
* LN1 normalize is folded into the feature chain: the range-wrap
  (round-to-int tensor_scalar + int32-input scalar_tensor_tensor, with
  per-partition scale/bias riding the scalar slots) and the Sin/Abs
  activations consume raw x directly -- no materialized normalized
  tensor and no int->float CAST on the token path.
* Transposes run in bf16 (1 cycle/row), two c-tiles packed per PSUM
  bank; the channel wrap chain reads transposed values straight from
  PSUM (no PSUM->SBUF copy ops).
* Each output tile is two DVE adds straight from PSUM (psum +
  residual, + channel bias) -- keeping the adds off the saturated PE
  and the copies off the scalar engine.
* PE FIFO is software-pipelined: mm1 of batch b+1 is split into two
  24-matmul chunks emitted around batch b's transposes, covering the
  two serial handoffs (LN2 chain, channel feature chain); mm3 runs
  q-major so the output adds overlap it; output DMAs trail one
  iteration so they never head-of-line-block the DVE FIFO.
* DMA: per-queue ring bandwidth is ~46GB/s, so batch-0's x tiles and
  the first token weights load as small split DMAs across rings, x is
  prefetched two batches ahead (before the big wchf load), and the
  final stores split across rings to shorten the drain.
* The last batch's ENTIRE pipeline prefix (ytok, mm2, LN2, normalize,
  transposes, channel features) is hoisted under the previous batch's
  mm3: the transposes splice in behind mm3's first output chunk (the
  normalize completes just in time), and the feature chain -- written
  into the freed token-feature pool to dodge a WAR stall -- runs under
  the remaining 15us of matmuls.  The final iteration is matmuls-only.

Measured ~185.5us/core (~233us baseline); note the part power-throttles
under sustained load, adding up to ~40us run-to-run.
"""

import numpy as np
import ml_dtypes

import concourse.bass as bass
import concourse.mybir as mybir
from concourse import tile
from concourse.bass_utils import run_bass_kernel_spmd
from concourse.masks import make_identity

AF = mybir.ActivationFunctionType
OP = mybir.AluOpType
FP32 = mybir.dt.float32
BF16 = mybir.dt.bfloat16
I32 = mybir.dt.int32

B, T, C, TD, G = 32, 512, 512, 256, 3
NCORES = 8
NB = B // NCORES          # batches per core
P = 128
EPS = 1e-5
PI = float(np.pi)
TWO_PI = float(2 * np.pi)
INV_2PI = float(1.0 / (2 * np.pi))
FOUR_PI = float(4 * np.pi)
NF = 6                    # product-basis features
NT = T // P               # 4 t-tiles
NC_ = C // P              # 4 c-tiles
NO_TOK = TD // P          # 2 token KAN hidden tiles


def _split_multi_waits(nc):
    """This walrus build accepts at most ONE sync-wait command per
    instruction.  Tile emits several.  Fix: before each multi-wait
    instruction, splice in same-engine NOPs carrying one wait each (a wait
    executed earlier on the same engine is semantically identical)."""
    f = nc.m.functions[0]
    per_engine = {}
    for bb in f.blocks:
        for inst in bb.instructions:
            si = getattr(inst, "sync_info", None)
            if si is not None and si.on_wait and len(si.on_wait) > 1:
                per_engine[inst.engine] = per_engine.get(inst.engine, 0) + (
                    len(si.on_wait) - 1)
    if not per_engine:
        return
    nop_pool = {}
    for eng, cnt in per_engine.items():
        nop_pool[eng] = [nc.engines[eng].nop(nofuse=True).ins for _ in range(cnt)]
    created = {id(i) for h in nop_pool.values() for i in h}
    for bb in f.blocks:
        bb.instructions[:] = [i for i in bb.instructions if id(i) not in created]
    for bb in f.blocks:
        out = []
        for inst in bb.instructions:
            si = getattr(inst, "sync_info", None)
            if si is not None and si.on_wait and len(si.on_wait) > 1:
                waits = list(si.on_wait)
                si.on_wait = [waits[-1]]
                for w in waits[:-1]:
                    nop = nop_pool[inst.engine].pop()
                    nop.sync_info = mybir.SyncInfo(on_wait=[w], on_update=[])
                    out.append(nop)
            out.append(inst)
        bb.instructions[:] = out


def _cheb_weights(coef):
    """coef (2, O, I, G) -> effective basis weights (I, 6, O) for the
    {s, c, s*c, s^2, s^3, c*s^2} basis, plus the constant term (O,).

    cos(1x)=c; cos(2x)=1-2s^2; cos(3x)=c-4c s^2
    sin(1x)=s; sin(2x)=2 s c ; sin(3x)=3s-4s^3
    """
    cosw = coef[0]
    sinw = coef[1]
    O, I, _ = cosw.shape
    w = np.zeros((I, NF, O), np.float64)
    w[:, 0, :] = (sinw[:, :, 0] + 3.0 * sinw[:, :, 2]).T      # s
    w[:, 1, :] = (cosw[:, :, 0] + cosw[:, :, 2]).T            # c
    w[:, 2, :] = (2.0 * sinw[:, :, 1]).T                      # s*c
    w[:, 3, :] = (-2.0 * cosw[:, :, 1]).T                     # s^2
    w[:, 4, :] = (-4.0 * sinw[:, :, 2]).T                     # s^3
    w[:, 5, :] = (-4.0 * cosw[:, :, 2]).T                     # c*s^2
    const = cosw[:, :, 1].sum(axis=1)                         # from the "1" of cos(2x)
    return w, const


def _build(apply_ln1, apply_ln2):
    nc = bass.Bass()
    x_in = nc.dram_tensor("x", [NB, T, C], FP32, kind="ExternalInput")
    y_out = nc.dram_tensor("y", [NB, T, C], FP32, kind="ExternalOutput")
    wtok_in = nc.dram_tensor("wtok", [NT, P, NF * TD], BF16, kind="ExternalInput")
    wchf_in = nc.dram_tensor("wchf", [NC_, P, NF * C], BF16, kind="ExternalInput")
    tlw_in = nc.dram_tensor("tlw", [NO_TOK, P, T], BF16, kind="ExternalInput")
    btok_in = nc.dram_tensor("btok", [P, NT], FP32, kind="ExternalInput")
    bch_in = nc.dram_tensor("bch", [P, C], BF16, kind="ExternalInput")
    ln_in = nc.dram_tensor("lnwb", [P, 4 * C], FP32, kind="ExternalInput")

    with tile.TileContext(nc) as tc, \
         tc.tile_pool(name="singles", bufs=1) as singles, \
         tc.tile_pool(name="xpool", bufs=3) as xpool, \
         tc.tile_pool(name="fpool", bufs=2) as fpool, \
         tc.tile_pool(name="f2pool", bufs=1) as f2pool, \
         tc.tile_pool(name="scratch", bufs=1) as scratch, \
         tc.tile_pool(name="ypool", bufs=2) as ypool, \
         tc.tile_pool(name="x1pool", bufs=2) as x1pool, \
         tc.tile_pool(name="opool", bufs=8) as opool, \
         tc.tile_pool(name="stats", bufs=2) as stats, \
         tc.tile_pool(name="ptokp", bufs=2, space="PSUM") as ptokp, \
         tc.tile_pool(name="pza", bufs=2, space="PSUM") as pza, \
         tc.tile_pool(name="trpz", bufs=2, space="PSUM") as trpz, \
         tc.tile_pool(name="poutp", bufs=2, space="PSUM") as poutp:

        # ---- batch-0 x first so the big weight DMAs don't block start ----
        def load_x(b, nsplit=2):
            xt = xpool.tile([P, NT, C], FP32, tag="X", name=f"X{b}")
            step = NT // nsplit
            for i in range(0, NT, step):
                nc.sync.dma_start(
                    out=xt[:, i:i + step, :],
                    in_=x_in[b, i * P:(i + step) * P, :].rearrange(
                        "(i p) c -> p i c", p=P))
            return [xt[:, i, :] for i in range(NT)]

        # batch-0: per-tile DMAs interleaved with per-tile wtok loads so
        # tile-0 stats and the first mm1 weights arrive ASAP
        X0t = xpool.tile([P, NT, C], FP32, tag="X", name="X0")
        wtok_all = singles.tile([P, NT, NF, TD], BF16, tag="wtok")
        H = C // 2
        for i in (0, 1):
            for hh in (0, 1):
                nc.sync.dma_start(out=X0t[:, i, hh * H:(hh + 1) * H],
                                  in_=x_in[0, i * P:(i + 1) * P, hh * H:(hh + 1) * H])
        wtok0r = wtok_in[0].rearrange("p (f o) -> p f o", f=NF)
        nc.sync.dma_start(out=wtok_all[:, 0, 0:NF // 2], in_=wtok0r[:, 0:NF // 2])
        nc.sync.dma_start(out=wtok_all[:, 0, NF // 2:], in_=wtok0r[:, NF // 2:])
        for i in (2, 3):
            nc.sync.dma_start(out=X0t[:, i, :], in_=x_in[0, i * P:(i + 1) * P, :])
        for i in range(1, NT):
            nc.sync.dma_start(out=wtok_all[:, i],
                              in_=wtok_in[i].rearrange("p (f o) -> p f o", f=NF))
        X1 = load_x(1) if NB > 1 else None
        X0 = [X0t[:, i, :] for i in range(NT)]
        wtok = [wtok_all[:, i] for i in range(NT)]

        ident = singles.tile([P, P], BF16, tag="ident")
        make_identity(nc, ident)
        ones128 = singles.tile([P, P], BF16, tag="ones128")
        nc.vector.memset(ones128, float(1.0 / 128.0))
        halfpi = singles.tile([P, 1], FP32, tag="halfpi")
        nc.vector.memset(halfpi, PI / 2)
        actwarm = singles.tile([P, 1], FP32, tag="actwarm")
        nc.scalar.activation(out=actwarm, in_=halfpi, func=AF.Sin)
        tlw = []
        for j in range(NO_TOK):
            t_ = singles.tile([P, T], BF16, tag=f"tlw{j}")
            nc.sync.dma_start(out=t_, in_=tlw_in[j])
            tlw.append(t_)
        btok = singles.tile([P, NT], FP32, tag="btok")
        nc.sync.dma_start(out=btok, in_=btok_in[:, :])

        def load_weights_late():
            wchf_all = singles.tile([P, NC_, NF, C], BF16, tag="wchf")
            nc.sync.dma_start(out=wchf_all,
                              in_=wchf_in.rearrange("m p (f o) -> p m f o", f=NF))
            wchf = [wchf_all[:, m] for m in range(NC_)]
            bch = singles.tile([P, C], BF16, tag="bch")
            nc.sync.dma_start(out=bch, in_=bch_in[:, :])
            lnwb = None
            if apply_ln2 and not apply_ln1:
                lnwb = singles.tile([P, 4, C], FP32, tag="lnwb")
                nc.sync.dma_start(out=lnwb, in_=ln_in.rearrange("p (k c) -> p k c", k=4))
            return wchf, bch, lnwb

        # ---- helpers ----
        def ln1_stats(X, tiles, gkey):
            """bn stats + rsqrt Newton + derived wrap scalars for a group of
            tiles.  Returns {tile: (rstd, nb, aa, bv, cc2) [P,1] slices}."""
            n = len(tiles)
            mvs = stats.tile([P, n, 2], FP32, tag=f"mvs{gkey}", name="mvs")
            for k, i in enumerate(tiles):
                st6 = stats.tile([P, 6], FP32, tag=f"st6_{i % 2}", name="st6")
                nc.vector.bn_stats(out=st6, in_=X[i])
                nc.vector.bn_aggr(out=mvs[:, k, :], in_=st6)
            mean = mvs[:, :, 0]
            var = mvs[:, :, 1]
            h = stats.tile([P, n], FP32, tag=f"h{gkey}", name="h")
            nc.vector.tensor_scalar(out=h, in0=var, scalar1=EPS, scalar2=-0.5,
                                    op0=OP.add, op1=OP.mult)
            yi = stats.tile([P, n], I32, tag=f"yi{gkey}", name="yi")
            nc.vector.tensor_scalar(out=yi, in0=var.bitcast(I32), scalar1=1,
                                    scalar2=None, op0=OP.logical_shift_right)
            nc.vector.tensor_scalar(out=yi, in0=yi, scalar1=-1,
                                    scalar2=0x5F3759DF, op0=OP.mult, op1=OP.add)
            rstd = yi.bitcast(FP32)
            a2 = stats.tile([P, n], FP32, tag=f"a2{gkey}", name="a2")
            for _ in range(2):
                nc.vector.tensor_tensor(out=a2, in0=rstd, in1=rstd, op=OP.mult)
                nc.vector.tensor_tensor(out=a2, in0=a2, in1=h, op=OP.mult)
                nc.vector.scalar_tensor_tensor(out=rstd, in0=a2, scalar=1.5,
                                               in1=rstd, op0=OP.add, op1=OP.mult)
            nb = stats.tile([P, n], FP32, tag=f"nb{gkey}", name="nb")
            nc.vector.scalar_tensor_tensor(out=nb, in0=mean, scalar=-1.0, in1=rstd,
                                           op0=OP.mult, op1=OP.mult)
            aa = stats.tile([P, n], FP32, tag=f"aa{gkey}", name="aa")
            nc.vector.tensor_scalar(out=aa, in0=rstd, scalar1=INV_2PI, scalar2=None,
                                    op0=OP.mult)
            bv = stats.tile([P, n], FP32, tag=f"bv{gkey}", name="bv")
            nc.vector.tensor_scalar(out=bv, in0=nb, scalar1=INV_2PI, scalar2=None,
                                    op0=OP.mult)
            cc2 = stats.tile([P, n], FP32, tag=f"cc2{gkey}", name="cc2")
            nc.vector.scalar_tensor_tensor(out=cc2, in0=h, scalar=FOUR_PI, in1=rstd,
                                           op0=OP.mult, op1=OP.mult)
            return {i: tuple(t[:, k:k + 1] for t in (rstd, nb, aa, bv, cc2))
                    for k, i in enumerate(tiles)}

        def feat_tiles(pool, pref, i):
            return [pool.tile([P, C], BF16, tag=f"{pref}_{i}_{k}", name=f"{pref}{i}b{k}")
                    for k in range(NF)]

        def features_from_x(xt, i, rstd, nb, aa, bv, cc2, pref):
            """Token-path features straight from raw x (LN folded in)."""
            if apply_ln1:
                # general path: materialize normalized tensor, then wrap
                xn = scratch.tile([P, C], FP32, tag=f"xn{i % 2}", name="xn")
                nc.scalar.activation(out=xn, in_=xt, func=AF.Identity,
                                     bias=nb, scale=rstd)
                nc.vector.tensor_mul(out=xn, in0=xn, in1=lnwb[:, 0, :])
                nc.vector.tensor_add(out=xn, in0=xn, in1=lnwb[:, 1, :])
                return features_from_norm(xn, f"F1_{i}", fpool, pref, i)
            f = feat_tiles(fpool, pref, i)
            ni = scratch.tile([P, C], I32, tag=f"ni1_{i % 2}", name="ni")
            nc.vector.tensor_scalar(out=ni, in0=xt, scalar1=aa,
                                    scalar2=bv, op0=OP.mult, op1=OP.add)
            rt = scratch.tile([P, C], FP32, tag=f"rt1_{i % 2}", name="rt")
            nc.vector.scalar_tensor_tensor(out=rt, in0=ni, scalar=cc2,
                                           in1=xt, op0=OP.mult, op1=OP.add)
            nc.scalar.activation(out=f[0], in_=rt, func=AF.Sin,
                                 scale=rstd, bias=nb)
            ab = scratch.tile([P, C], FP32, tag=f"ab1_{i % 2}", name="ab")
            nc.scalar.activation(out=ab, in_=rt, func=AF.Abs,
                                 scale=rstd, bias=nb)
            nc.scalar.activation(out=f[1], in_=ab, func=AF.Sin, scale=-1.0,
                                 bias=halfpi[:, :])
            nc.vector.tensor_mul(out=f[3], in0=f[0], in1=f[0])   # ss
            nc.vector.tensor_mul(out=f[2], in0=f[0], in1=f[1])   # sc
            nc.vector.tensor_mul(out=f[4], in0=f[3], in1=f[0])   # sss
            nc.vector.tensor_mul(out=f[5], in0=f[3], in1=f[1])   # css
            return f

        def features_from_norm(src, key, pool, pref, i):
            """Channel-path features from an already-normalized source
            (SBUF tile or PSUM transpose slice)."""
            f = feat_tiles(pool, pref, i)
            ni = scratch.tile([P, C], I32, tag=f"ni_{key}" if apply_ln1 else f"ni2_{i % 2}",
                             name="ni")
            nc.vector.tensor_scalar(out=ni, in0=src, scalar1=INV_2PI, scalar2=None,
                                    op0=OP.mult)
            rt = scratch.tile([P, C], FP32, tag=f"rt_{key}" if apply_ln1 else f"rt2_{i % 2}",
                             name="rt")
            nc.vector.scalar_tensor_tensor(out=rt, in0=ni, scalar=-TWO_PI,
                                           in1=src, op0=OP.mult, op1=OP.add)
            nc.scalar.activation(out=f[0], in_=rt, func=AF.Sin)
            ab = scratch.tile([P, C], FP32, tag=f"ab_{key}" if apply_ln1 else f"ab2_{i % 2}",
                             name="ab")
            nc.scalar.activation(out=ab, in_=rt, func=AF.Abs)
            nc.scalar.activation(out=f[1], in_=ab, func=AF.Sin, scale=-1.0,
                                 bias=halfpi[:, :])
            nc.vector.tensor_mul(out=f[3], in0=f[0], in1=f[0])
            nc.vector.tensor_mul(out=f[2], in0=f[0], in1=f[1])
            nc.vector.tensor_mul(out=f[4], in0=f[3], in1=f[0])
            nc.vector.tensor_mul(out=f[5], in0=f[3], in1=f[1])
            return f

        def stage1_prefetch(X, gsize=NT, groups=None):
            """LN1 + token features for a batch whose x is already loading.
            Small group sizes start tile-0's feature chain earlier (used
            during the DMA-bound fill)."""
            if groups is None:
                groups = [tuple(range(g, g + gsize)) for g in range(0, NT, gsize)]
            feats = [None] * NT
            for g, tiles in enumerate(groups):
                sc = ln1_stats(X, tiles, f"{len(tiles)}_{g % 2}")
                for i in tiles:
                    feats[i] = features_from_x(X[i], i, *sc[i], "F1")
            return feats

        def mm1(feats, ptok, tiles):
            for i in tiles:
                for j in range(NO_TOK):
                    for f in range(NF):
                        nc.tensor.matmul(ptok[j], wtok[i][:, f, j * P:(j + 1) * P],
                                         feats[i][f], start=(i == 0 and f == 0),
                                         stop=(i == NT - 1 and f == NF - 1))

        def ln2_chain(s1, e2, n, gkey):
            """LN2 rsqrt chain on DVE (latency-critical for the transposes).
            s1/e2 [P,n] slices -> (rstd2, nm2) [P,n]."""
            mn = stats.tile([P, n], FP32, tag=f"mn{gkey}", name="mn")
            nc.vector.tensor_scalar_mul(out=mn, in0=s1, scalar1=1.0 / C)
            vr = stats.tile([P, n], FP32, tag=f"vr{gkey}", name="vr")
            nc.vector.tensor_mul(out=vr, in0=mn, in1=mn)
            nc.vector.scalar_tensor_tensor(out=vr, in0=e2, scalar=1.0 / C, in1=vr,
                                           op0=OP.mult, op1=OP.subtract)
            h2 = stats.tile([P, n], FP32, tag=f"h2{gkey}", name="h2")
            nc.vector.tensor_scalar(out=h2, in0=vr, scalar1=EPS, scalar2=-0.5,
                                    op0=OP.add, op1=OP.mult)
            yi2 = stats.tile([P, n], I32, tag=f"yi2{gkey}", name="yi2")
            nc.vector.tensor_scalar(out=yi2, in0=vr.bitcast(I32), scalar1=1,
                                    scalar2=None, op0=OP.logical_shift_right)
            nc.vector.tensor_scalar(out=yi2, in0=yi2, scalar1=-1,
                                    scalar2=0x5F3759DF, op0=OP.mult, op1=OP.add)
            rstd2 = yi2.bitcast(FP32)
            a2 = stats.tile([P, n], FP32, tag=f"a2b{gkey}", name="a2b")
            for _ in range(2):
                nc.vector.tensor_mul(out=a2, in0=rstd2, in1=rstd2)
                nc.vector.tensor_mul(out=a2, in0=a2, in1=h2)
                nc.vector.scalar_tensor_tensor(out=rstd2, in0=a2, scalar=1.5,
                                               in1=rstd2, op0=OP.add, op1=OP.mult)
            nm2 = stats.tile([P, n], FP32, tag=f"nm2{gkey}", name="nm2")
            nc.vector.scalar_tensor_tensor(out=nm2, in0=mn, scalar=-1.0,
                                           in1=rstd2, op0=OP.mult, op1=OP.mult)
            return rstd2, nm2

        def mm2_resid(q, X, ytok, s1, e2, pz_pool):
            pz = pz_pool.tile([P, C], FP32, tag="pz", name="pz")
            for j in range(NO_TOK):
                nc.tensor.matmul(pz, tlw[j][:, q * P:(q + 1) * P], ytok[j],
                                 start=(j == 0), stop=(j == NO_TOK - 1))
            xt = x1pool.tile([P, C], BF16, tag=f"x1_{q}", name=f"x1_{q}")
            nc.vector.scalar_tensor_tensor(out=xt, in0=pz,
                                           scalar=btok[:, q:q + 1],
                                           in1=X[q], op0=OP.add, op1=OP.add,
                                           accum_out=s1[:, q:q + 1])
            sq = scratch.tile([P, C], FP32, tag="sq", name="sq")
            nc.vector.scalar_tensor_tensor(out=sq, in0=xt, scalar=1.0, in1=xt,
                                           op0=OP.mult, op1=OP.mult,
                                           accum_out=e2[:, q:q + 1])
            return xt

        def normalize_xn2(q, x1q, rstd2, nm2, k):
            xq = ypool.tile([P, C], BF16, tag=f"xn2_{q}", name=f"xn2_{q}", bufs=1)
            if apply_ln2:
                tmp = scratch.tile([P, C], FP32, tag="lntmp", name="lntmp")
                nc.scalar.activation(out=tmp, in_=x1q, func=AF.Identity,
                                     bias=nm2[:, k:k + 1], scale=rstd2[:, k:k + 1])
                nc.vector.tensor_mul(out=tmp, in0=tmp, in1=lnwb[:, 2, :])
                nc.vector.scalar_tensor_tensor(out=xq, in0=tmp, scalar=1.0,
                                               in1=lnwb[:, 3, :], op0=OP.mult,
                                               op1=OP.add)
            else:
                nc.scalar.activation(out=xq, in_=x1q, func=AF.Identity,
                                     bias=nm2[:, k:k + 1], scale=rstd2[:, k:k + 1])
            return xq

        def alloc_tr():
            # c-tiles m packed 2 per PSUM bank (the fp32 bank tile is viewed
            # as [P, 2C] bf16 so the tag matches the pz allocations rotating
            # through the same 2 banks)
            return [trpz.tile([P, C], FP32, tag="pz", name=f"tr{h}").bitcast(BF16)
                    for h in range(2)]

        def transpose_i(tr, xn2i, i):
            for m in range(NC_):
                nc.tensor.transpose(
                    tr[m // 2][:, (m % 2) * C + i * P:(m % 2) * C + (i + 1) * P],
                    xn2i[:, m * P:(m + 1) * P], ident)

        def ytok_copies(ptok):
            ytok = []
            for j in range(NO_TOK):
                ysb = ypool.tile([P, C], BF16, tag=f"ytok{j}", name="ysb")
                nc.scalar.copy(out=ysb, in_=ptok[j])
                ytok.append(ysb)
            return ytok

        def token_out(b, X, ytok):
            """mm2, residual+LN2 stats, normalize -> xn2."""
            s1 = stats.tile([P, NT], FP32, tag="s1", name="s1")
            e2 = stats.tile([P, NT], FP32, tag="e2", name="e2")
            x1 = [mm2_resid(q, X, ytok, s1, e2, pza if q < 2 else trpz)
                  for q in range(NT)]
            rstd2, nm2 = ln2_chain(s1, e2, NT, "")
            xn2 = [normalize_xn2(q, x1[q], rstd2, nm2, q) for q in range(NT)]
            return x1, xn2

        def token_out_last(b, X, ytok):
            """Last batch: no next-batch mm1 to cover the LN2 chain, so
            process pair-wise and interleave the transposes."""
            s1 = stats.tile([P, NT], FP32, tag="s1", name="s1")
            e2 = stats.tile([P, NT], FP32, tag="e2", name="e2")
            tr = alloc_tr()
            x1 = []
            for g in range(2):
                qs = (2 * g, 2 * g + 1)
                for q in qs:
                    x1.append(mm2_resid(q, X, ytok, s1, e2, pza))
                rstd2, nm2 = ln2_chain(s1[:, 2 * g:2 * g + 2],
                                       e2[:, 2 * g:2 * g + 2], 2, f"p{g}")
                for k, q in enumerate(qs):
                    xq = normalize_xn2(q, x1[q], rstd2, nm2, k)
                    transpose_i(tr, xq, q)
            return x1, tr

        def transposes(xn2):
            tr = alloc_tr()
            for i in range(NT):
                transpose_i(tr, xn2[i], i)
            return tr

        def channel_feats(tr, pool=None, pref="F2"):
            # the final batch borrows the (freed) token-feature pool so its
            # chain isn't WAR-blocked behind the previous mm3's reads
            return [features_from_norm(tr[m // 2][:, (m % 2) * C:(m % 2 + 1) * C],
                                       f"c{m}", pool or f2pool, pref, m)
                    for m in range(NC_)]

        def mm3(fch, after_q0=None):
            """q-major channel matmuls; after_q0 lets the caller splice
            other PE work (the final batch's transposes) behind the first
            output's accumulation, where its normalize is just ready."""
            pouts = []
            for q in range(NT):
                pout = poutp.tile([P, C], FP32, tag="pout", name=f"pout{q}")
                for m in range(NC_):
                    for f in range(NF):
                        nc.tensor.matmul(pout, fch[m][f][:, q * P:(q + 1) * P],
                                         wchf[m][:, f, :],
                                         start=(m == 0 and f == 0),
                                         stop=(m == NC_ - 1 and f == NF - 1))
                pouts.append(pout)
                if q == 0 and after_q0 is not None:
                    after_q0()
            return pouts

        def emit_out(b, pouts, x1, final=False):
            """residual add on DVE straight from PSUM, then store.  Emitted
            one iteration late so it never head-of-line-blocks the next
            batch's feature chain on the DVE FIFO.  For the final batch the
            stores split across queues so the tail transfer isn't bound by
            one ring's bandwidth."""
            for q in range(NT):
                ot = opool.tile([P, C], FP32, tag="out", name="out")
                nc.vector.tensor_tensor(out=ot, in0=pouts[q], in1=x1[q],
                                        op=OP.add)
                nc.vector.tensor_tensor(out=ot, in0=ot, in1=bch, op=OP.add)
                if final and q >= 2:
                    for hh in range(4):
                        nc.sync.dma_start(
                            out=y_out[b, q * P:(q + 1) * P,
                                      hh * (C // 4):(hh + 1) * (C // 4)],
                            in_=ot[:, hh * (C // 4):(hh + 1) * (C // 4)])
                else:
                    nc.sync.dma_start(out=y_out[b, q * P:(q + 1) * P, :], in_=ot)

        # ---- software-pipelined emission over batches ----
        # DMA order: X(0), wtok, [x(1)], then the big wchf -- the fill is
        # HBM-bandwidth-bound, so batch-0's dependencies go first.
        if apply_ln1:
            lnwb = singles.tile([P, 4, C], FP32, tag="lnwb")
            nc.sync.dma_start(out=lnwb, in_=ln_in.rearrange("p (k c) -> p k c", k=4))
        Xc, featsc = X0, stage1_prefetch(X0, groups=[(0,), (1,), (2, 3)])
        Xmap = {0: X0}
        if NB > 1:
            Xmap[1] = X1
        if NB > 2:
            Xmap[2] = load_x(2)
        wchf, bch, lnwb2 = load_weights_late()
        if not apply_ln1:
            lnwb = lnwb2
        ptokc = [ptokp.tile([P, C], FP32, tag="ptok", name=f"ptok{j}")
                 for j in range(NO_TOK)]
        mm1(featsc, ptokc, range(NT))
        pending_out = None
        hoisted = None   # last batch's token stage, pre-emitted an iteration early
        for b in range(NB):
            X, feats, ptok = Xc, featsc, ptokc
            # b=0: the next batch's Sin chain goes ahead of the ytok copies
            # on the ACT queue -- the copies wait for mm1(0) anyway, while
            # the sins' inputs are ready earlier (kills the ramp gaps).
            # Steady state: copies first (they gate mm2 on the PE; the sins'
            # inputs arrive early under the mm3 cover).
            if b == 0 and b + 1 < NB:
                Xc = Xmap[b + 1]
                featsc = stage1_prefetch(Xc, gsize=2)
                ytok = ytok_copies(ptok)
                if b + 2 < NB and (b + 2) not in Xmap:
                    Xmap[b + 2] = load_x(b + 2)
            elif hoisted is None:
                ytok = ytok_copies(ptok)
                if b + 1 < NB:
                    Xc = Xmap[b + 1]
                    featsc = stage1_prefetch(Xc, gsize=NT)
                    if b + 2 < NB and (b + 2) not in Xmap:
                        Xmap[b + 2] = load_x(b + 2)
            if b + 1 < NB:
                x1, xn2 = token_out(b, X, ytok)
                if pending_out is not None:
                    emit_out(b - 1, *pending_out)
                ptokc = [ptokp.tile([P, C], FP32, tag="ptok", name=f"ptok{j}")
                         for j in range(NO_TOK)]
                mm1(featsc, ptokc, (0, 1))
                tr = transposes(xn2)
                mm1(featsc, ptokc, (2, 3))
                fch = channel_feats(tr)
                if b + 1 == NB - 1:
                    # hoist the final batch's token stage (ytok, mm2, LN2,
                    # normalize) ahead of this mm3 so its serial chain runs
                    # under the 21us matmul cover; its transposes splice in
                    # behind this mm3's first output (the normalize is just
                    # done by then) and its feature chain runs under the
                    # remaining 15us of matmuls.
                    ytok_l = ytok_copies(ptokc)
                    x1_l, xn2_l = token_out(b + 1, Xc, ytok_l)
                    box = []
                    pending_out = (mm3(fch, after_q0=lambda: box.append(
                        transposes(xn2_l))), x1)
                    hoisted = (x1_l, channel_feats(box[0], pool=fpool, pref="F1"))
                else:
                    pending_out = (mm3(fch), x1)
            else:
                if hoisted is not None:
                    x1, fch = hoisted
                else:
                    x1, tr = token_out_last(b, X, ytok)
                    fch = channel_feats(tr)
                if pending_out is not None:
                    emit_out(b - 1, *pending_out)
                pending_out = (mm3(fch), x1)
        emit_out(NB - 1, *pending_out, final=True)

    _split_multi_waits(nc)
    return nc


_CACHE = {}


def _get_nc(apply_ln1, apply_ln2):
    key = (apply_ln1, apply_ln2)
    if key not in _CACHE:
        _CACHE[key] = _build(apply_ln1, apply_ln2)
    return _CACHE[key]


def prepare_in_maps(inputs):
    return _prepare(**inputs)


def _prepare(x, ln1_w, ln1_b, tok_coef, tok_kbias, tok_lw, tok_lb,
             ln2_w, ln2_b, ch_coef, ch_kbias, ch_lw, ch_lb):
    x = np.asarray(x, np.float32)
    f64 = np.float64

    wtok_eff, tok_const = _cheb_weights(np.asarray(tok_coef, f64))  # (T,6,TD)
    wch_eff, ch_const = _cheb_weights(np.asarray(ch_coef, f64))     # (C,6,2C)

    kbias_tok = np.asarray(tok_kbias, f64).reshape(-1) + tok_const
    kbias_ch = np.asarray(ch_kbias, f64).reshape(-1) + ch_const
    bias_tok = np.asarray(tok_lb, f64) + np.asarray(tok_lw, f64) @ kbias_tok
    bias_ch = np.asarray(ch_lb, f64) + np.asarray(ch_lw, f64) @ kbias_ch

    # fold the channel post-KAN linear into the KAN weights (fp64)
    wchf = np.einsum("cfo,ko->cfk", wch_eff, np.asarray(ch_lw, f64))  # (C,6,C)

    wtok_np = wtok_eff.reshape(NT, P, NF * TD).astype(ml_dtypes.bfloat16)
    wchf_np = wchf.reshape(NC_, P, NF * C).astype(ml_dtypes.bfloat16)
    tlw_np = np.ascontiguousarray(np.asarray(tok_lw, f64).T).reshape(
        NO_TOK, P, T).astype(ml_dtypes.bfloat16)
    btok_np = np.ascontiguousarray(bias_tok.reshape(NT, P).T).astype(np.float32)
    bch_np = np.broadcast_to(bias_ch.astype(ml_dtypes.bfloat16), (P, C)).copy()
    lnwb_np = np.broadcast_to(
        np.concatenate([np.asarray(ln1_w, f64), np.asarray(ln1_b, f64),
                        np.asarray(ln2_w, f64), np.asarray(ln2_b, f64)]).astype(
            np.float32), (P, 4 * C)).copy()

    apply_ln1 = not (np.all(np.asarray(ln1_w) == 1.0) and np.all(np.asarray(ln1_b) == 0.0))
    apply_ln2 = not (np.all(np.asarray(ln2_w) == 1.0) and np.all(np.asarray(ln2_b) == 0.0))

    shared = dict(wtok=wtok_np, wchf=wchf_np, tlw=tlw_np,
                  btok=btok_np, bch=bch_np, lnwb=lnwb_np)
    in_maps = []
    for core in range(NCORES):
        m = dict(shared)
        m["x"] = np.ascontiguousarray(x[core * NB:(core + 1) * NB])
        in_maps.append(m)
    return {"build_key": (apply_ln1, apply_ln2), "in_maps": in_maps}


def kernel(**inputs):
    prep = _prepare(**inputs)
    nc = _get_nc(*prep["build_key"])
    res = run_bass_kernel_spmd(nc, prep["in_maps"], list(range(NCORES)))
    return np.concatenate([res.results[i]["y"] for i in range(NCORES)], axis=0)
